# revision 11
# baseline (speedup 1.0000x reference)
"""Trainium2 Bass kernel for nn_MoREModelSynthesisIOptionB (moe_routing).

Sharding: 8 NeuronCores. Token-data-parallel for MoE/recurrent/lm_head
(core c owns token blocks {c, 15-c} of 128 tokens), head-parallel for
attention (core c owns head c; head weight slices are passed as per-core
input data so the compiled program is identical on every core).
Activations are kept transposed ([d, t]) so weight matrices serve as the
stationary matmul operand exactly as stored. Dense expert dispatch with
the one-hot top-1 combine mask applied to the gelu output before the w2
matmul (PSUM accumulates over experts). fp32 matmuls throughout.
Collectives per layer: AllGather of normed x (attention input), AllToAll
of per-head attention outputs back to token shards.
"""
import os
import sys
import numpy as np

sys.path.insert(0, "/opt/trn_rl_repo")
sys.path.insert(0, "/opt/trn_rl_repo/concourse")

from concourse import bass, bacc, tile, mybir, masks  # noqa: E402
from concourse import bass_utils  # noqa: E402
from concourse.alu_op_type import AluOpType  # noqa: E402

AF = mybir.ActivationFunctionType
F32 = mybir.dt.float32

NCORES = 8
B, S, V, D, H, L, E, NR = 2, 1024, 32000, 512, 8, 4, 8, 2
DFE = 2 * D
DFM = 4 * D
HD = D // H
T = B * S
NBLK = T // 128
TLOC = 256
DC = D // 128        # 4
FE = DFE // 128      # 8
FM = DFM // 128      # 16
ADC = 2 * D // 128   # 8
EPS = 1e-6

CORE_BLOCKS = [[c, NBLK - 1 - c] for c in range(NCORES)]
BLK_SRC = [(m, 0) if m < NCORES else (NBLK - 1 - m, 1) for m in range(NBLK)]

N_LAYERS = int(os.environ.get("KLAYERS", str(L)))
DO_HEAD = os.environ.get("KHEAD", "1") == "1"

_CACHE = {}
LAST_RESULT = None


def build():
    nc = bacc.Bacc("TRN2", target_bir_lowering=False, debug=False,
                   enable_asserts=False, num_devices=NCORES)

    def din(name, shape):
        return nc.dram_tensor(name, list(shape), F32, kind="ExternalInput")

    x0T_d = din("x0T", [D, TLOC])
    s0T_d = din("s0T", [L, D, TLOC])
    wqh_d = din("wqh", [L, D, HD])
    wkh_d = din("wkh", [L, D, HD])
    wvh_d = din("wvh", [L, D, HD])
    wo_d = din("wo", [L, D, D])
    anw_d = din("anw", [L, 2, D])
    rtr_d = din("rtr", [L, D, E])
    ew1_d = din("ew1", [L, E, D, DFE])
    ew2_d = din("ew2", [L, E, DFE, D])
    srad_d = din("srad", [L, 2 * D, D])
    srwv_d = din("srwv", [L, D, D])
    srwo_d = din("srwo", [L, D, D])
    srm1_d = din("srm1", [L, D, DFM])
    srm2_d = din("srm2", [L, DFM, D])
    srnw_d = din("srnw", [L, 4, D])
    onw_d = din("onw", [D])
    lmh_d = din("lmh", [D, V])
    tri_d = din("tri", [128, 128])

    logits_d = nc.dram_tensor("logits", [TLOC, V], F32, kind="ExternalOutput")
    aux_d = nc.dram_tensor("aux", [L, 2, E], F32, kind="ExternalOutput")
    dbg_d = nc.dram_tensor("dbg", [D, TLOC], F32, kind="ExternalOutput")

    rg = [list(range(NCORES))]

    def r128(ap):
        return ap.rearrange("(a p) f -> p a f", p=128)

    with tile.TileContext(nc) as tc:
        octx = [
            tc.tile_pool(name="cpool", bufs=1),
            tc.tile_pool(name="wpool", bufs=2),
            tc.tile_pool(name="apool", bufs=1),
            tc.tile_pool(name="dram", bufs=2, space="DRAM"),
        ]
        cpool, wpool, apool, dram = [p.__enter__() for p in octx]
        try:
            ident = cpool.tile([128, 128], F32)
            masks.make_identity(nc, ident[:])
            ones_col = cpool.tile([128, 1], F32)
            nc.vector.memset(ones_col[:], 1.0)
            tri = cpool.tile([128, 128], F32)
            nc.sync.dma_start(tri[:], tri_d.ap())

            xT = cpool.tile([128, DC, TLOC], F32, name="xT")
            nc.sync.dma_start(xT[:], r128(x0T_d.ap()))

            def rmsT(dst, src, w_col, ps_r):
                sq = apool.tile([128, DC, TLOC], F32, tag="rms_sq",
                                name="rms_sq")
                for kc in range(DC):
                    nc.vector.tensor_tensor(sq[:, kc, :], src[:, kc, :],
                                            src[:, kc, :], AluOpType.mult)
                ss = ps_r.tile([1, TLOC], F32, tag="rms_ss", name="rms_ss",
                               bufs=2)
                for kc in range(DC):
                    nc.tensor.matmul(ss[:], ones_col[:], sq[:, kc, :],
                                     start=(kc == 0), stop=(kc == DC - 1))
                st = apool.tile([1, TLOC], F32, tag="rms_st", name="rms_st",
                                bufs=2)
                nc.vector.tensor_scalar(st[:], ss[:], 1.0 / D, EPS,
                                        AluOpType.mult, AluOpType.add)
                st2 = apool.tile([1, TLOC], F32, tag="rms_st2", name="rms_st2",
                                 bufs=2)
                nc.scalar.sqrt(st2[:], st[:])
                st3 = apool.tile([1, TLOC], F32, tag="rms_st3", name="rms_st3",
                                 bufs=2)
                nc.vector.reciprocal(st3[:], st2[:])
                bc = apool.tile([128, TLOC], F32, tag="rms_bc", name="rms_bc",
                                bufs=2)
                nc.gpsimd.partition_broadcast(bc[:], st3[:])
                for kc in range(DC):
                    nc.vector.scalar_tensor_tensor(
                        dst[:, kc, :], src[:, kc, :], w_col[:, kc:kc + 1],
                        bc[:], AluOpType.mult, AluOpType.mult)

            def load_wcol(dram_ap, tag):
                t = wpool.tile([128, DC], F32, tag=tag, name=tag)
                nc.sync.dma_start(t[:], dram_ap.rearrange("(a p) -> p a",
                                                          p=128))
                return t

            # ================= layers =================
            for l in range(N_LAYERS):
                anw1 = load_wcol(anw_d.ap()[l, 0], "anw1")
                anw2 = load_wcol(anw_d.ap()[l, 1], "anw2")

                # ---------- attention ----------
                with (tc.tile_pool(name="aap", bufs=1) as aap,
                      tc.tile_pool(name="ps_r", bufs=2, space="PSUM") as ps_r):
                    wqh = aap.tile([128, DC, HD], F32, tag="wqh", name="wqh")
                    nc.sync.dma_start(wqh[:], r128(wqh_d.ap()[l]))
                    wkh = aap.tile([128, DC, HD], F32, tag="wkh", name="wkh")
                    nc.sync.dma_start(wkh[:], r128(wkh_d.ap()[l]))
                    wvh = aap.tile([128, DC, HD], F32, tag="wvh", name="wvh")
                    nc.sync.dma_start(wvh[:], r128(wvh_d.ap()[l]))
                    wo = aap.tile([128, DC, D], F32, tag="wo", name="wo")
                    nc.sync.dma_start(wo[:], r128(wo_d.ap()[l]))

                    xn = aap.tile([128, DC, TLOC], F32, tag="xn", name="xn")
                    rmsT(xn[:], xT[:], anw1[:], ps_r)

                    ag_in = dram.tile([D * TLOC], F32, tag="ag_in",
                                      name="ag_in")
                    nc.sync.dma_start(
                        ag_in[:].rearrange("(a p f) -> p a f", p=128, a=DC),
                        xn[:])
                    ag_out = dram.tile([NCORES, D * TLOC], F32, tag="ag_out",
                                       name="ag_out", addr_space="Shared")
                    nc.gpsimd.collective_compute(
                        "AllGather", AluOpType.bypass, replica_groups=rg,
                        ins=[ag_in.opt()], outs=[ag_out.opt()])

                    qT = aap.tile([64, NCORES, TLOC], F32, tag="qT", name="qT")
                    kT = aap.tile([64, NCORES, TLOC], F32, tag="kT", name="kT")
                    v2 = aap.tile([128, NCORES, 2, HD + 1], F32, tag="v2",
                                  name="v2")
                    nc.vector.memset(v2[:, :, :, HD:HD + 1], 1.0)
                    with tc.tile_pool(name="ps_qk", bufs=3,
                                      space="PSUM") as ps_qk:
                        for s_ in range(NCORES):
                            xa = aap.tile([128, DC, TLOC], F32, tag="xa",
                                          name="xa", bufs=2)
                            nc.sync.dma_start(
                                xa[:],
                                ag_out[:].rearrange(
                                    "s (a p f) -> s p a f", p=128, a=DC)[s_])
                            pq = ps_qk.tile([64, TLOC], F32, tag="pqk",
                                            name="pq")
                            pk = ps_qk.tile([64, TLOC], F32, tag="pqk",
                                            name="pk")
                            for kc in range(DC):
                                nc.tensor.matmul(pq[:], wqh[:, kc, :],
                                                 xa[:, kc, :], start=(kc == 0),
                                                 stop=(kc == DC - 1))
                            for kc in range(DC):
                                nc.tensor.matmul(pk[:], wkh[:, kc, :],
                                                 xa[:, kc, :], start=(kc == 0),
                                                 stop=(kc == DC - 1))
                            nc.scalar.activation(
                                qT[:, s_, :], pq[:], AF.Copy,
                                scale=1.0 / float(np.sqrt(HD)))
                            nc.scalar.copy(kT[:, s_, :], pk[:])
                            for ti in range(2):
                                pv = ps_qk.tile([128, TLOC], F32, tag="pqk",
                                                name="pv")[:, 0:HD]
                                for kc in range(DC):
                                    nc.tensor.matmul(
                                        pv[:],
                                        xa[:, kc, ti * 128:(ti + 1) * 128],
                                        wvh[:, kc, :], start=(kc == 0),
                                        stop=(kc == DC - 1))
                                nc.scalar.copy(v2[:, s_, ti, 0:HD], pv[:])

                    ao = aap.tile([64, NCORES, TLOC], F32, tag="ao", name="ao")
                    den = aap.tile([1, NCORES, TLOC], F32, tag="den",
                                   name="den")
                    with (tc.tile_pool(name="ps_s", bufs=4,
                                       space="PSUM") as ps_s,
                          tc.tile_pool(name="ps_pv", bufs=2,
                                       space="PSUM") as ps_pv):
                        for qb in range(NBLK):
                            batch, qpos = qb // NCORES, qb % NCORES
                            qs, qh = BLK_SRC[qb]
                            q_ap = qT[:, qs, qh * 128:(qh + 1) * 128]
                            pv = ps_pv.tile([HD + 1, 128], F32, tag="pv_acc",
                                            name="pv_acc")
                            nkb = qpos + 1
                            for kb in range(nkb):
                                m = batch * NCORES + kb
                                ks, kh = BLK_SRC[m]
                                st = ps_s.tile([128, 128], F32, tag="st",
                                               name="st")
                                nc.tensor.matmul(
                                    st[:],
                                    kT[:, ks, kh * 128:(kh + 1) * 128],
                                    q_ap, start=True, stop=True)
                                if kb == nkb - 1:
                                    nc.vector.tensor_tensor(
                                        st[:], st[:], tri[:], AluOpType.add)
                                es = aap.tile([128, 128], F32, tag="es",
                                              name="es", bufs=3)
                                nc.scalar.activation(es[:], st[:], AF.Exp)
                                nc.tensor.matmul(
                                    pv[:], v2[:, ks, kh, :], es[:],
                                    start=(kb == 0), stop=(kb == nkb - 1))
                            nc.scalar.copy(
                                ao[:, qs, qh * 128:(qh + 1) * 128],
                                pv[0:HD, :])
                            nc.scalar.copy(
                                den[:, qs, qh * 128:(qh + 1) * 128],
                                pv[HD:HD + 1, :])

                    rden = aap.tile([1, NCORES, TLOC], F32, tag="rden",
                                    name="rden")
                    nc.vector.reciprocal(rden[:].opt(), den[:].opt())
                    rbc = aap.tile([64, NCORES, TLOC], F32, tag="rbc",
                                   name="rbc")
                    nc.gpsimd.partition_broadcast(
                        rbc[:].opt(), rden[:].opt(), channels=64)
                    aos = aap.tile([64, NCORES, TLOC], F32, tag="aos",
                                   name="aos")
                    nc.vector.tensor_tensor(aos[:].opt(), ao[:].opt(),
                                            rbc[:].opt(), AluOpType.mult)
                    a2a_in = dram.tile([NCORES, 64 * TLOC], F32, tag="a2a_in",
                                       name="a2a_in")
                    for s_ in range(NCORES):
                        nc.sync.dma_start(
                            a2a_in[:].rearrange("s (p f) -> s p f", p=64)[s_],
                            aos[:, s_, :])
                    a2a_out = dram.tile([NCORES, 64 * TLOC], F32,
                                        tag="a2a_out", name="a2a_out")
                    nc.gpsimd.collective_compute(
                        "AllToAll", AluOpType.bypass, replica_groups=rg,
                        ins=[a2a_in.opt()], outs=[a2a_out.opt()])
                    atT = aap.tile([128, DC, TLOC], F32, tag="atT", name="atT")
                    nc.sync.dma_start(
                        atT[:],
                        a2a_out[:].rearrange("s (p f) -> (s p) f", p=64)
                        .rearrange("(a p) f -> p a f", p=128))

                    with tc.tile_pool(name="ps_o", bufs=2,
                                      space="PSUM") as ps_o:
                        for mc in range(DC):
                            po = ps_o.tile([128, TLOC], F32, tag="po",
                                           name="po")
                            for kc in range(DC):
                                nc.tensor.matmul(
                                    po[:], wo[:, kc, mc * 128:(mc + 1) * 128],
                                    atT[:, kc, :], start=(kc == 0),
                                    stop=(kc == DC - 1))
                            nc.vector.tensor_tensor(xT[:, mc, :], xT[:, mc, :],
                                                    po[:], AluOpType.add)

                # ---------- router ----------
                xf = apool.tile([128, DC, TLOC], F32, tag="xf", name="xf")
                cwT = apool.tile([1, E, TLOC], F32, tag="cwT", name="cwT")
                with (tc.tile_pool(name="ps_l", bufs=2, space="PSUM") as ps_l,
                      tc.tile_pool(name="ps_x", bufs=3, space="PSUM") as ps_x):
                    rmsT(xf[:], xT[:], anw2[:], ps_l)
                    rtr = apool.tile([128, DC, E], F32, tag="rtr", name="rtr",
                                     bufs=2)
                    nc.sync.dma_start(rtr[:], r128(rtr_d.ap()[l]))
                    ohp = apool.tile([128, 2, E], F32, tag="ohp", name="ohp")
                    prb = apool.tile([128, 2, E], F32, tag="prb", name="prb")
                    for ti in range(2):
                        pl = ps_l.tile([128, E], F32, tag="pl", name="pl",
                                        bufs=1)
                        for kc in range(DC):
                            nc.tensor.matmul(
                                pl[:], xf[:, kc, ti * 128:(ti + 1) * 128],
                                rtr[:, kc, :], start=(kc == 0),
                                stop=(kc == DC - 1))
                        lg = apool.tile([128, E], F32, tag="lg", name="lg",
                                        bufs=2)
                        nc.vector.tensor_copy(lg[:], pl[:])
                        mx = apool.tile([128, 1], F32, tag="mx", name="mx",
                                        bufs=2)
                        nc.vector.tensor_reduce(mx[:], lg[:],
                                                mybir.AxisListType.X,
                                                AluOpType.max)
                        nc.vector.tensor_scalar(ohp[:, ti, :], lg[:], mx[:],
                                                None, AluOpType.is_equal)
                        nmx = apool.tile([128, 1], F32, tag="nmx", name="nmx",
                                         bufs=2)
                        nc.vector.tensor_scalar(nmx[:], mx[:], -1.0, None,
                                                AluOpType.mult)
                        rs = apool.tile([128, 1], F32, tag="rs", name="rs",
                                        bufs=2)
                        ex = apool.tile([128, E], F32, tag="ex", name="ex",
                                        bufs=2)
                        nc.scalar.activation(ex[:], lg[:], AF.Exp,
                                             bias=nmx[:], accum_out=rs[:])
                        rrs = apool.tile([128, 1], F32, tag="rrs", name="rrs",
                                         bufs=2)
                        nc.vector.reciprocal(rrs[:], rs[:])
                        nc.vector.tensor_scalar(prb[:, ti, :], ex[:], rrs[:],
                                                None, AluOpType.mult)
                    pf = ps_x.tile([1, E], F32, tag="pf", name="pf", bufs=1)
                    pp = ps_x.tile([1, E], F32, tag="pp", name="pp", bufs=1)
                    for ti in range(2):
                        nc.tensor.matmul(pf[:], ones_col[:], ohp[:, ti, :],
                                         start=(ti == 0), stop=(ti == 1))
                    for ti in range(2):
                        nc.tensor.matmul(pp[:], ones_col[:], prb[:, ti, :],
                                         start=(ti == 0), stop=(ti == 1))
                    auxs = apool.tile([1, 2, E], F32, tag="auxs", name="auxs")
                    nc.vector.tensor_copy(auxs[:, 0, :], pf[:])
                    nc.vector.tensor_copy(auxs[:, 1, :], pp[:])
                    nc.sync.dma_start(aux_d.ap()[l], auxs[:].opt())

                    for ti in range(2):
                        pt = ps_x.tile([E, 128], F32, tag="pt", name="pt",
                                         bufs=1)
                        nc.tensor.transpose(pt[:], ohp[:, ti, :], ident[:])
                        ptc = apool.tile([E, 128], F32, tag="ptc", name="ptc",
                                         bufs=2)
                        nc.vector.tensor_copy(ptc[:], pt[:])
                        nc.sync.dma_start(
                            cwT[0:1, :, ti * 128:(ti + 1) * 128],
                            ptc[:])

                # ---------- experts (dense, masked before w2) ----------
                eoutT = apool.tile([128, DC, TLOC], F32, tag="eoutT",
                                   name="eoutT")
                with (tc.tile_pool(name="wep", bufs=2) as wep,
                      tc.tile_pool(name="ps_h", bufs=3, space="PSUM") as ps_h,
                      tc.tile_pool(name="ps_eo", bufs=4,
                                   space="PSUM") as ps_eo):
                    eo = [ps_eo.tile([128, TLOC], F32, tag=f"eo{mc}",
                                     name=f"eo{mc}", bufs=1)
                          for mc in range(DC)]
                    for e_ in range(E):
                        w1 = wep.tile([128, DC, DFE], F32, tag="ew1",
                                      name="ew1")
                        nc.sync.dma_start(w1[:], r128(ew1_d.ap()[l, e_]))
                        w2 = wep.tile([128, FE, D], F32, tag="ew2", name="ew2")
                        nc.sync.dma_start(w2[:], r128(ew2_d.ap()[l, e_]))
                        bce = apool.tile([128, TLOC], F32, tag="bce",
                                         name="bce", bufs=2)
                        nc.gpsimd.partition_broadcast(bce[:],
                                                      cwT[0:1, e_, :])
                        for fc in range(FE):
                            ph = ps_h.tile([128, TLOC], F32, tag="ph",
                                           name="ph")
                            for kc in range(DC):
                                nc.tensor.matmul(
                                    ph[:], w1[:, kc, fc * 128:(fc + 1) * 128],
                                    xf[:, kc, :], start=(kc == 0),
                                    stop=(kc == DC - 1))
                            hr = apool.tile([128, TLOC], F32, tag="hr",
                                            name="hr", bufs=2)
                            nc.scalar.activation(hr[:], ph[:], AF.Gelu)
                            hs = apool.tile([128, TLOC], F32, tag="hs",
                                            name="hs", bufs=3)
                            nc.vector.tensor_tensor(hs[:], hr[:], bce[:],
                                                    AluOpType.mult)
                            for mc in range(DC):
                                nc.tensor.matmul(
                                    eo[mc][:],
                                    w2[:, fc, mc * 128:(mc + 1) * 128],
                                    hs[:],
                                    start=(e_ == 0 and fc == 0),
                                    stop=(e_ == E - 1 and fc == FE - 1))
                    for mc in range(DC):
                        nc.vector.tensor_copy(eoutT[:, mc, :], eo[mc][:])

                # ---------- shared recurrent ----------
                with tc.tile_pool(name="wrp", bufs=1) as wrp:
                    srad = wrp.tile([128, ADC, D], F32, tag="srad",
                                    name="srad")
                    nc.sync.dma_start(srad[:], r128(srad_d.ap()[l]))
                    srwv = wrp.tile([128, DC, D], F32, tag="srwv", name="srwv")
                    nc.sync.dma_start(srwv[:], r128(srwv_d.ap()[l]))
                    srwo = wrp.tile([128, DC, D], F32, tag="srwo", name="srwo")
                    nc.sync.dma_start(srwo[:], r128(srwo_d.ap()[l]))
                    srm1 = wrp.tile([128, DC, DFM], F32, tag="srm1",
                                    name="srm1")
                    nc.sync.dma_start(srm1[:], r128(srm1_d.ap()[l]))
                    srm2 = wrp.tile([128, FM, D], F32, tag="srm2", name="srm2")
                    nc.sync.dma_start(srm2[:], r128(srm2_d.ap()[l]))
                    n1 = load_wcol(srnw_d.ap()[l, 0], "srn1")
                    n2 = load_wcol(srnw_d.ap()[l, 1], "srn2")
                    n3 = load_wcol(srnw_d.ap()[l, 2], "srn3")
                    n4 = load_wcol(srnw_d.ap()[l, 3], "srn4")

                    state = apool.tile([128, DC, TLOC], F32, tag="state",
                                       name="state")
                    nc.sync.dma_start(state[:], r128(s0T_d.ap()[l]))

                    for r_ in range(NR):
                        with (tc.tile_pool(name="ps_a", bufs=2,
                                           space="PSUM") as ps_a,
                              tc.tile_pool(name="ps_pm", bufs=4,
                                           space="PSUM") as ps_pm,
                              tc.tile_pool(name="ps_rr", bufs=2,
                                           space="PSUM") as ps_rr):
                            s1 = apool.tile([128, DC, TLOC], F32, tag="s1",
                                            name="s1")
                            for mc in range(DC):
                                pa = ps_a.tile([128, TLOC], F32, tag="pa",
                                               name="pa")
                                for kc in range(ADC):
                                    rhs = (state[:, kc, :] if kc < DC
                                           else eoutT[:, kc - DC, :])
                                    nc.tensor.matmul(
                                        pa[:],
                                        srad[:, kc, mc * 128:(mc + 1) * 128],
                                        rhs, start=(kc == 0),
                                        stop=(kc == ADC - 1))
                                nc.scalar.copy(s1[:, mc, :], pa[:])
                            sn = apool.tile([128, DC, TLOC], F32, tag="sn",
                                            name="sn")
                            rmsT(sn[:], s1[:], n1[:], ps_rr)
                            av = apool.tile([128, DC, TLOC], F32, tag="av",
                                            name="av")
                            for mc in range(DC):
                                pa = ps_a.tile([128, TLOC], F32, tag="pa",
                                               name="pa")
                                for kc in range(DC):
                                    nc.tensor.matmul(
                                        pa[:],
                                        srwv[:, kc, mc * 128:(mc + 1) * 128],
                                        sn[:, kc, :], start=(kc == 0),
                                        stop=(kc == DC - 1))
                                nc.scalar.copy(av[:, mc, :], pa[:])
                            r2 = apool.tile([128, DC, TLOC], F32, tag="r2",
                                            name="r2")
                            for mc in range(DC):
                                pa = ps_a.tile([128, TLOC], F32, tag="pa",
                                               name="pa")
                                for kc in range(DC):
                                    nc.tensor.matmul(
                                        pa[:],
                                        srwo[:, kc, mc * 128:(mc + 1) * 128],
                                        av[:, kc, :], start=(kc == 0),
                                        stop=(kc == DC - 1))
                                nc.vector.tensor_tensor(
                                    r2[:, mc, :], s1[:, mc, :], pa[:],
                                    AluOpType.add)
                            r2n = apool.tile([128, DC, TLOC], F32, tag="r2n",
                                             name="r2n")
                            rmsT(r2n[:], r2[:], n2[:], ps_rr)
                            s3n = apool.tile([128, DC, TLOC], F32, tag="s3n",
                                             name="s3n")
                            rmsT(s3n[:], r2n[:], n3[:], ps_rr)
                            pm = [ps_pm.tile([128, TLOC], F32, tag=f"pm{mc}",
                                             name=f"pm{mc}", bufs=1)
                                  for mc in range(DC)]
                            for fc in range(FM):
                                pa = ps_a.tile([128, TLOC], F32, tag="pa",
                                               name="pa")
                                for kc in range(DC):
                                    nc.tensor.matmul(
                                        pa[:],
                                        srm1[:, kc, fc * 128:(fc + 1) * 128],
                                        s3n[:, kc, :], start=(kc == 0),
                                        stop=(kc == DC - 1))
                                ms = apool.tile([128, TLOC], F32, tag="ms",
                                                name="ms", bufs=3)
                                nc.scalar.activation(ms[:], pa[:], AF.Silu)
                                for mc in range(DC):
                                    nc.tensor.matmul(
                                        pm[mc][:],
                                        srm2[:, fc, mc * 128:(mc + 1) * 128],
                                        ms[:], start=(fc == 0),
                                        stop=(fc == FM - 1))
                            r4 = apool.tile([128, DC, TLOC], F32, tag="r4",
                                            name="r4")
                            for mc in range(DC):
                                nc.vector.tensor_tensor(
                                    r4[:, mc, :], r2n[:, mc, :], pm[mc][:],
                                    AluOpType.add)
                            stn = apool.tile([128, DC, TLOC], F32,
                                             tag="state_n", name="state_n")
                            rmsT(stn[:], r4[:], n4[:], ps_rr)
                            state = stn

                for mc in range(DC):
                    nc.vector.tensor_tensor(xT[:, mc, :], xT[:, mc, :],
                                            state[:, mc, :], AluOpType.add)

            # ---------- final norm + lm_head ----------
            nc.sync.dma_start(r128(dbg_d.ap()), xT[:])
            if DO_HEAD:
                onw = load_wcol(onw_d.ap(), "onw")
                xo = apool.tile([128, DC, TLOC], F32, tag="xo", name="xo")
                with tc.tile_pool(name="ps_f", bufs=2, space="PSUM") as ps_f:
                    rmsT(xo[:], xT[:], onw[:], ps_f)
                with (tc.tile_pool(name="wlm", bufs=3) as wlm,
                      tc.tile_pool(name="ps_lm", bufs=4,
                                   space="PSUM") as ps_lm):
                    NV = 512
                    for vs in range(0, V, NV):
                        nv = min(NV, V - vs)
                        lw = wlm.tile([128, DC, NV], F32, tag="lmw",
                                      name="lmw")
                        nc.sync.dma_start(
                            lw[:, :, 0:nv],
                            lmh_d.ap()[:, vs:vs + nv].rearrange(
                                "(a p) f -> p a f", p=128))
                        for ti in range(2):
                            pl2 = ps_lm.tile([128, NV], F32, tag="plm",
                                             name="plm")
                            for kc in range(DC):
                                nc.tensor.matmul(
                                    pl2[:, 0:nv],
                                    xo[:, kc, ti * 128:(ti + 1) * 128],
                                    lw[:, kc, 0:nv], start=(kc == 0),
                                    stop=(kc == DC - 1))
                            ot = apool.tile([128, NV], F32, tag="ot",
                                            name="ot", bufs=3)
                            nc.scalar.copy(ot[:, 0:nv], pl2[:, 0:nv])
                            nc.sync.dma_start(
                                logits_d.ap()[ti * 128:(ti + 1) * 128,
                                              vs:vs + nv],
                                ot[:, 0:nv])
        finally:
            for p in reversed(octx):
                p.__exit__(None, None, None)

    nc.compile()
    return nc


def _state_inits():
    import jax
    import jax.numpy as jnp
    key = jax.random.key(42)
    out = []
    for i in range(L):
        s = jax.random.normal(jax.random.fold_in(key, i), (T, D), jnp.float32)
        out.append(np.asarray(s) * np.float32(0.02))
    return out


def kernel(input_ids, params):
    global LAST_RESULT
    input_ids = np.asarray(input_ids)
    p = params

    if "nc" not in _CACHE:
        _CACHE["nc"] = build()
    nc = _CACHE["nc"]

    tok = np.asarray(p["tok_emb"], dtype=np.float32)
    pos = np.asarray(p["pos_emb"], dtype=np.float32)[:S]
    x0 = tok[np.asarray(input_ids).reshape(-1)].reshape(B, S, D) + pos[None]
    x0 = x0.reshape(T, D)

    s0 = _state_inits()

    tri = np.zeros((128, 128), dtype=np.float32)
    for k_ in range(128):
        tri[k_, :k_] = -1.0e30

    def f32(a):
        return np.ascontiguousarray(np.asarray(a, dtype=np.float32))

    layers = p["layers"]
    shared = {
        "wo": f32(np.stack([lp["attn_wo"] for lp in layers])),
        "anw": f32(np.stack([[lp["norm1"], lp["norm2"]] for lp in layers])),
        "rtr": f32(np.stack([lp["router"] for lp in layers])),
        "ew1": f32(np.stack([lp["exp_w1"] for lp in layers])),
        "ew2": f32(np.stack([lp["exp_w2"] for lp in layers])),
        "srad": f32(np.stack([lp["sr"]["adapter"] for lp in layers])),
        "srwv": f32(np.stack([lp["sr"]["attn_wv"] for lp in layers])),
        "srwo": f32(np.stack([lp["sr"]["attn_wo"] for lp in layers])),
        "srm1": f32(np.stack([lp["sr"]["mlp_w1"] for lp in layers])),
        "srm2": f32(np.stack([lp["sr"]["mlp_w2"] for lp in layers])),
        "srnw": f32(np.stack([[lp["sr"]["n1"], lp["sr"]["n2"],
                               lp["sr"]["n3"], lp["sr"]["n4"]]
                              for lp in layers])),
        "onw": f32(p["norm_out"]),
        "lmh": f32(p["lm_head"]),
        "tri": tri,
    }
    wq = f32(np.stack([lp["attn_wq"] for lp in layers]))
    wk = f32(np.stack([lp["attn_wk"] for lp in layers]))
    wv = f32(np.stack([lp["attn_wv"] for lp in layers]))

    in_maps = []
    for c in range(NCORES):
        rows = np.concatenate([np.arange(b_ * 128, b_ * 128 + 128)
                               for b_ in CORE_BLOCKS[c]])
        im = dict(shared)
        im["x0T"] = np.ascontiguousarray(x0[rows].T)
        im["s0T"] = np.ascontiguousarray(
            np.stack([s0[li][rows].T for li in range(L)]))
        hs = slice(c * HD, (c + 1) * HD)
        im["wqh"] = np.ascontiguousarray(wq[:, :, hs])
        im["wkh"] = np.ascontiguousarray(wk[:, :, hs])
        im["wvh"] = np.ascontiguousarray(wv[:, :, hs])
        in_maps.append(im)

    res = bass_utils.run_bass_kernel_spmd(
        nc, in_maps, core_ids=list(range(NCORES)))
    LAST_RESULT = res

    logits = np.zeros((T, V), dtype=np.float32)
    fsum = np.zeros((L, E), dtype=np.float64)
    psum = np.zeros((L, E), dtype=np.float64)
    for c in range(NCORES):
        o = res.results[c]
        lg = o["logits"]
        for j, b_ in enumerate(CORE_BLOCKS[c]):
            logits[b_ * 128:(b_ + 1) * 128] = lg[j * 128:(j + 1) * 128]
        fsum += o["aux"][:, 0, :]
        psum += o["aux"][:, 1, :]

    total_aux = np.float32(0.0)
    for li in range(L):
        f = (fsum[li] / T).astype(np.float32)
        pr = (psum[li] / T).astype(np.float32)
        total_aux = np.float32(total_aux + np.float32(E) *
                               np.float32(np.sum(f * pr, dtype=np.float32)))
    return logits.reshape(B, S, V), total_aux


# revision 12
# speedup vs baseline: 1.0732x; 1.0732x over previous
"""Trainium2 Bass kernel for nn_MoREModelSynthesisIOptionB (moe_routing).

Sharding: 8 NeuronCores. Token-data-parallel for MoE/recurrent/lm_head
(core c owns token blocks {c, 15-c} of 128 tokens), head-parallel for
attention (core c owns head c; head weight slices are passed as per-core
input data so the compiled program is identical on every core).
Activations are kept transposed ([d, t]) so weight matrices serve as the
stationary matmul operand exactly as stored. Dense expert dispatch with
the one-hot top-1 combine mask applied to the gelu output before the w2
matmul (PSUM accumulates over experts). fp32 matmuls throughout.
Collectives per layer: AllGather of normed x (attention input), AllToAll
of per-head attention outputs back to token shards.
"""
import os
import sys
import numpy as np

sys.path.insert(0, "/opt/trn_rl_repo")
sys.path.insert(0, "/opt/trn_rl_repo/concourse")

from concourse import bass, bacc, tile, mybir, masks  # noqa: E402
from concourse import bass_utils  # noqa: E402
from concourse.alu_op_type import AluOpType  # noqa: E402

AF = mybir.ActivationFunctionType
F32 = mybir.dt.float32

NCORES = 8
B, S, V, D, H, L, E, NR = 2, 1024, 32000, 512, 8, 4, 8, 2
DFE = 2 * D
DFM = 4 * D
HD = D // H
T = B * S
NBLK = T // 128
TLOC = 256
DC = D // 128        # 4
FE = DFE // 128      # 8
FM = DFM // 128      # 16
ADC = 2 * D // 128   # 8
EPS = 1e-6

CORE_BLOCKS = [[c, NBLK - 1 - c] for c in range(NCORES)]
BLK_SRC = [(m, 0) if m < NCORES else (NBLK - 1 - m, 1) for m in range(NBLK)]

N_LAYERS = int(os.environ.get("KLAYERS", str(L)))
DO_HEAD = os.environ.get("KHEAD", "1") == "1"

_CACHE = {}
LAST_RESULT = None


def build():
    nc = bacc.Bacc("TRN2", target_bir_lowering=False, debug=False,
                   enable_asserts=False, num_devices=NCORES)

    def din(name, shape):
        return nc.dram_tensor(name, list(shape), F32, kind="ExternalInput")

    x0T_d = din("x0T", [D, TLOC])
    s0T_d = din("s0T", [L, D, TLOC])
    wqh_d = din("wqh", [L, D, HD])
    wkh_d = din("wkh", [L, D, HD])
    wvh_d = din("wvh", [L, D, HD])
    wo_d = din("wo", [L, D, D])
    anw_d = din("anw", [L, 2, D])
    rtr_d = din("rtr", [L, D, E])
    ew1_d = din("ew1", [L, E, D, DFE])
    ew2_d = din("ew2", [L, E, DFE, D])
    srad_d = din("srad", [L, 2 * D, D])
    srwv_d = din("srwv", [L, D, D])
    srwo_d = din("srwo", [L, D, D])
    srm1_d = din("srm1", [L, D, DFM])
    srm2_d = din("srm2", [L, DFM, D])
    srnw_d = din("srnw", [L, 4, D])
    onw_d = din("onw", [D])
    lmh_d = nc.dram_tensor("lmh", [D, V], mybir.dt.float16,
                           kind="ExternalInput")
    tri_d = din("tri", [128, 128])

    logits_d = nc.dram_tensor("logits", [TLOC, V], F32, kind="ExternalOutput")
    aux_d = nc.dram_tensor("aux", [L, 2, E], F32, kind="ExternalOutput")
    dbg_d = nc.dram_tensor("dbg", [D, TLOC], F32, kind="ExternalOutput")

    rg = [list(range(NCORES))]

    def r128(ap):
        return ap.rearrange("(a p) f -> p a f", p=128)

    with tile.TileContext(nc) as tc:
        octx = [
            tc.tile_pool(name="cpool", bufs=1),
            tc.tile_pool(name="wpool", bufs=2),
            tc.tile_pool(name="apool", bufs=1),
            tc.tile_pool(name="dram", bufs=2, space="DRAM"),
        ]
        cpool, wpool, apool, dram = [p.__enter__() for p in octx]
        try:
            ident = cpool.tile([128, 128], F32)
            masks.make_identity(nc, ident[:])
            ones_col = cpool.tile([128, 1], F32)
            nc.vector.memset(ones_col[:], 1.0)
            tri = cpool.tile([128, 128], F32)
            nc.sync.dma_start(tri[:], tri_d.ap())

            xT = cpool.tile([128, DC, TLOC], F32, name="xT")
            nc.sync.dma_start(xT[:], r128(x0T_d.ap()))

            def rmsT(dst, src, w_col, ps_r):
                sq = apool.tile([128, DC, TLOC], F32, tag="rms_sq",
                                name="rms_sq")
                for kc in range(DC):
                    nc.vector.tensor_tensor(sq[:, kc, :], src[:, kc, :],
                                            src[:, kc, :], AluOpType.mult)
                ss = ps_r.tile([1, TLOC], F32, tag="rms_ss", name="rms_ss",
                               bufs=2)
                for kc in range(DC):
                    nc.tensor.matmul(ss[:], ones_col[:], sq[:, kc, :],
                                     start=(kc == 0), stop=(kc == DC - 1))
                st = apool.tile([1, TLOC], F32, tag="rms_st", name="rms_st",
                                bufs=2)
                nc.vector.tensor_scalar(st[:], ss[:], 1.0 / D, EPS,
                                        AluOpType.mult, AluOpType.add)
                st2 = apool.tile([1, TLOC], F32, tag="rms_st2", name="rms_st2",
                                 bufs=2)
                nc.scalar.sqrt(st2[:], st[:])
                st3 = apool.tile([1, TLOC], F32, tag="rms_st3", name="rms_st3",
                                 bufs=2)
                nc.vector.reciprocal(st3[:], st2[:])
                bc = apool.tile([128, TLOC], F32, tag="rms_bc", name="rms_bc",
                                bufs=2)
                nc.gpsimd.partition_broadcast(bc[:], st3[:])
                for kc in range(DC):
                    nc.vector.scalar_tensor_tensor(
                        dst[:, kc, :], src[:, kc, :], w_col[:, kc:kc + 1],
                        bc[:], AluOpType.mult, AluOpType.mult)

            def load_wcol(dram_ap, tag):
                t = wpool.tile([128, DC], F32, tag=tag, name=tag)
                nc.sync.dma_start(t[:], dram_ap.rearrange("(a p) -> p a",
                                                          p=128))
                return t

            # ================= layers =================
            for l in range(N_LAYERS):
                anw1 = load_wcol(anw_d.ap()[l, 0], "anw1")
                anw2 = load_wcol(anw_d.ap()[l, 1], "anw2")

                # ---------- attention ----------
                with (tc.tile_pool(name="aap", bufs=1) as aap,
                      tc.tile_pool(name="ps_r", bufs=2, space="PSUM") as ps_r):
                    wqh = aap.tile([128, DC, HD], F32, tag="wqh", name="wqh")
                    nc.sync.dma_start(wqh[:], r128(wqh_d.ap()[l]))
                    wkh = aap.tile([128, DC, HD], F32, tag="wkh", name="wkh")
                    nc.sync.dma_start(wkh[:], r128(wkh_d.ap()[l]))
                    wvh = aap.tile([128, DC, HD], F32, tag="wvh", name="wvh")
                    nc.sync.dma_start(wvh[:], r128(wvh_d.ap()[l]))
                    wo = aap.tile([128, DC, D], F32, tag="wo", name="wo")
                    nc.sync.dma_start(wo[:], r128(wo_d.ap()[l]))

                    xn = aap.tile([128, DC, TLOC], F32, tag="xn", name="xn")
                    rmsT(xn[:], xT[:], anw1[:], ps_r)

                    ag_in = dram.tile([D * TLOC], F32, tag="ag_in",
                                      name="ag_in")
                    nc.sync.dma_start(
                        ag_in[:].rearrange("(a p f) -> p a f", p=128, a=DC),
                        xn[:])
                    ag_out = dram.tile([NCORES, D * TLOC], F32, tag="ag_out",
                                       name="ag_out", addr_space="Shared")
                    nc.gpsimd.collective_compute(
                        "AllGather", AluOpType.bypass, replica_groups=rg,
                        ins=[ag_in.opt()], outs=[ag_out.opt()])

                    qT = aap.tile([64, NCORES, TLOC], F32, tag="qT", name="qT")
                    kT = aap.tile([64, NCORES, TLOC], F32, tag="kT", name="kT")
                    v2 = aap.tile([128, NCORES, 2, HD + 1], F32, tag="v2",
                                  name="v2")
                    nc.vector.memset(v2[:, :, :, HD:HD + 1], 1.0)
                    with tc.tile_pool(name="ps_qk", bufs=3,
                                      space="PSUM") as ps_qk:
                        for s_ in range(NCORES):
                            xa = aap.tile([128, DC, TLOC], F32, tag="xa",
                                          name="xa", bufs=2)
                            nc.sync.dma_start(
                                xa[:],
                                ag_out[:].rearrange(
                                    "s (a p f) -> s p a f", p=128, a=DC)[s_])
                            pq = ps_qk.tile([64, TLOC], F32, tag="pqk",
                                            name="pq")
                            pk = ps_qk.tile([64, TLOC], F32, tag="pqk",
                                            name="pk")
                            for kc in range(DC):
                                nc.tensor.matmul(pq[:], wqh[:, kc, :],
                                                 xa[:, kc, :], start=(kc == 0),
                                                 stop=(kc == DC - 1))
                            for kc in range(DC):
                                nc.tensor.matmul(pk[:], wkh[:, kc, :],
                                                 xa[:, kc, :], start=(kc == 0),
                                                 stop=(kc == DC - 1))
                            nc.scalar.activation(
                                qT[:, s_, :], pq[:], AF.Copy,
                                scale=1.0 / float(np.sqrt(HD)))
                            nc.scalar.copy(kT[:, s_, :], pk[:])
                            for ti in range(2):
                                pv = ps_qk.tile([128, TLOC], F32, tag="pqk",
                                                name="pv")[:, 0:HD]
                                for kc in range(DC):
                                    nc.tensor.matmul(
                                        pv[:],
                                        xa[:, kc, ti * 128:(ti + 1) * 128],
                                        wvh[:, kc, :], start=(kc == 0),
                                        stop=(kc == DC - 1))
                                nc.scalar.copy(v2[:, s_, ti, 0:HD], pv[:])

                    ao = aap.tile([64, NCORES, TLOC], F32, tag="ao", name="ao")
                    den = aap.tile([1, NCORES, TLOC], F32, tag="den",
                                   name="den")
                    with (tc.tile_pool(name="ps_s", bufs=4,
                                       space="PSUM") as ps_s,
                          tc.tile_pool(name="ps_pv", bufs=2,
                                       space="PSUM") as ps_pv):
                        for qb in range(NBLK):
                            batch, qpos = qb // NCORES, qb % NCORES
                            qs, qh = BLK_SRC[qb]
                            q_ap = qT[:, qs, qh * 128:(qh + 1) * 128]
                            pv = ps_pv.tile([HD + 1, 128], F32, tag="pv_acc",
                                            name="pv_acc")
                            nkb = qpos + 1
                            for kb in range(nkb):
                                m = batch * NCORES + kb
                                ks, kh = BLK_SRC[m]
                                st = ps_s.tile([128, 128], F32, tag="st",
                                               name="st")
                                nc.tensor.matmul(
                                    st[:],
                                    kT[:, ks, kh * 128:(kh + 1) * 128],
                                    q_ap, start=True, stop=True)
                                if kb == nkb - 1:
                                    nc.vector.tensor_tensor(
                                        st[:], st[:], tri[:], AluOpType.add)
                                es = aap.tile([128, 128], F32, tag="es",
                                              name="es", bufs=3)
                                nc.scalar.activation(es[:], st[:], AF.Exp)
                                nc.tensor.matmul(
                                    pv[:], v2[:, ks, kh, :], es[:],
                                    start=(kb == 0), stop=(kb == nkb - 1))
                            nc.scalar.copy(
                                ao[:, qs, qh * 128:(qh + 1) * 128],
                                pv[0:HD, :])
                            nc.scalar.copy(
                                den[:, qs, qh * 128:(qh + 1) * 128],
                                pv[HD:HD + 1, :])

                    rden = aap.tile([1, NCORES, TLOC], F32, tag="rden",
                                    name="rden")
                    nc.vector.reciprocal(rden[:].opt(), den[:].opt())
                    rbc = aap.tile([64, NCORES, TLOC], F32, tag="rbc",
                                   name="rbc")
                    nc.gpsimd.partition_broadcast(
                        rbc[:].opt(), rden[:].opt(), channels=64)
                    aos = aap.tile([64, NCORES, TLOC], F32, tag="aos",
                                   name="aos")
                    nc.vector.tensor_tensor(aos[:].opt(), ao[:].opt(),
                                            rbc[:].opt(), AluOpType.mult)
                    a2a_in = dram.tile([NCORES, 64 * TLOC], F32, tag="a2a_in",
                                       name="a2a_in")
                    for s_ in range(NCORES):
                        nc.sync.dma_start(
                            a2a_in[:].rearrange("s (p f) -> s p f", p=64)[s_],
                            aos[:, s_, :])
                    a2a_out = dram.tile([NCORES, 64 * TLOC], F32,
                                        tag="a2a_out", name="a2a_out")
                    nc.gpsimd.collective_compute(
                        "AllToAll", AluOpType.bypass, replica_groups=rg,
                        ins=[a2a_in.opt()], outs=[a2a_out.opt()])
                    atT = aap.tile([128, DC, TLOC], F32, tag="atT", name="atT")
                    nc.sync.dma_start(
                        atT[:],
                        a2a_out[:].rearrange("s (p f) -> (s p) f", p=64)
                        .rearrange("(a p) f -> p a f", p=128))

                    with tc.tile_pool(name="ps_o", bufs=2,
                                      space="PSUM") as ps_o:
                        for mc in range(DC):
                            po = ps_o.tile([128, TLOC], F32, tag="po",
                                           name="po")
                            for kc in range(DC):
                                nc.tensor.matmul(
                                    po[:], wo[:, kc, mc * 128:(mc + 1) * 128],
                                    atT[:, kc, :], start=(kc == 0),
                                    stop=(kc == DC - 1))
                            nc.vector.tensor_tensor(xT[:, mc, :], xT[:, mc, :],
                                                    po[:], AluOpType.add)

                # ---------- router ----------
                xf = apool.tile([128, DC, TLOC], F32, tag="xf", name="xf")
                cwT = apool.tile([1, E, TLOC], F32, tag="cwT", name="cwT")
                with (tc.tile_pool(name="ps_l", bufs=2, space="PSUM") as ps_l,
                      tc.tile_pool(name="ps_x", bufs=3, space="PSUM") as ps_x):
                    rmsT(xf[:], xT[:], anw2[:], ps_l)
                    rtr = apool.tile([128, DC, E], F32, tag="rtr", name="rtr",
                                     bufs=2)
                    nc.sync.dma_start(rtr[:], r128(rtr_d.ap()[l]))
                    ohp = apool.tile([128, 2, E], F32, tag="ohp", name="ohp")
                    prb = apool.tile([128, 2, E], F32, tag="prb", name="prb")
                    for ti in range(2):
                        pl = ps_l.tile([128, E], F32, tag="pl", name="pl",
                                        bufs=1)
                        for kc in range(DC):
                            nc.tensor.matmul(
                                pl[:], xf[:, kc, ti * 128:(ti + 1) * 128],
                                rtr[:, kc, :], start=(kc == 0),
                                stop=(kc == DC - 1))
                        lg = apool.tile([128, E], F32, tag="lg", name="lg",
                                        bufs=2)
                        nc.vector.tensor_copy(lg[:], pl[:])
                        mx = apool.tile([128, 1], F32, tag="mx", name="mx",
                                        bufs=2)
                        nc.vector.tensor_reduce(mx[:], lg[:],
                                                mybir.AxisListType.X,
                                                AluOpType.max)
                        nc.vector.tensor_scalar(ohp[:, ti, :], lg[:], mx[:],
                                                None, AluOpType.is_equal)
                        nmx = apool.tile([128, 1], F32, tag="nmx", name="nmx",
                                         bufs=2)
                        nc.vector.tensor_scalar(nmx[:], mx[:], -1.0, None,
                                                AluOpType.mult)
                        rs = apool.tile([128, 1], F32, tag="rs", name="rs",
                                        bufs=2)
                        ex = apool.tile([128, E], F32, tag="ex", name="ex",
                                        bufs=2)
                        nc.scalar.activation(ex[:], lg[:], AF.Exp,
                                             bias=nmx[:], accum_out=rs[:])
                        rrs = apool.tile([128, 1], F32, tag="rrs", name="rrs",
                                         bufs=2)
                        nc.vector.reciprocal(rrs[:], rs[:])
                        nc.vector.tensor_scalar(prb[:, ti, :], ex[:], rrs[:],
                                                None, AluOpType.mult)
                    pf = ps_x.tile([1, E], F32, tag="pf", name="pf", bufs=1)
                    pp = ps_x.tile([1, E], F32, tag="pp", name="pp", bufs=1)
                    for ti in range(2):
                        nc.tensor.matmul(pf[:], ones_col[:], ohp[:, ti, :],
                                         start=(ti == 0), stop=(ti == 1))
                    for ti in range(2):
                        nc.tensor.matmul(pp[:], ones_col[:], prb[:, ti, :],
                                         start=(ti == 0), stop=(ti == 1))
                    auxs = apool.tile([1, 2, E], F32, tag="auxs", name="auxs")
                    nc.vector.tensor_copy(auxs[:, 0, :], pf[:])
                    nc.vector.tensor_copy(auxs[:, 1, :], pp[:])
                    nc.sync.dma_start(aux_d.ap()[l], auxs[:].opt())

                    for ti in range(2):
                        pt = ps_x.tile([E, 128], F32, tag="pt", name="pt",
                                         bufs=1)
                        nc.tensor.transpose(pt[:], ohp[:, ti, :], ident[:])
                        ptc = apool.tile([E, 128], F32, tag="ptc", name="ptc",
                                         bufs=2)
                        nc.vector.tensor_copy(ptc[:], pt[:])
                        nc.sync.dma_start(
                            cwT[0:1, :, ti * 128:(ti + 1) * 128],
                            ptc[:])

                # ---------- experts (dense, masked before w2) ----------
                eoutT = apool.tile([128, DC, TLOC], F32, tag="eoutT",
                                   name="eoutT")
                with (tc.tile_pool(name="wep", bufs=2) as wep,
                      tc.tile_pool(name="ps_h", bufs=3, space="PSUM") as ps_h,
                      tc.tile_pool(name="ps_eo", bufs=4,
                                   space="PSUM") as ps_eo):
                    eo = [ps_eo.tile([128, TLOC], F32, tag=f"eo{mc}",
                                     name=f"eo{mc}", bufs=1)
                          for mc in range(DC)]
                    for e_ in range(E):
                        w1 = wep.tile([128, DC, DFE], F32, tag="ew1",
                                      name="ew1")
                        nc.sync.dma_start(w1[:], r128(ew1_d.ap()[l, e_]))
                        w2 = wep.tile([128, FE, D], F32, tag="ew2", name="ew2")
                        nc.sync.dma_start(w2[:], r128(ew2_d.ap()[l, e_]))
                        bce = apool.tile([128, TLOC], F32, tag="bce",
                                         name="bce", bufs=2)
                        nc.gpsimd.partition_broadcast(bce[:],
                                                      cwT[0:1, e_, :])
                        for fc in range(FE):
                            ph = ps_h.tile([128, TLOC], F32, tag="ph",
                                           name="ph")
                            for kc in range(DC):
                                nc.tensor.matmul(
                                    ph[:], w1[:, kc, fc * 128:(fc + 1) * 128],
                                    xf[:, kc, :], start=(kc == 0),
                                    stop=(kc == DC - 1))
                            hr = apool.tile([128, TLOC], F32, tag="hr",
                                            name="hr", bufs=2)
                            nc.scalar.activation(hr[:], ph[:], AF.Gelu)
                            hs = apool.tile([128, TLOC], F32, tag="hs",
                                            name="hs", bufs=3)
                            nc.vector.tensor_tensor(hs[:], hr[:], bce[:],
                                                    AluOpType.mult)
                            for mc in range(DC):
                                nc.tensor.matmul(
                                    eo[mc][:],
                                    w2[:, fc, mc * 128:(mc + 1) * 128],
                                    hs[:],
                                    start=(e_ == 0 and fc == 0),
                                    stop=(e_ == E - 1 and fc == FE - 1))
                    for mc in range(DC):
                        nc.vector.tensor_copy(eoutT[:, mc, :], eo[mc][:])

                # ---------- shared recurrent ----------
                with tc.tile_pool(name="wrp", bufs=1) as wrp:
                    srad = wrp.tile([128, ADC, D], F32, tag="srad",
                                    name="srad")
                    nc.sync.dma_start(srad[:], r128(srad_d.ap()[l]))
                    srwv = wrp.tile([128, DC, D], F32, tag="srwv", name="srwv")
                    nc.sync.dma_start(srwv[:], r128(srwv_d.ap()[l]))
                    srwo = wrp.tile([128, DC, D], F32, tag="srwo", name="srwo")
                    nc.sync.dma_start(srwo[:], r128(srwo_d.ap()[l]))
                    srm1 = wrp.tile([128, DC, DFM], F32, tag="srm1",
                                    name="srm1")
                    nc.sync.dma_start(srm1[:], r128(srm1_d.ap()[l]))
                    srm2 = wrp.tile([128, FM, D], F32, tag="srm2", name="srm2")
                    nc.sync.dma_start(srm2[:], r128(srm2_d.ap()[l]))
                    n1 = load_wcol(srnw_d.ap()[l, 0], "srn1")
                    n2 = load_wcol(srnw_d.ap()[l, 1], "srn2")
                    n3 = load_wcol(srnw_d.ap()[l, 2], "srn3")
                    n4 = load_wcol(srnw_d.ap()[l, 3], "srn4")

                    state = apool.tile([128, DC, TLOC], F32, tag="state",
                                       name="state")
                    nc.sync.dma_start(state[:], r128(s0T_d.ap()[l]))

                    for r_ in range(NR):
                        with (tc.tile_pool(name="ps_a", bufs=2,
                                           space="PSUM") as ps_a,
                              tc.tile_pool(name="ps_pm", bufs=4,
                                           space="PSUM") as ps_pm,
                              tc.tile_pool(name="ps_rr", bufs=2,
                                           space="PSUM") as ps_rr):
                            s1 = apool.tile([128, DC, TLOC], F32, tag="s1",
                                            name="s1")
                            for mc in range(DC):
                                pa = ps_a.tile([128, TLOC], F32, tag="pa",
                                               name="pa")
                                for kc in range(ADC):
                                    rhs = (state[:, kc, :] if kc < DC
                                           else eoutT[:, kc - DC, :])
                                    nc.tensor.matmul(
                                        pa[:],
                                        srad[:, kc, mc * 128:(mc + 1) * 128],
                                        rhs, start=(kc == 0),
                                        stop=(kc == ADC - 1))
                                nc.scalar.copy(s1[:, mc, :], pa[:])
                            sn = apool.tile([128, DC, TLOC], F32, tag="sn",
                                            name="sn")
                            rmsT(sn[:], s1[:], n1[:], ps_rr)
                            av = apool.tile([128, DC, TLOC], F32, tag="av",
                                            name="av")
                            for mc in range(DC):
                                pa = ps_a.tile([128, TLOC], F32, tag="pa",
                                               name="pa")
                                for kc in range(DC):
                                    nc.tensor.matmul(
                                        pa[:],
                                        srwv[:, kc, mc * 128:(mc + 1) * 128],
                                        sn[:, kc, :], start=(kc == 0),
                                        stop=(kc == DC - 1))
                                nc.scalar.copy(av[:, mc, :], pa[:])
                            r2 = apool.tile([128, DC, TLOC], F32, tag="r2",
                                            name="r2")
                            for mc in range(DC):
                                pa = ps_a.tile([128, TLOC], F32, tag="pa",
                                               name="pa")
                                for kc in range(DC):
                                    nc.tensor.matmul(
                                        pa[:],
                                        srwo[:, kc, mc * 128:(mc + 1) * 128],
                                        av[:, kc, :], start=(kc == 0),
                                        stop=(kc == DC - 1))
                                nc.vector.tensor_tensor(
                                    r2[:, mc, :], s1[:, mc, :], pa[:],
                                    AluOpType.add)
                            r2n = apool.tile([128, DC, TLOC], F32, tag="r2n",
                                             name="r2n")
                            rmsT(r2n[:], r2[:], n2[:], ps_rr)
                            s3n = apool.tile([128, DC, TLOC], F32, tag="s3n",
                                             name="s3n")
                            rmsT(s3n[:], r2n[:], n3[:], ps_rr)
                            pm = [ps_pm.tile([128, TLOC], F32, tag=f"pm{mc}",
                                             name=f"pm{mc}", bufs=1)
                                  for mc in range(DC)]
                            for fc in range(FM):
                                pa = ps_a.tile([128, TLOC], F32, tag="pa",
                                               name="pa")
                                for kc in range(DC):
                                    nc.tensor.matmul(
                                        pa[:],
                                        srm1[:, kc, fc * 128:(fc + 1) * 128],
                                        s3n[:, kc, :], start=(kc == 0),
                                        stop=(kc == DC - 1))
                                ms = apool.tile([128, TLOC], F32, tag="ms",
                                                name="ms", bufs=3)
                                nc.scalar.activation(ms[:], pa[:], AF.Silu)
                                for mc in range(DC):
                                    nc.tensor.matmul(
                                        pm[mc][:],
                                        srm2[:, fc, mc * 128:(mc + 1) * 128],
                                        ms[:], start=(fc == 0),
                                        stop=(fc == FM - 1))
                            r4 = apool.tile([128, DC, TLOC], F32, tag="r4",
                                            name="r4")
                            for mc in range(DC):
                                nc.vector.tensor_tensor(
                                    r4[:, mc, :], r2n[:, mc, :], pm[mc][:],
                                    AluOpType.add)
                            stn = apool.tile([128, DC, TLOC], F32,
                                             tag="state_n", name="state_n")
                            rmsT(stn[:], r4[:], n4[:], ps_rr)
                            state = stn

                for mc in range(DC):
                    nc.vector.tensor_tensor(xT[:, mc, :], xT[:, mc, :],
                                            state[:, mc, :], AluOpType.add)

            # ---------- final norm + lm_head ----------
            nc.sync.dma_start(r128(dbg_d.ap()), xT[:])
            if DO_HEAD:
                onw = load_wcol(onw_d.ap(), "onw")
                xo = apool.tile([128, DC, TLOC], F32, tag="xo", name="xo")
                with tc.tile_pool(name="ps_f", bufs=2, space="PSUM") as ps_f:
                    rmsT(xo[:], xT[:], onw[:], ps_f)
                xoh = apool.tile([128, DC, TLOC], mybir.dt.float16,
                                 tag="xoh", name="xoh")
                nc.vector.tensor_copy(xoh[:].opt(), xo[:].opt())
                with (tc.tile_pool(name="wlm", bufs=3) as wlm,
                      tc.tile_pool(name="ps_lm", bufs=4,
                                   space="PSUM") as ps_lm):
                    NV = 512
                    for vs in range(0, V, NV):
                        nv = min(NV, V - vs)
                        lw = wlm.tile([128, DC, NV], mybir.dt.float16,
                                      tag="lmw", name="lmw")
                        nc.sync.dma_start(
                            lw[:, :, 0:nv],
                            lmh_d.ap()[:, vs:vs + nv].rearrange(
                                "(a p) f -> p a f", p=128))
                        for ti in range(2):
                            pl2 = ps_lm.tile([128, NV], F32, tag="plm",
                                             name="plm")
                            for kc in range(DC):
                                nc.tensor.matmul(
                                    pl2[:, 0:nv],
                                    xoh[:, kc, ti * 128:(ti + 1) * 128],
                                    lw[:, kc, 0:nv], start=(kc == 0),
                                    stop=(kc == DC - 1))
                            ot = apool.tile([128, NV], F32, tag="ot",
                                            name="ot", bufs=3)
                            nc.scalar.copy(ot[:, 0:nv], pl2[:, 0:nv])
                            nc.sync.dma_start(
                                logits_d.ap()[ti * 128:(ti + 1) * 128,
                                              vs:vs + nv],
                                ot[:, 0:nv])
        finally:
            for p in reversed(octx):
                p.__exit__(None, None, None)

    nc.compile()
    return nc


def _state_inits():
    import jax
    import jax.numpy as jnp
    key = jax.random.key(42)
    out = []
    for i in range(L):
        s = jax.random.normal(jax.random.fold_in(key, i), (T, D), jnp.float32)
        out.append(np.asarray(s) * np.float32(0.02))
    return out


def kernel(input_ids, params):
    global LAST_RESULT
    input_ids = np.asarray(input_ids)
    p = params

    if "nc" not in _CACHE:
        _CACHE["nc"] = build()
    nc = _CACHE["nc"]

    tok = np.asarray(p["tok_emb"], dtype=np.float32)
    pos = np.asarray(p["pos_emb"], dtype=np.float32)[:S]
    x0 = tok[np.asarray(input_ids).reshape(-1)].reshape(B, S, D) + pos[None]
    x0 = x0.reshape(T, D)

    s0 = _state_inits()

    tri = np.zeros((128, 128), dtype=np.float32)
    for k_ in range(128):
        tri[k_, :k_] = -1.0e30

    def f32(a):
        return np.ascontiguousarray(np.asarray(a, dtype=np.float32))

    layers = p["layers"]
    shared = {
        "wo": f32(np.stack([lp["attn_wo"] for lp in layers])),
        "anw": f32(np.stack([[lp["norm1"], lp["norm2"]] for lp in layers])),
        "rtr": f32(np.stack([lp["router"] for lp in layers])),
        "ew1": f32(np.stack([lp["exp_w1"] for lp in layers])),
        "ew2": f32(np.stack([lp["exp_w2"] for lp in layers])),
        "srad": f32(np.stack([lp["sr"]["adapter"] for lp in layers])),
        "srwv": f32(np.stack([lp["sr"]["attn_wv"] for lp in layers])),
        "srwo": f32(np.stack([lp["sr"]["attn_wo"] for lp in layers])),
        "srm1": f32(np.stack([lp["sr"]["mlp_w1"] for lp in layers])),
        "srm2": f32(np.stack([lp["sr"]["mlp_w2"] for lp in layers])),
        "srnw": f32(np.stack([[lp["sr"]["n1"], lp["sr"]["n2"],
                               lp["sr"]["n3"], lp["sr"]["n4"]]
                              for lp in layers])),
        "onw": f32(p["norm_out"]),
        "lmh": np.ascontiguousarray(
            np.asarray(p["lm_head"]).astype(np.float16)),
        "tri": tri,
    }
    wq = f32(np.stack([lp["attn_wq"] for lp in layers]))
    wk = f32(np.stack([lp["attn_wk"] for lp in layers]))
    wv = f32(np.stack([lp["attn_wv"] for lp in layers]))

    in_maps = []
    for c in range(NCORES):
        rows = np.concatenate([np.arange(b_ * 128, b_ * 128 + 128)
                               for b_ in CORE_BLOCKS[c]])
        im = dict(shared)
        im["x0T"] = np.ascontiguousarray(x0[rows].T)
        im["s0T"] = np.ascontiguousarray(
            np.stack([s0[li][rows].T for li in range(L)]))
        hs = slice(c * HD, (c + 1) * HD)
        im["wqh"] = np.ascontiguousarray(wq[:, :, hs])
        im["wkh"] = np.ascontiguousarray(wk[:, :, hs])
        im["wvh"] = np.ascontiguousarray(wv[:, :, hs])
        in_maps.append(im)

    res = bass_utils.run_bass_kernel_spmd(
        nc, in_maps, core_ids=list(range(NCORES)))
    LAST_RESULT = res

    logits = np.zeros((T, V), dtype=np.float32)
    fsum = np.zeros((L, E), dtype=np.float64)
    psum = np.zeros((L, E), dtype=np.float64)
    for c in range(NCORES):
        o = res.results[c]
        lg = o["logits"]
        for j, b_ in enumerate(CORE_BLOCKS[c]):
            logits[b_ * 128:(b_ + 1) * 128] = lg[j * 128:(j + 1) * 128]
        fsum += o["aux"][:, 0, :]
        psum += o["aux"][:, 1, :]

    total_aux = np.float32(0.0)
    for li in range(L):
        f = (fsum[li] / T).astype(np.float32)
        pr = (psum[li] / T).astype(np.float32)
        total_aux = np.float32(total_aux + np.float32(E) *
                               np.float32(np.sum(f * pr, dtype=np.float32)))
    return logits.reshape(B, S, V), total_aux


# revision 13
# speedup vs baseline: 1.1322x; 1.0550x over previous
"""Trainium2 Bass kernel for nn_MoREModelSynthesisIOptionB (moe_routing).

Sharding: 8 NeuronCores. Token-data-parallel for MoE/recurrent/lm_head
(core c owns token blocks {c, 15-c} of 128 tokens), head-parallel for
attention (core c owns head c; head weight slices are passed as per-core
input data so the compiled program is identical on every core).
Activations are kept transposed ([d, t]) so weight matrices serve as the
stationary matmul operand exactly as stored. Dense expert dispatch with
the one-hot top-1 combine mask applied to the gelu output before the w2
matmul (PSUM accumulates over experts). fp32 matmuls throughout.
Collectives per layer: AllGather of normed x (attention input), AllToAll
of per-head attention outputs back to token shards.
"""
import os
import sys
import numpy as np

sys.path.insert(0, "/opt/trn_rl_repo")
sys.path.insert(0, "/opt/trn_rl_repo/concourse")

from concourse import bass, bacc, tile, mybir, masks  # noqa: E402
from concourse import bass_utils  # noqa: E402
from concourse.alu_op_type import AluOpType  # noqa: E402

AF = mybir.ActivationFunctionType
F32 = mybir.dt.float32

NCORES = 8
B, S, V, D, H, L, E, NR = 2, 1024, 32000, 512, 8, 4, 8, 2
DFE = 2 * D
DFM = 4 * D
HD = D // H
T = B * S
NBLK = T // 128
TLOC = 256
DC = D // 128        # 4
FE = DFE // 128      # 8
FM = DFM // 128      # 16
ADC = 2 * D // 128   # 8
EPS = 1e-6

CORE_BLOCKS = [[c, NBLK - 1 - c] for c in range(NCORES)]
BLK_SRC = [(m, 0) if m < NCORES else (NBLK - 1 - m, 1) for m in range(NBLK)]

N_LAYERS = int(os.environ.get("KLAYERS", str(L)))
DO_HEAD = os.environ.get("KHEAD", "1") == "1"

_CACHE = {}
LAST_RESULT = None


def build():
    nc = bacc.Bacc("TRN2", target_bir_lowering=False, debug=False,
                   enable_asserts=False, num_devices=NCORES)

    def din(name, shape):
        return nc.dram_tensor(name, list(shape), F32, kind="ExternalInput")

    x0T_d = din("x0T", [D, TLOC])
    s0T_d = din("s0T", [L, D, TLOC])
    wqh_d = din("wqh", [L, D, HD])
    wkh_d = din("wkh", [L, D, HD])
    wvh_d = din("wvh", [L, D, HD])
    wo_d = din("wo", [L, D, D])
    anw_d = din("anw", [L, 2, D])
    rtr_d = din("rtr", [L, D, E])
    ew1_d = din("ew1", [L, E, D, DFE])
    ew2_d = din("ew2", [L, E, DFE, D])
    srad_d = din("srad", [L, 2 * D, D])
    srwv_d = din("srwv", [L, D, D])
    srwo_d = din("srwo", [L, D, D])
    srm1_d = din("srm1", [L, D, DFM])
    srm2_d = din("srm2", [L, DFM, D])
    srnw_d = din("srnw", [L, 4, D])
    onw_d = din("onw", [D])
    lmh_d = nc.dram_tensor("lmh", [D, V], mybir.dt.float16,
                           kind="ExternalInput")
    tri_d = din("tri", [128, 128])
    F16 = mybir.dt.float16
    ew1h_d = nc.dram_tensor("ew1h", [E, D, DFE], F16, kind="ExternalInput")
    ew2h_d = nc.dram_tensor("ew2h", [E, DFE, D], F16, kind="ExternalInput")

    logits_d = nc.dram_tensor("logits", [TLOC, V], F32, kind="ExternalOutput")
    aux_d = nc.dram_tensor("aux", [L, 2, E], F32, kind="ExternalOutput")
    dbg_d = nc.dram_tensor("dbg", [D, TLOC], F32, kind="ExternalOutput")

    rg = [list(range(NCORES))]

    def r128(ap):
        return ap.rearrange("(a p) f -> p a f", p=128)

    with tile.TileContext(nc) as tc:
        octx = [
            tc.tile_pool(name="cpool", bufs=1),
            tc.tile_pool(name="wpool", bufs=2),
            tc.tile_pool(name="apool", bufs=1),
            tc.tile_pool(name="dram", bufs=2, space="DRAM"),
        ]
        cpool, wpool, apool, dram = [p.__enter__() for p in octx]
        try:
            ident = cpool.tile([128, 128], F32)
            masks.make_identity(nc, ident[:])
            ones_col = cpool.tile([128, 1], F32)
            nc.vector.memset(ones_col[:], 1.0)
            tri = cpool.tile([128, 128], F32)
            nc.sync.dma_start(tri[:], tri_d.ap())

            xT = cpool.tile([128, DC, TLOC], F32, name="xT")
            nc.sync.dma_start(xT[:], r128(x0T_d.ap()))

            def rmsT(dst, src, w_col, ps_r):
                sq = apool.tile([128, DC, TLOC], F32, tag="rms_sq",
                                name="rms_sq")
                for kc in range(DC):
                    nc.vector.tensor_tensor(sq[:, kc, :], src[:, kc, :],
                                            src[:, kc, :], AluOpType.mult)
                ss = ps_r.tile([1, TLOC], F32, tag="rms_ss", name="rms_ss",
                               bufs=2)
                for kc in range(DC):
                    nc.tensor.matmul(ss[:], ones_col[:], sq[:, kc, :],
                                     start=(kc == 0), stop=(kc == DC - 1))
                st = apool.tile([1, TLOC], F32, tag="rms_st", name="rms_st",
                                bufs=2)
                nc.vector.tensor_scalar(st[:], ss[:], 1.0 / D, EPS,
                                        AluOpType.mult, AluOpType.add)
                st2 = apool.tile([1, TLOC], F32, tag="rms_st2", name="rms_st2",
                                 bufs=2)
                nc.scalar.sqrt(st2[:], st[:])
                st3 = apool.tile([1, TLOC], F32, tag="rms_st3", name="rms_st3",
                                 bufs=2)
                nc.vector.reciprocal(st3[:], st2[:])
                bc = apool.tile([128, TLOC], F32, tag="rms_bc", name="rms_bc",
                                bufs=2)
                nc.gpsimd.partition_broadcast(bc[:], st3[:])
                for kc in range(DC):
                    nc.vector.scalar_tensor_tensor(
                        dst[:, kc, :], src[:, kc, :], w_col[:, kc:kc + 1],
                        bc[:], AluOpType.mult, AluOpType.mult)

            def load_wcol(dram_ap, tag):
                t = wpool.tile([128, DC], F32, tag=tag, name=tag)
                nc.sync.dma_start(t[:], dram_ap.rearrange("(a p) -> p a",
                                                          p=128))
                return t

            # ================= layers =================
            for l in range(N_LAYERS):
                anw1 = load_wcol(anw_d.ap()[l, 0], "anw1")
                anw2 = load_wcol(anw_d.ap()[l, 1], "anw2")

                # ---------- attention ----------
                with (tc.tile_pool(name="aap", bufs=1) as aap,
                      tc.tile_pool(name="ps_r", bufs=2, space="PSUM") as ps_r):
                    wqh = aap.tile([128, DC, HD], F32, tag="wqh", name="wqh")
                    nc.sync.dma_start(wqh[:], r128(wqh_d.ap()[l]))
                    wkh = aap.tile([128, DC, HD], F32, tag="wkh", name="wkh")
                    nc.sync.dma_start(wkh[:], r128(wkh_d.ap()[l]))
                    wvh = aap.tile([128, DC, HD], F32, tag="wvh", name="wvh")
                    nc.sync.dma_start(wvh[:], r128(wvh_d.ap()[l]))
                    wo = aap.tile([128, DC, D], F32, tag="wo", name="wo")
                    nc.sync.dma_start(wo[:], r128(wo_d.ap()[l]))

                    xn = aap.tile([128, DC, TLOC], F32, tag="xn", name="xn")
                    rmsT(xn[:], xT[:], anw1[:], ps_r)

                    ag_in = dram.tile([D * TLOC], F32, tag="ag_in",
                                      name="ag_in")
                    nc.sync.dma_start(
                        ag_in[:].rearrange("(a p f) -> p a f", p=128, a=DC),
                        xn[:])
                    ag_out = dram.tile([NCORES, D * TLOC], F32, tag="ag_out",
                                       name="ag_out", addr_space="Shared")
                    nc.gpsimd.collective_compute(
                        "AllGather", AluOpType.bypass, replica_groups=rg,
                        ins=[ag_in.opt()], outs=[ag_out.opt()])

                    qT = aap.tile([64, NCORES, TLOC], F32, tag="qT", name="qT")
                    kT = aap.tile([64, NCORES, TLOC], F32, tag="kT", name="kT")
                    v2 = aap.tile([128, NCORES, 2, HD + 1], F32, tag="v2",
                                  name="v2")
                    nc.vector.memset(v2[:, :, :, HD:HD + 1], 1.0)
                    with tc.tile_pool(name="ps_qk", bufs=3,
                                      space="PSUM") as ps_qk:
                        for s_ in range(NCORES):
                            xa = aap.tile([128, DC, TLOC], F32, tag="xa",
                                          name="xa", bufs=2)
                            nc.sync.dma_start(
                                xa[:],
                                ag_out[:].rearrange(
                                    "s (a p f) -> s p a f", p=128, a=DC)[s_])
                            pq = ps_qk.tile([64, TLOC], F32, tag="pqk",
                                            name="pq")
                            pk = ps_qk.tile([64, TLOC], F32, tag="pqk",
                                            name="pk")
                            for kc in range(DC):
                                nc.tensor.matmul(pq[:], wqh[:, kc, :],
                                                 xa[:, kc, :], start=(kc == 0),
                                                 stop=(kc == DC - 1))
                            for kc in range(DC):
                                nc.tensor.matmul(pk[:], wkh[:, kc, :],
                                                 xa[:, kc, :], start=(kc == 0),
                                                 stop=(kc == DC - 1))
                            nc.scalar.activation(
                                qT[:, s_, :], pq[:], AF.Copy,
                                scale=1.0 / float(np.sqrt(HD)))
                            nc.scalar.copy(kT[:, s_, :], pk[:])
                            for ti in range(2):
                                pv = ps_qk.tile([128, TLOC], F32, tag="pqk",
                                                name="pv")[:, 0:HD]
                                for kc in range(DC):
                                    nc.tensor.matmul(
                                        pv[:],
                                        xa[:, kc, ti * 128:(ti + 1) * 128],
                                        wvh[:, kc, :], start=(kc == 0),
                                        stop=(kc == DC - 1))
                                nc.scalar.copy(v2[:, s_, ti, 0:HD], pv[:])

                    ao = aap.tile([64, NCORES, TLOC], F32, tag="ao", name="ao")
                    den = aap.tile([1, NCORES, TLOC], F32, tag="den",
                                   name="den")
                    with (tc.tile_pool(name="ps_s", bufs=4,
                                       space="PSUM") as ps_s,
                          tc.tile_pool(name="ps_pv", bufs=2,
                                       space="PSUM") as ps_pv):
                        for qb in range(NBLK):
                            batch, qpos = qb // NCORES, qb % NCORES
                            qs, qh = BLK_SRC[qb]
                            q_ap = qT[:, qs, qh * 128:(qh + 1) * 128]
                            pv = ps_pv.tile([HD + 1, 128], F32, tag="pv_acc",
                                            name="pv_acc")
                            nkb = qpos + 1
                            for kb in range(nkb):
                                m = batch * NCORES + kb
                                ks, kh = BLK_SRC[m]
                                st = ps_s.tile([128, 128], F32, tag="st",
                                               name="st")
                                nc.tensor.matmul(
                                    st[:],
                                    kT[:, ks, kh * 128:(kh + 1) * 128],
                                    q_ap, start=True, stop=True)
                                if kb == nkb - 1:
                                    nc.vector.tensor_tensor(
                                        st[:], st[:], tri[:], AluOpType.add)
                                es = aap.tile([128, 128], F32, tag="es",
                                              name="es", bufs=3)
                                nc.scalar.activation(es[:], st[:], AF.Exp)
                                nc.tensor.matmul(
                                    pv[:], v2[:, ks, kh, :], es[:],
                                    start=(kb == 0), stop=(kb == nkb - 1))
                            nc.scalar.copy(
                                ao[:, qs, qh * 128:(qh + 1) * 128],
                                pv[0:HD, :])
                            nc.scalar.copy(
                                den[:, qs, qh * 128:(qh + 1) * 128],
                                pv[HD:HD + 1, :])

                    rden = aap.tile([1, NCORES, TLOC], F32, tag="rden",
                                    name="rden")
                    nc.vector.reciprocal(rden[:].opt(), den[:].opt())
                    rbc = aap.tile([64, NCORES, TLOC], F32, tag="rbc",
                                   name="rbc")
                    nc.gpsimd.partition_broadcast(
                        rbc[:].opt(), rden[:].opt(), channels=64)
                    aos = aap.tile([64, NCORES, TLOC], F32, tag="aos",
                                   name="aos")
                    nc.vector.tensor_tensor(aos[:].opt(), ao[:].opt(),
                                            rbc[:].opt(), AluOpType.mult)
                    a2a_in = dram.tile([NCORES, 64 * TLOC], F32, tag="a2a_in",
                                       name="a2a_in")
                    for s_ in range(NCORES):
                        nc.sync.dma_start(
                            a2a_in[:].rearrange("s (p f) -> s p f", p=64)[s_],
                            aos[:, s_, :])
                    a2a_out = dram.tile([NCORES, 64 * TLOC], F32,
                                        tag="a2a_out", name="a2a_out")
                    nc.gpsimd.collective_compute(
                        "AllToAll", AluOpType.bypass, replica_groups=rg,
                        ins=[a2a_in.opt()], outs=[a2a_out.opt()])
                    atT = aap.tile([128, DC, TLOC], F32, tag="atT", name="atT")
                    nc.sync.dma_start(
                        atT[:],
                        a2a_out[:].rearrange("s (p f) -> (s p) f", p=64)
                        .rearrange("(a p) f -> p a f", p=128))

                    with tc.tile_pool(name="ps_o", bufs=2,
                                      space="PSUM") as ps_o:
                        for mc in range(DC):
                            po = ps_o.tile([128, TLOC], F32, tag="po",
                                           name="po")
                            for kc in range(DC):
                                nc.tensor.matmul(
                                    po[:], wo[:, kc, mc * 128:(mc + 1) * 128],
                                    atT[:, kc, :], start=(kc == 0),
                                    stop=(kc == DC - 1))
                            nc.vector.tensor_tensor(xT[:, mc, :], xT[:, mc, :],
                                                    po[:], AluOpType.add)

                # ---------- router ----------
                xf = apool.tile([128, DC, TLOC], F32, tag="xf", name="xf")
                cwT = apool.tile([1, E, TLOC], F32, tag="cwT", name="cwT")
                with (tc.tile_pool(name="ps_l", bufs=2, space="PSUM") as ps_l,
                      tc.tile_pool(name="ps_x", bufs=3, space="PSUM") as ps_x):
                    rmsT(xf[:], xT[:], anw2[:], ps_l)
                    rtr = apool.tile([128, DC, E], F32, tag="rtr", name="rtr",
                                     bufs=2)
                    nc.sync.dma_start(rtr[:], r128(rtr_d.ap()[l]))
                    ohp = apool.tile([128, 2, E], F32, tag="ohp", name="ohp")
                    prb = apool.tile([128, 2, E], F32, tag="prb", name="prb")
                    for ti in range(2):
                        pl = ps_l.tile([128, E], F32, tag="pl", name="pl",
                                        bufs=1)
                        for kc in range(DC):
                            nc.tensor.matmul(
                                pl[:], xf[:, kc, ti * 128:(ti + 1) * 128],
                                rtr[:, kc, :], start=(kc == 0),
                                stop=(kc == DC - 1))
                        lg = apool.tile([128, E], F32, tag="lg", name="lg",
                                        bufs=2)
                        nc.vector.tensor_copy(lg[:], pl[:])
                        mx = apool.tile([128, 1], F32, tag="mx", name="mx",
                                        bufs=2)
                        nc.vector.tensor_reduce(mx[:], lg[:],
                                                mybir.AxisListType.X,
                                                AluOpType.max)
                        nc.vector.tensor_scalar(ohp[:, ti, :], lg[:], mx[:],
                                                None, AluOpType.is_equal)
                        nmx = apool.tile([128, 1], F32, tag="nmx", name="nmx",
                                         bufs=2)
                        nc.vector.tensor_scalar(nmx[:], mx[:], -1.0, None,
                                                AluOpType.mult)
                        rs = apool.tile([128, 1], F32, tag="rs", name="rs",
                                        bufs=2)
                        ex = apool.tile([128, E], F32, tag="ex", name="ex",
                                        bufs=2)
                        nc.scalar.activation(ex[:], lg[:], AF.Exp,
                                             bias=nmx[:], accum_out=rs[:])
                        rrs = apool.tile([128, 1], F32, tag="rrs", name="rrs",
                                         bufs=2)
                        nc.vector.reciprocal(rrs[:], rs[:])
                        nc.vector.tensor_scalar(prb[:, ti, :], ex[:], rrs[:],
                                                None, AluOpType.mult)
                    pf = ps_x.tile([1, E], F32, tag="pf", name="pf", bufs=1)
                    pp = ps_x.tile([1, E], F32, tag="pp", name="pp", bufs=1)
                    for ti in range(2):
                        nc.tensor.matmul(pf[:], ones_col[:], ohp[:, ti, :],
                                         start=(ti == 0), stop=(ti == 1))
                    for ti in range(2):
                        nc.tensor.matmul(pp[:], ones_col[:], prb[:, ti, :],
                                         start=(ti == 0), stop=(ti == 1))
                    auxs = apool.tile([1, 2, E], F32, tag="auxs", name="auxs")
                    nc.vector.tensor_copy(auxs[:, 0, :], pf[:])
                    nc.vector.tensor_copy(auxs[:, 1, :], pp[:])
                    nc.sync.dma_start(aux_d.ap()[l], auxs[:].opt())

                    for ti in range(2):
                        pt = ps_x.tile([E, 128], F32, tag="pt", name="pt",
                                         bufs=1)
                        nc.tensor.transpose(pt[:], ohp[:, ti, :], ident[:])
                        ptc = apool.tile([E, 128], F32, tag="ptc", name="ptc",
                                         bufs=2)
                        nc.vector.tensor_copy(ptc[:], pt[:])
                        nc.sync.dma_start(
                            cwT[0:1, :, ti * 128:(ti + 1) * 128],
                            ptc[:])

                # ---------- experts (dense, masked before w2) ----------
                eoutT = apool.tile([128, DC, TLOC], F32, tag="eoutT",
                                   name="eoutT")
                fp16x = (l == L - 1)
                edt = mybir.dt.float16 if fp16x else F32
                with (tc.tile_pool(name="wep", bufs=2) as wep,
                      tc.tile_pool(name="ps_h", bufs=3, space="PSUM") as ps_h,
                      tc.tile_pool(name="ps_eo", bufs=4,
                                   space="PSUM") as ps_eo):
                    if fp16x:
                        xfh = apool.tile([128, DC, TLOC], edt, tag="xfh",
                                         name="xfh")
                        nc.vector.tensor_copy(xfh[:].opt(), xf[:].opt())
                    else:
                        xfh = xf
                    eo = [ps_eo.tile([128, TLOC], F32, tag=f"eo{mc}",
                                     name=f"eo{mc}", bufs=1)
                          for mc in range(DC)]
                    for e_ in range(E):
                        w1 = wep.tile([128, DC, DFE], edt, tag="ew1",
                                      name="ew1")
                        nc.sync.dma_start(
                            w1[:], r128(ew1h_d.ap()[e_]) if fp16x
                            else r128(ew1_d.ap()[l, e_]))
                        w2 = wep.tile([128, FE, D], edt, tag="ew2", name="ew2")
                        nc.sync.dma_start(
                            w2[:], r128(ew2h_d.ap()[e_]) if fp16x
                            else r128(ew2_d.ap()[l, e_]))
                        bce = apool.tile([128, TLOC], F32, tag="bce",
                                         name="bce", bufs=2)
                        nc.gpsimd.partition_broadcast(bce[:],
                                                      cwT[0:1, e_, :])
                        for fc in range(FE):
                            ph = ps_h.tile([128, TLOC], F32, tag="ph",
                                           name="ph")
                            for kc in range(DC):
                                nc.tensor.matmul(
                                    ph[:], w1[:, kc, fc * 128:(fc + 1) * 128],
                                    xfh[:, kc, :], start=(kc == 0),
                                    stop=(kc == DC - 1))
                            hr = apool.tile([128, TLOC], F32, tag="hr",
                                            name="hr", bufs=2)
                            nc.scalar.activation(hr[:], ph[:], AF.Gelu)
                            hs = apool.tile([128, TLOC], edt, tag="hs",
                                            name="hs", bufs=3)
                            nc.vector.tensor_tensor(hs[:], hr[:], bce[:],
                                                    AluOpType.mult)
                            for mc in range(DC):
                                nc.tensor.matmul(
                                    eo[mc][:],
                                    w2[:, fc, mc * 128:(mc + 1) * 128],
                                    hs[:],
                                    start=(e_ == 0 and fc == 0),
                                    stop=(e_ == E - 1 and fc == FE - 1))
                    for mc in range(DC):
                        nc.vector.tensor_copy(eoutT[:, mc, :], eo[mc][:])

                # ---------- shared recurrent ----------
                with tc.tile_pool(name="wrp", bufs=1) as wrp:
                    srad = wrp.tile([128, ADC, D], F32, tag="srad",
                                    name="srad")
                    nc.sync.dma_start(srad[:], r128(srad_d.ap()[l]))
                    srwv = wrp.tile([128, DC, D], F32, tag="srwv", name="srwv")
                    nc.sync.dma_start(srwv[:], r128(srwv_d.ap()[l]))
                    srwo = wrp.tile([128, DC, D], F32, tag="srwo", name="srwo")
                    nc.sync.dma_start(srwo[:], r128(srwo_d.ap()[l]))
                    srm1 = wrp.tile([128, DC, DFM], F32, tag="srm1",
                                    name="srm1")
                    nc.sync.dma_start(srm1[:], r128(srm1_d.ap()[l]))
                    srm2 = wrp.tile([128, FM, D], F32, tag="srm2", name="srm2")
                    nc.sync.dma_start(srm2[:], r128(srm2_d.ap()[l]))
                    n1 = load_wcol(srnw_d.ap()[l, 0], "srn1")
                    n2 = load_wcol(srnw_d.ap()[l, 1], "srn2")
                    n3 = load_wcol(srnw_d.ap()[l, 2], "srn3")
                    n4 = load_wcol(srnw_d.ap()[l, 3], "srn4")

                    state = apool.tile([128, DC, TLOC], F32, tag="state",
                                       name="state")
                    nc.sync.dma_start(state[:], r128(s0T_d.ap()[l]))

                    for r_ in range(NR):
                        with (tc.tile_pool(name="ps_a", bufs=2,
                                           space="PSUM") as ps_a,
                              tc.tile_pool(name="ps_pm", bufs=4,
                                           space="PSUM") as ps_pm,
                              tc.tile_pool(name="ps_rr", bufs=2,
                                           space="PSUM") as ps_rr):
                            s1 = apool.tile([128, DC, TLOC], F32, tag="s1",
                                            name="s1")
                            for mc in range(DC):
                                pa = ps_a.tile([128, TLOC], F32, tag="pa",
                                               name="pa")
                                for kc in range(ADC):
                                    rhs = (state[:, kc, :] if kc < DC
                                           else eoutT[:, kc - DC, :])
                                    nc.tensor.matmul(
                                        pa[:],
                                        srad[:, kc, mc * 128:(mc + 1) * 128],
                                        rhs, start=(kc == 0),
                                        stop=(kc == ADC - 1))
                                nc.scalar.copy(s1[:, mc, :], pa[:])
                            sn = apool.tile([128, DC, TLOC], F32, tag="sn",
                                            name="sn")
                            rmsT(sn[:], s1[:], n1[:], ps_rr)
                            av = apool.tile([128, DC, TLOC], F32, tag="av",
                                            name="av")
                            for mc in range(DC):
                                pa = ps_a.tile([128, TLOC], F32, tag="pa",
                                               name="pa")
                                for kc in range(DC):
                                    nc.tensor.matmul(
                                        pa[:],
                                        srwv[:, kc, mc * 128:(mc + 1) * 128],
                                        sn[:, kc, :], start=(kc == 0),
                                        stop=(kc == DC - 1))
                                nc.scalar.copy(av[:, mc, :], pa[:])
                            r2 = apool.tile([128, DC, TLOC], F32, tag="r2",
                                            name="r2")
                            for mc in range(DC):
                                pa = ps_a.tile([128, TLOC], F32, tag="pa",
                                               name="pa")
                                for kc in range(DC):
                                    nc.tensor.matmul(
                                        pa[:],
                                        srwo[:, kc, mc * 128:(mc + 1) * 128],
                                        av[:, kc, :], start=(kc == 0),
                                        stop=(kc == DC - 1))
                                nc.vector.tensor_tensor(
                                    r2[:, mc, :], s1[:, mc, :], pa[:],
                                    AluOpType.add)
                            r2n = apool.tile([128, DC, TLOC], F32, tag="r2n",
                                             name="r2n")
                            rmsT(r2n[:], r2[:], n2[:], ps_rr)
                            s3n = apool.tile([128, DC, TLOC], F32, tag="s3n",
                                             name="s3n")
                            rmsT(s3n[:], r2n[:], n3[:], ps_rr)
                            pm = [ps_pm.tile([128, TLOC], F32, tag=f"pm{mc}",
                                             name=f"pm{mc}", bufs=1)
                                  for mc in range(DC)]
                            for fc in range(FM):
                                pa = ps_a.tile([128, TLOC], F32, tag="pa",
                                               name="pa")
                                for kc in range(DC):
                                    nc.tensor.matmul(
                                        pa[:],
                                        srm1[:, kc, fc * 128:(fc + 1) * 128],
                                        s3n[:, kc, :], start=(kc == 0),
                                        stop=(kc == DC - 1))
                                ms = apool.tile([128, TLOC], F32, tag="ms",
                                                name="ms", bufs=3)
                                nc.scalar.activation(ms[:], pa[:], AF.Silu)
                                for mc in range(DC):
                                    nc.tensor.matmul(
                                        pm[mc][:],
                                        srm2[:, fc, mc * 128:(mc + 1) * 128],
                                        ms[:], start=(fc == 0),
                                        stop=(fc == FM - 1))
                            r4 = apool.tile([128, DC, TLOC], F32, tag="r4",
                                            name="r4")
                            for mc in range(DC):
                                nc.vector.tensor_tensor(
                                    r4[:, mc, :], r2n[:, mc, :], pm[mc][:],
                                    AluOpType.add)
                            stn = apool.tile([128, DC, TLOC], F32,
                                             tag="state_n", name="state_n")
                            rmsT(stn[:], r4[:], n4[:], ps_rr)
                            state = stn

                for mc in range(DC):
                    nc.vector.tensor_tensor(xT[:, mc, :], xT[:, mc, :],
                                            state[:, mc, :], AluOpType.add)

            # ---------- final norm + lm_head ----------
            nc.sync.dma_start(r128(dbg_d.ap()), xT[:])
            if DO_HEAD:
                onw = load_wcol(onw_d.ap(), "onw")
                xo = apool.tile([128, DC, TLOC], F32, tag="xo", name="xo")
                with tc.tile_pool(name="ps_f", bufs=2, space="PSUM") as ps_f:
                    rmsT(xo[:], xT[:], onw[:], ps_f)
                xoh = apool.tile([128, DC, TLOC], mybir.dt.float16,
                                 tag="xoh", name="xoh")
                nc.vector.tensor_copy(xoh[:].opt(), xo[:].opt())
                with (tc.tile_pool(name="wlm", bufs=3) as wlm,
                      tc.tile_pool(name="ps_lm", bufs=4,
                                   space="PSUM") as ps_lm):
                    NV = 512
                    for vs in range(0, V, NV):
                        nv = min(NV, V - vs)
                        lw = wlm.tile([128, DC, NV], mybir.dt.float16,
                                      tag="lmw", name="lmw")
                        nc.sync.dma_start(
                            lw[:, :, 0:nv],
                            lmh_d.ap()[:, vs:vs + nv].rearrange(
                                "(a p) f -> p a f", p=128))
                        for ti in range(2):
                            pl2 = ps_lm.tile([128, NV], F32, tag="plm",
                                             name="plm")
                            for kc in range(DC):
                                nc.tensor.matmul(
                                    pl2[:, 0:nv],
                                    xoh[:, kc, ti * 128:(ti + 1) * 128],
                                    lw[:, kc, 0:nv], start=(kc == 0),
                                    stop=(kc == DC - 1))
                            ot = apool.tile([128, NV], F32, tag="ot",
                                            name="ot", bufs=3)
                            nc.scalar.copy(ot[:, 0:nv], pl2[:, 0:nv])
                            nc.sync.dma_start(
                                logits_d.ap()[ti * 128:(ti + 1) * 128,
                                              vs:vs + nv],
                                ot[:, 0:nv])
        finally:
            for p in reversed(octx):
                p.__exit__(None, None, None)

    nc.compile()
    return nc


def _state_inits():
    import jax
    import jax.numpy as jnp
    key = jax.random.key(42)
    out = []
    for i in range(L):
        s = jax.random.normal(jax.random.fold_in(key, i), (T, D), jnp.float32)
        out.append(np.asarray(s) * np.float32(0.02))
    return out


def kernel(input_ids, params):
    global LAST_RESULT
    input_ids = np.asarray(input_ids)
    p = params

    if "nc" not in _CACHE:
        _CACHE["nc"] = build()
    nc = _CACHE["nc"]

    tok = np.asarray(p["tok_emb"], dtype=np.float32)
    pos = np.asarray(p["pos_emb"], dtype=np.float32)[:S]
    x0 = tok[np.asarray(input_ids).reshape(-1)].reshape(B, S, D) + pos[None]
    x0 = x0.reshape(T, D)

    s0 = _state_inits()

    tri = np.zeros((128, 128), dtype=np.float32)
    for k_ in range(128):
        tri[k_, :k_] = -1.0e30

    def f32(a):
        return np.ascontiguousarray(np.asarray(a, dtype=np.float32))

    layers = p["layers"]
    shared = {
        "wo": f32(np.stack([lp["attn_wo"] for lp in layers])),
        "anw": f32(np.stack([[lp["norm1"], lp["norm2"]] for lp in layers])),
        "rtr": f32(np.stack([lp["router"] for lp in layers])),
        "ew1": f32(np.stack([lp["exp_w1"] for lp in layers])),
        "ew2": f32(np.stack([lp["exp_w2"] for lp in layers])),
        "srad": f32(np.stack([lp["sr"]["adapter"] for lp in layers])),
        "srwv": f32(np.stack([lp["sr"]["attn_wv"] for lp in layers])),
        "srwo": f32(np.stack([lp["sr"]["attn_wo"] for lp in layers])),
        "srm1": f32(np.stack([lp["sr"]["mlp_w1"] for lp in layers])),
        "srm2": f32(np.stack([lp["sr"]["mlp_w2"] for lp in layers])),
        "srnw": f32(np.stack([[lp["sr"]["n1"], lp["sr"]["n2"],
                               lp["sr"]["n3"], lp["sr"]["n4"]]
                              for lp in layers])),
        "onw": f32(p["norm_out"]),
        "lmh": np.ascontiguousarray(
            np.asarray(p["lm_head"]).astype(np.float16)),
        "tri": tri,
        "ew1h": np.ascontiguousarray(
            np.asarray(layers[L - 1]["exp_w1"]).astype(np.float16)),
        "ew2h": np.ascontiguousarray(
            np.asarray(layers[L - 1]["exp_w2"]).astype(np.float16)),
    }
    wq = f32(np.stack([lp["attn_wq"] for lp in layers]))
    wk = f32(np.stack([lp["attn_wk"] for lp in layers]))
    wv = f32(np.stack([lp["attn_wv"] for lp in layers]))

    in_maps = []
    for c in range(NCORES):
        rows = np.concatenate([np.arange(b_ * 128, b_ * 128 + 128)
                               for b_ in CORE_BLOCKS[c]])
        im = dict(shared)
        im["x0T"] = np.ascontiguousarray(x0[rows].T)
        im["s0T"] = np.ascontiguousarray(
            np.stack([s0[li][rows].T for li in range(L)]))
        hs = slice(c * HD, (c + 1) * HD)
        im["wqh"] = np.ascontiguousarray(wq[:, :, hs])
        im["wkh"] = np.ascontiguousarray(wk[:, :, hs])
        im["wvh"] = np.ascontiguousarray(wv[:, :, hs])
        in_maps.append(im)

    res = bass_utils.run_bass_kernel_spmd(
        nc, in_maps, core_ids=list(range(NCORES)))
    LAST_RESULT = res

    logits = np.zeros((T, V), dtype=np.float32)
    fsum = np.zeros((L, E), dtype=np.float64)
    psum = np.zeros((L, E), dtype=np.float64)
    for c in range(NCORES):
        o = res.results[c]
        lg = o["logits"]
        for j, b_ in enumerate(CORE_BLOCKS[c]):
            logits[b_ * 128:(b_ + 1) * 128] = lg[j * 128:(j + 1) * 128]
        fsum += o["aux"][:, 0, :]
        psum += o["aux"][:, 1, :]

    total_aux = np.float32(0.0)
    for li in range(L):
        f = (fsum[li] / T).astype(np.float32)
        pr = (psum[li] / T).astype(np.float32)
        total_aux = np.float32(total_aux + np.float32(E) *
                               np.float32(np.sum(f * pr, dtype=np.float32)))
    return logits.reshape(B, S, V), total_aux


# revision 14
# speedup vs baseline: 1.1978x; 1.0580x over previous
"""Trainium2 Bass kernel for nn_MoREModelSynthesisIOptionB (moe_routing).

Sharding: 8 NeuronCores. Token-data-parallel for MoE/recurrent/lm_head
(core c owns token blocks {c, 15-c} of 128 tokens), head-parallel for
attention (core c owns head c; head weight slices are passed as per-core
input data so the compiled program is identical on every core).
Activations are kept transposed ([d, t]) so weight matrices serve as the
stationary matmul operand exactly as stored. Dense expert dispatch with
the one-hot top-1 combine mask applied to the gelu output before the w2
matmul (PSUM accumulates over experts). fp32 matmuls throughout.
Collectives per layer: AllGather of normed x (attention input), AllToAll
of per-head attention outputs back to token shards.
"""
import os
import sys
import numpy as np

sys.path.insert(0, "/opt/trn_rl_repo")
sys.path.insert(0, "/opt/trn_rl_repo/concourse")

from concourse import bass, bacc, tile, mybir, masks  # noqa: E402
from concourse import bass_utils  # noqa: E402
from concourse.alu_op_type import AluOpType  # noqa: E402

AF = mybir.ActivationFunctionType
F32 = mybir.dt.float32

NCORES = 8
B, S, V, D, H, L, E, NR = 2, 1024, 32000, 512, 8, 4, 8, 2
DFE = 2 * D
DFM = 4 * D
HD = D // H
T = B * S
NBLK = T // 128
TLOC = 256
DC = D // 128        # 4
FE = DFE // 128      # 8
FM = DFM // 128      # 16
ADC = 2 * D // 128   # 8
EPS = 1e-6

CORE_BLOCKS = [[c, NBLK - 1 - c] for c in range(NCORES)]
BLK_SRC = [(m, 0) if m < NCORES else (NBLK - 1 - m, 1) for m in range(NBLK)]

N_LAYERS = int(os.environ.get("KLAYERS", str(L)))
DO_HEAD = os.environ.get("KHEAD", "1") == "1"

_CACHE = {}
LAST_RESULT = None


def build():
    nc = bacc.Bacc("TRN2", target_bir_lowering=False, debug=False,
                   enable_asserts=False, num_devices=NCORES)

    def din(name, shape):
        return nc.dram_tensor(name, list(shape), F32, kind="ExternalInput")

    x0T_d = din("x0T", [D, TLOC])
    s0T_d = din("s0T", [L, D, TLOC])
    wqh_d = din("wqh", [L, D, HD])
    wkh_d = din("wkh", [L, D, HD])
    wvh_d = din("wvh", [L, D, HD])
    wo_d = din("wo", [L, D, D])
    anw_d = din("anw", [L, 2, D])
    rtr_d = din("rtr", [L, D, E])
    ew1_d = din("ew1", [L, E, D, DFE])
    ew2_d = din("ew2", [L, E, DFE, D])
    srad_d = din("srad", [L, 2 * D, D])
    srwv_d = din("srwv", [L, D, D])
    srwo_d = din("srwo", [L, D, D])
    srm1_d = din("srm1", [L, D, DFM])
    srm2_d = din("srm2", [L, DFM, D])
    srnw_d = din("srnw", [L, 4, D])
    onw_d = din("onw", [D])
    lmh_d = nc.dram_tensor("lmh", [D, V], mybir.dt.float16,
                           kind="ExternalInput")
    tri_d = din("tri", [128, 128])
    F16 = mybir.dt.float16
    ew1h_d = nc.dram_tensor("ew1h", [E, D, DFE], F16, kind="ExternalInput")
    ew2h_d = nc.dram_tensor("ew2h", [E, DFE, D], F16, kind="ExternalInput")
    sradh_d = nc.dram_tensor("sradh", [2 * D, D], F16, kind="ExternalInput")
    srwvh_d = nc.dram_tensor("srwvh", [D, D], F16, kind="ExternalInput")
    srwoh_d = nc.dram_tensor("srwoh", [D, D], F16, kind="ExternalInput")
    srm1h_d = nc.dram_tensor("srm1h", [D, DFM], F16, kind="ExternalInput")
    srm2h_d = nc.dram_tensor("srm2h", [DFM, D], F16, kind="ExternalInput")

    logits_d = nc.dram_tensor("logits", [TLOC, V], F32, kind="ExternalOutput")
    aux_d = nc.dram_tensor("aux", [L, 2, E], F32, kind="ExternalOutput")
    dbg_d = nc.dram_tensor("dbg", [D, TLOC], F32, kind="ExternalOutput")

    rg = [list(range(NCORES))]

    def r128(ap):
        return ap.rearrange("(a p) f -> p a f", p=128)

    with tile.TileContext(nc) as tc:
        octx = [
            tc.tile_pool(name="cpool", bufs=1),
            tc.tile_pool(name="wpool", bufs=2),
            tc.tile_pool(name="apool", bufs=1),
            tc.tile_pool(name="dram", bufs=2, space="DRAM"),
        ]
        cpool, wpool, apool, dram = [p.__enter__() for p in octx]
        try:
            ident = cpool.tile([128, 128], F32)
            masks.make_identity(nc, ident[:])
            ones_col = cpool.tile([128, 1], F32)
            nc.vector.memset(ones_col[:], 1.0)
            tri = cpool.tile([128, 128], F32)
            nc.sync.dma_start(tri[:], tri_d.ap())

            xT = cpool.tile([128, DC, TLOC], F32, name="xT")
            nc.sync.dma_start(xT[:], r128(x0T_d.ap()))

            def rmsT(dst, src, w_col, ps_r):
                sq = apool.tile([128, DC, TLOC], F32, tag="rms_sq",
                                name="rms_sq")
                for kc in range(DC):
                    nc.vector.tensor_tensor(sq[:, kc, :], src[:, kc, :],
                                            src[:, kc, :], AluOpType.mult)
                ss = ps_r.tile([1, TLOC], F32, tag="rms_ss", name="rms_ss",
                               bufs=2)
                for kc in range(DC):
                    nc.tensor.matmul(ss[:], ones_col[:], sq[:, kc, :],
                                     start=(kc == 0), stop=(kc == DC - 1))
                st = apool.tile([1, TLOC], F32, tag="rms_st", name="rms_st",
                                bufs=2)
                nc.vector.tensor_scalar(st[:], ss[:], 1.0 / D, EPS,
                                        AluOpType.mult, AluOpType.add)
                st2 = apool.tile([1, TLOC], F32, tag="rms_st2", name="rms_st2",
                                 bufs=2)
                nc.scalar.sqrt(st2[:], st[:])
                st3 = apool.tile([1, TLOC], F32, tag="rms_st3", name="rms_st3",
                                 bufs=2)
                nc.vector.reciprocal(st3[:], st2[:])
                bc = apool.tile([128, TLOC], F32, tag="rms_bc", name="rms_bc",
                                bufs=2)
                nc.gpsimd.partition_broadcast(bc[:], st3[:])
                for kc in range(DC):
                    nc.vector.scalar_tensor_tensor(
                        dst[:, kc, :], src[:, kc, :], w_col[:, kc:kc + 1],
                        bc[:], AluOpType.mult, AluOpType.mult)

            def load_wcol(dram_ap, tag):
                t = wpool.tile([128, DC], F32, tag=tag, name=tag)
                nc.sync.dma_start(t[:], dram_ap.rearrange("(a p) -> p a",
                                                          p=128))
                return t

            # ================= layers =================
            for l in range(N_LAYERS):
                anw1 = load_wcol(anw_d.ap()[l, 0], "anw1")
                anw2 = load_wcol(anw_d.ap()[l, 1], "anw2")

                # ---------- attention ----------
                with (tc.tile_pool(name="aap", bufs=1) as aap,
                      tc.tile_pool(name="ps_r", bufs=2, space="PSUM") as ps_r):
                    wqh = aap.tile([128, DC, HD], F32, tag="wqh", name="wqh")
                    nc.sync.dma_start(wqh[:], r128(wqh_d.ap()[l]))
                    wkh = aap.tile([128, DC, HD], F32, tag="wkh", name="wkh")
                    nc.sync.dma_start(wkh[:], r128(wkh_d.ap()[l]))
                    wvh = aap.tile([128, DC, HD], F32, tag="wvh", name="wvh")
                    nc.sync.dma_start(wvh[:], r128(wvh_d.ap()[l]))
                    wo = aap.tile([128, DC, D], F32, tag="wo", name="wo")
                    nc.sync.dma_start(wo[:], r128(wo_d.ap()[l]))

                    xn = aap.tile([128, DC, TLOC], F32, tag="xn", name="xn")
                    rmsT(xn[:], xT[:], anw1[:], ps_r)

                    ag_in = dram.tile([D * TLOC], F32, tag="ag_in",
                                      name="ag_in")
                    nc.sync.dma_start(
                        ag_in[:].rearrange("(a p f) -> p a f", p=128, a=DC),
                        xn[:])
                    ag_out = dram.tile([NCORES, D * TLOC], F32, tag="ag_out",
                                       name="ag_out", addr_space="Shared")
                    nc.gpsimd.collective_compute(
                        "AllGather", AluOpType.bypass, replica_groups=rg,
                        ins=[ag_in.opt()], outs=[ag_out.opt()])

                    qT = aap.tile([64, NCORES, TLOC], F32, tag="qT", name="qT")
                    kT = aap.tile([64, NCORES, TLOC], F32, tag="kT", name="kT")
                    v2 = aap.tile([128, NCORES, 2, HD + 1], F32, tag="v2",
                                  name="v2")
                    nc.vector.memset(v2[:, :, :, HD:HD + 1], 1.0)
                    with tc.tile_pool(name="ps_qk", bufs=3,
                                      space="PSUM") as ps_qk:
                        for s_ in range(NCORES):
                            xa = aap.tile([128, DC, TLOC], F32, tag="xa",
                                          name="xa", bufs=2)
                            nc.sync.dma_start(
                                xa[:],
                                ag_out[:].rearrange(
                                    "s (a p f) -> s p a f", p=128, a=DC)[s_])
                            pq = ps_qk.tile([64, TLOC], F32, tag="pqk",
                                            name="pq")
                            pk = ps_qk.tile([64, TLOC], F32, tag="pqk",
                                            name="pk")
                            for kc in range(DC):
                                nc.tensor.matmul(pq[:], wqh[:, kc, :],
                                                 xa[:, kc, :], start=(kc == 0),
                                                 stop=(kc == DC - 1))
                            for kc in range(DC):
                                nc.tensor.matmul(pk[:], wkh[:, kc, :],
                                                 xa[:, kc, :], start=(kc == 0),
                                                 stop=(kc == DC - 1))
                            nc.scalar.activation(
                                qT[:, s_, :], pq[:], AF.Copy,
                                scale=1.0 / float(np.sqrt(HD)))
                            nc.scalar.copy(kT[:, s_, :], pk[:])
                            for ti in range(2):
                                pv = ps_qk.tile([128, TLOC], F32, tag="pqk",
                                                name="pv")[:, 0:HD]
                                for kc in range(DC):
                                    nc.tensor.matmul(
                                        pv[:],
                                        xa[:, kc, ti * 128:(ti + 1) * 128],
                                        wvh[:, kc, :], start=(kc == 0),
                                        stop=(kc == DC - 1))
                                nc.scalar.copy(v2[:, s_, ti, 0:HD], pv[:])

                    ao = aap.tile([64, NCORES, TLOC], F32, tag="ao", name="ao")
                    den = aap.tile([1, NCORES, TLOC], F32, tag="den",
                                   name="den")
                    with (tc.tile_pool(name="ps_s", bufs=4,
                                       space="PSUM") as ps_s,
                          tc.tile_pool(name="ps_pv", bufs=2,
                                       space="PSUM") as ps_pv):
                        for qb in range(NBLK):
                            batch, qpos = qb // NCORES, qb % NCORES
                            qs, qh = BLK_SRC[qb]
                            q_ap = qT[:, qs, qh * 128:(qh + 1) * 128]
                            pv = ps_pv.tile([HD + 1, 128], F32, tag="pv_acc",
                                            name="pv_acc")
                            nkb = qpos + 1
                            for kb in range(nkb):
                                m = batch * NCORES + kb
                                ks, kh = BLK_SRC[m]
                                st = ps_s.tile([128, 128], F32, tag="st",
                                               name="st")
                                nc.tensor.matmul(
                                    st[:],
                                    kT[:, ks, kh * 128:(kh + 1) * 128],
                                    q_ap, start=True, stop=True)
                                if kb == nkb - 1:
                                    nc.vector.tensor_tensor(
                                        st[:], st[:], tri[:], AluOpType.add)
                                es = aap.tile([128, 128], F32, tag="es",
                                              name="es", bufs=3)
                                nc.scalar.activation(es[:], st[:], AF.Exp)
                                nc.tensor.matmul(
                                    pv[:], v2[:, ks, kh, :], es[:],
                                    start=(kb == 0), stop=(kb == nkb - 1))
                            nc.scalar.copy(
                                ao[:, qs, qh * 128:(qh + 1) * 128],
                                pv[0:HD, :])
                            nc.scalar.copy(
                                den[:, qs, qh * 128:(qh + 1) * 128],
                                pv[HD:HD + 1, :])

                    rden = aap.tile([1, NCORES, TLOC], F32, tag="rden",
                                    name="rden")
                    nc.vector.reciprocal(rden[:].opt(), den[:].opt())
                    rbc = aap.tile([64, NCORES, TLOC], F32, tag="rbc",
                                   name="rbc")
                    nc.gpsimd.partition_broadcast(
                        rbc[:].opt(), rden[:].opt(), channels=64)
                    aos = aap.tile([64, NCORES, TLOC], F32, tag="aos",
                                   name="aos")
                    nc.vector.tensor_tensor(aos[:].opt(), ao[:].opt(),
                                            rbc[:].opt(), AluOpType.mult)
                    a2a_in = dram.tile([NCORES, 64 * TLOC], F32, tag="a2a_in",
                                       name="a2a_in")
                    for s_ in range(NCORES):
                        nc.sync.dma_start(
                            a2a_in[:].rearrange("s (p f) -> s p f", p=64)[s_],
                            aos[:, s_, :])
                    a2a_out = dram.tile([NCORES, 64 * TLOC], F32,
                                        tag="a2a_out", name="a2a_out")
                    nc.gpsimd.collective_compute(
                        "AllToAll", AluOpType.bypass, replica_groups=rg,
                        ins=[a2a_in.opt()], outs=[a2a_out.opt()])
                    atT = aap.tile([128, DC, TLOC], F32, tag="atT", name="atT")
                    nc.sync.dma_start(
                        atT[:],
                        a2a_out[:].rearrange("s (p f) -> (s p) f", p=64)
                        .rearrange("(a p) f -> p a f", p=128))

                    with tc.tile_pool(name="ps_o", bufs=2,
                                      space="PSUM") as ps_o:
                        for mc in range(DC):
                            po = ps_o.tile([128, TLOC], F32, tag="po",
                                           name="po")
                            for kc in range(DC):
                                nc.tensor.matmul(
                                    po[:], wo[:, kc, mc * 128:(mc + 1) * 128],
                                    atT[:, kc, :], start=(kc == 0),
                                    stop=(kc == DC - 1))
                            nc.vector.tensor_tensor(xT[:, mc, :], xT[:, mc, :],
                                                    po[:], AluOpType.add)

                # ---------- router ----------
                xf = apool.tile([128, DC, TLOC], F32, tag="xf", name="xf")
                cwT = apool.tile([1, E, TLOC], F32, tag="cwT", name="cwT")
                with (tc.tile_pool(name="ps_l", bufs=2, space="PSUM") as ps_l,
                      tc.tile_pool(name="ps_x", bufs=3, space="PSUM") as ps_x):
                    rmsT(xf[:], xT[:], anw2[:], ps_l)
                    rtr = apool.tile([128, DC, E], F32, tag="rtr", name="rtr",
                                     bufs=2)
                    nc.sync.dma_start(rtr[:], r128(rtr_d.ap()[l]))
                    ohp = apool.tile([128, 2, E], F32, tag="ohp", name="ohp")
                    prb = apool.tile([128, 2, E], F32, tag="prb", name="prb")
                    for ti in range(2):
                        pl = ps_l.tile([128, E], F32, tag="pl", name="pl",
                                        bufs=1)
                        for kc in range(DC):
                            nc.tensor.matmul(
                                pl[:], xf[:, kc, ti * 128:(ti + 1) * 128],
                                rtr[:, kc, :], start=(kc == 0),
                                stop=(kc == DC - 1))
                        lg = apool.tile([128, E], F32, tag="lg", name="lg",
                                        bufs=2)
                        nc.vector.tensor_copy(lg[:], pl[:])
                        mx = apool.tile([128, 1], F32, tag="mx", name="mx",
                                        bufs=2)
                        nc.vector.tensor_reduce(mx[:], lg[:],
                                                mybir.AxisListType.X,
                                                AluOpType.max)
                        nc.vector.tensor_scalar(ohp[:, ti, :], lg[:], mx[:],
                                                None, AluOpType.is_equal)
                        nmx = apool.tile([128, 1], F32, tag="nmx", name="nmx",
                                         bufs=2)
                        nc.vector.tensor_scalar(nmx[:], mx[:], -1.0, None,
                                                AluOpType.mult)
                        rs = apool.tile([128, 1], F32, tag="rs", name="rs",
                                        bufs=2)
                        ex = apool.tile([128, E], F32, tag="ex", name="ex",
                                        bufs=2)
                        nc.scalar.activation(ex[:], lg[:], AF.Exp,
                                             bias=nmx[:], accum_out=rs[:])
                        rrs = apool.tile([128, 1], F32, tag="rrs", name="rrs",
                                         bufs=2)
                        nc.vector.reciprocal(rrs[:], rs[:])
                        nc.vector.tensor_scalar(prb[:, ti, :], ex[:], rrs[:],
                                                None, AluOpType.mult)
                    pf = ps_x.tile([1, E], F32, tag="pf", name="pf", bufs=1)
                    pp = ps_x.tile([1, E], F32, tag="pp", name="pp", bufs=1)
                    for ti in range(2):
                        nc.tensor.matmul(pf[:], ones_col[:], ohp[:, ti, :],
                                         start=(ti == 0), stop=(ti == 1))
                    for ti in range(2):
                        nc.tensor.matmul(pp[:], ones_col[:], prb[:, ti, :],
                                         start=(ti == 0), stop=(ti == 1))
                    auxs = apool.tile([1, 2, E], F32, tag="auxs", name="auxs")
                    nc.vector.tensor_copy(auxs[:, 0, :], pf[:])
                    nc.vector.tensor_copy(auxs[:, 1, :], pp[:])
                    nc.sync.dma_start(aux_d.ap()[l], auxs[:].opt())

                    for ti in range(2):
                        pt = ps_x.tile([E, 128], F32, tag="pt", name="pt",
                                         bufs=1)
                        nc.tensor.transpose(pt[:], ohp[:, ti, :], ident[:])
                        ptc = apool.tile([E, 128], F32, tag="ptc", name="ptc",
                                         bufs=2)
                        nc.vector.tensor_copy(ptc[:], pt[:])
                        nc.sync.dma_start(
                            cwT[0:1, :, ti * 128:(ti + 1) * 128],
                            ptc[:])

                # ---------- experts (dense, masked before w2) ----------
                eoutT = apool.tile([128, DC, TLOC], F32, tag="eoutT",
                                   name="eoutT")
                fp16x = (l == L - 1)
                edt = mybir.dt.float16 if fp16x else F32
                with (tc.tile_pool(name="wep", bufs=2) as wep,
                      tc.tile_pool(name="ps_h", bufs=3, space="PSUM") as ps_h,
                      tc.tile_pool(name="ps_eo", bufs=4,
                                   space="PSUM") as ps_eo):
                    if fp16x:
                        xfh = apool.tile([128, DC, TLOC], edt, tag="xfh",
                                         name="xfh")
                        nc.vector.tensor_copy(xfh[:].opt(), xf[:].opt())
                    else:
                        xfh = xf
                    eo = [ps_eo.tile([128, TLOC], F32, tag=f"eo{mc}",
                                     name=f"eo{mc}", bufs=1)
                          for mc in range(DC)]
                    for e_ in range(E):
                        w1 = wep.tile([128, DC, DFE], edt, tag="ew1",
                                      name="ew1")
                        nc.sync.dma_start(
                            w1[:], r128(ew1h_d.ap()[e_]) if fp16x
                            else r128(ew1_d.ap()[l, e_]))
                        w2 = wep.tile([128, FE, D], edt, tag="ew2", name="ew2")
                        nc.sync.dma_start(
                            w2[:], r128(ew2h_d.ap()[e_]) if fp16x
                            else r128(ew2_d.ap()[l, e_]))
                        bce = apool.tile([128, TLOC], F32, tag="bce",
                                         name="bce", bufs=2)
                        nc.gpsimd.partition_broadcast(bce[:],
                                                      cwT[0:1, e_, :])
                        for fc in range(FE):
                            ph = ps_h.tile([128, TLOC], F32, tag="ph",
                                           name="ph")
                            for kc in range(DC):
                                nc.tensor.matmul(
                                    ph[:], w1[:, kc, fc * 128:(fc + 1) * 128],
                                    xfh[:, kc, :], start=(kc == 0),
                                    stop=(kc == DC - 1))
                            hr = apool.tile([128, TLOC], F32, tag="hr",
                                            name="hr", bufs=2)
                            nc.scalar.activation(hr[:], ph[:], AF.Gelu)
                            hs = apool.tile([128, TLOC], edt, tag="hs",
                                            name="hs", bufs=3)
                            nc.vector.tensor_tensor(hs[:], hr[:], bce[:],
                                                    AluOpType.mult)
                            for mc in range(DC):
                                nc.tensor.matmul(
                                    eo[mc][:],
                                    w2[:, fc, mc * 128:(mc + 1) * 128],
                                    hs[:],
                                    start=(e_ == 0 and fc == 0),
                                    stop=(e_ == E - 1 and fc == FE - 1))
                    for mc in range(DC):
                        nc.vector.tensor_copy(eoutT[:, mc, :], eo[mc][:])

                # ---------- shared recurrent ----------
                rdt = mybir.dt.float16 if fp16x else F32
                with tc.tile_pool(name="wrp", bufs=1) as wrp:
                    srad = wrp.tile([128, ADC, D], rdt, tag="srad",
                                    name="srad")
                    nc.sync.dma_start(srad[:], r128(sradh_d.ap()) if fp16x
                                      else r128(srad_d.ap()[l]))
                    srwv = wrp.tile([128, DC, D], rdt, tag="srwv", name="srwv")
                    nc.sync.dma_start(srwv[:], r128(srwvh_d.ap()) if fp16x
                                      else r128(srwv_d.ap()[l]))
                    srwo = wrp.tile([128, DC, D], rdt, tag="srwo", name="srwo")
                    nc.sync.dma_start(srwo[:], r128(srwoh_d.ap()) if fp16x
                                      else r128(srwo_d.ap()[l]))
                    srm1 = wrp.tile([128, DC, DFM], rdt, tag="srm1",
                                    name="srm1")
                    nc.sync.dma_start(srm1[:], r128(srm1h_d.ap()) if fp16x
                                      else r128(srm1_d.ap()[l]))
                    srm2 = wrp.tile([128, FM, D], rdt, tag="srm2", name="srm2")
                    nc.sync.dma_start(srm2[:], r128(srm2h_d.ap()) if fp16x
                                      else r128(srm2_d.ap()[l]))
                    n1 = load_wcol(srnw_d.ap()[l, 0], "srn1")
                    n2 = load_wcol(srnw_d.ap()[l, 1], "srn2")
                    n3 = load_wcol(srnw_d.ap()[l, 2], "srn3")
                    n4 = load_wcol(srnw_d.ap()[l, 3], "srn4")

                    state = apool.tile([128, DC, TLOC], F32, tag="state",
                                       name="state")
                    nc.sync.dma_start(state[:], r128(s0T_d.ap()[l]))
                    if fp16x:
                        st16 = apool.tile([128, DC, TLOC], rdt, tag="st16",
                                          name="st16")
                        nc.vector.tensor_copy(st16[:].opt(), state[:].opt())
                        state = st16
                        eo16 = apool.tile([128, DC, TLOC], rdt, tag="eo16",
                                          name="eo16")
                        nc.vector.tensor_copy(eo16[:].opt(), eoutT[:].opt())
                        eoutT = eo16

                    for r_ in range(NR):
                        with (tc.tile_pool(name="ps_a", bufs=2,
                                           space="PSUM") as ps_a,
                              tc.tile_pool(name="ps_pm", bufs=4,
                                           space="PSUM") as ps_pm,
                              tc.tile_pool(name="ps_rr", bufs=2,
                                           space="PSUM") as ps_rr):
                            s1 = apool.tile([128, DC, TLOC], F32, tag="s1",
                                            name="s1")
                            for mc in range(DC):
                                pa = ps_a.tile([128, TLOC], F32, tag="pa",
                                               name="pa")
                                for kc in range(ADC):
                                    rhs = (state[:, kc, :] if kc < DC
                                           else eoutT[:, kc - DC, :])
                                    nc.tensor.matmul(
                                        pa[:],
                                        srad[:, kc, mc * 128:(mc + 1) * 128],
                                        rhs, start=(kc == 0),
                                        stop=(kc == ADC - 1))
                                nc.scalar.copy(s1[:, mc, :], pa[:])
                            sn = apool.tile([128, DC, TLOC], rdt, tag="sn",
                                            name="sn")
                            rmsT(sn[:], s1[:], n1[:], ps_rr)
                            av = apool.tile([128, DC, TLOC], rdt, tag="av",
                                            name="av")
                            for mc in range(DC):
                                pa = ps_a.tile([128, TLOC], F32, tag="pa",
                                               name="pa")
                                for kc in range(DC):
                                    nc.tensor.matmul(
                                        pa[:],
                                        srwv[:, kc, mc * 128:(mc + 1) * 128],
                                        sn[:, kc, :], start=(kc == 0),
                                        stop=(kc == DC - 1))
                                nc.scalar.copy(av[:, mc, :], pa[:])
                            r2 = apool.tile([128, DC, TLOC], F32, tag="r2",
                                            name="r2")
                            for mc in range(DC):
                                pa = ps_a.tile([128, TLOC], F32, tag="pa",
                                               name="pa")
                                for kc in range(DC):
                                    nc.tensor.matmul(
                                        pa[:],
                                        srwo[:, kc, mc * 128:(mc + 1) * 128],
                                        av[:, kc, :], start=(kc == 0),
                                        stop=(kc == DC - 1))
                                nc.vector.tensor_tensor(
                                    r2[:, mc, :], s1[:, mc, :], pa[:],
                                    AluOpType.add)
                            r2n = apool.tile([128, DC, TLOC], F32, tag="r2n",
                                             name="r2n")
                            rmsT(r2n[:], r2[:], n2[:], ps_rr)
                            s3n = apool.tile([128, DC, TLOC], rdt,
                                             tag="s3n", name="s3n")
                            rmsT(s3n[:], r2n[:], n3[:], ps_rr)
                            pm = [ps_pm.tile([128, TLOC], F32, tag=f"pm{mc}",
                                             name=f"pm{mc}", bufs=1)
                                  for mc in range(DC)]
                            for fc in range(FM):
                                pa = ps_a.tile([128, TLOC], F32, tag="pa",
                                               name="pa")
                                for kc in range(DC):
                                    nc.tensor.matmul(
                                        pa[:],
                                        srm1[:, kc, fc * 128:(fc + 1) * 128],
                                        s3n[:, kc, :], start=(kc == 0),
                                        stop=(kc == DC - 1))
                                ms = apool.tile([128, TLOC], rdt, tag="ms",
                                                name="ms", bufs=3)
                                nc.scalar.activation(ms[:], pa[:], AF.Silu)
                                for mc in range(DC):
                                    nc.tensor.matmul(
                                        pm[mc][:],
                                        srm2[:, fc, mc * 128:(mc + 1) * 128],
                                        ms[:], start=(fc == 0),
                                        stop=(fc == FM - 1))
                            r4 = apool.tile([128, DC, TLOC], F32, tag="r4",
                                            name="r4")
                            for mc in range(DC):
                                nc.vector.tensor_tensor(
                                    r4[:, mc, :], r2n[:, mc, :], pm[mc][:],
                                    AluOpType.add)
                            stn = apool.tile([128, DC, TLOC], rdt,
                                             tag="state_n", name="state_n")
                            rmsT(stn[:], r4[:], n4[:], ps_rr)
                            state = stn

                for mc in range(DC):
                    nc.vector.tensor_tensor(xT[:, mc, :], xT[:, mc, :],
                                            state[:, mc, :], AluOpType.add)

            # ---------- final norm + lm_head ----------
            nc.sync.dma_start(r128(dbg_d.ap()), xT[:])
            if DO_HEAD:
                onw = load_wcol(onw_d.ap(), "onw")
                xo = apool.tile([128, DC, TLOC], F32, tag="xo", name="xo")
                with tc.tile_pool(name="ps_f", bufs=2, space="PSUM") as ps_f:
                    rmsT(xo[:], xT[:], onw[:], ps_f)
                xoh = apool.tile([128, DC, TLOC], mybir.dt.float16,
                                 tag="xoh", name="xoh")
                nc.vector.tensor_copy(xoh[:].opt(), xo[:].opt())
                with (tc.tile_pool(name="wlm", bufs=3) as wlm,
                      tc.tile_pool(name="ps_lm", bufs=4,
                                   space="PSUM") as ps_lm):
                    NV = 512
                    for vs in range(0, V, NV):
                        nv = min(NV, V - vs)
                        lw = wlm.tile([128, DC, NV], mybir.dt.float16,
                                      tag="lmw", name="lmw")
                        nc.sync.dma_start(
                            lw[:, :, 0:nv],
                            lmh_d.ap()[:, vs:vs + nv].rearrange(
                                "(a p) f -> p a f", p=128))
                        for ti in range(2):
                            pl2 = ps_lm.tile([128, NV], F32, tag="plm",
                                             name="plm")
                            for kc in range(DC):
                                nc.tensor.matmul(
                                    pl2[:, 0:nv],
                                    xoh[:, kc, ti * 128:(ti + 1) * 128],
                                    lw[:, kc, 0:nv], start=(kc == 0),
                                    stop=(kc == DC - 1))
                            ot = apool.tile([128, NV], F32, tag="ot",
                                            name="ot", bufs=3)
                            nc.scalar.copy(ot[:, 0:nv], pl2[:, 0:nv])
                            nc.sync.dma_start(
                                logits_d.ap()[ti * 128:(ti + 1) * 128,
                                              vs:vs + nv],
                                ot[:, 0:nv])
        finally:
            for p in reversed(octx):
                p.__exit__(None, None, None)

    nc.compile()
    return nc


def _state_inits():
    import jax
    import jax.numpy as jnp
    key = jax.random.key(42)
    out = []
    for i in range(L):
        s = jax.random.normal(jax.random.fold_in(key, i), (T, D), jnp.float32)
        out.append(np.asarray(s) * np.float32(0.02))
    return out


def kernel(input_ids, params):
    global LAST_RESULT
    input_ids = np.asarray(input_ids)
    p = params

    if "nc" not in _CACHE:
        _CACHE["nc"] = build()
    nc = _CACHE["nc"]

    tok = np.asarray(p["tok_emb"], dtype=np.float32)
    pos = np.asarray(p["pos_emb"], dtype=np.float32)[:S]
    x0 = tok[np.asarray(input_ids).reshape(-1)].reshape(B, S, D) + pos[None]
    x0 = x0.reshape(T, D)

    s0 = _state_inits()

    tri = np.zeros((128, 128), dtype=np.float32)
    for k_ in range(128):
        tri[k_, :k_] = -1.0e30

    def f32(a):
        return np.ascontiguousarray(np.asarray(a, dtype=np.float32))

    layers = p["layers"]
    shared = {
        "wo": f32(np.stack([lp["attn_wo"] for lp in layers])),
        "anw": f32(np.stack([[lp["norm1"], lp["norm2"]] for lp in layers])),
        "rtr": f32(np.stack([lp["router"] for lp in layers])),
        "ew1": f32(np.stack([lp["exp_w1"] for lp in layers])),
        "ew2": f32(np.stack([lp["exp_w2"] for lp in layers])),
        "srad": f32(np.stack([lp["sr"]["adapter"] for lp in layers])),
        "srwv": f32(np.stack([lp["sr"]["attn_wv"] for lp in layers])),
        "srwo": f32(np.stack([lp["sr"]["attn_wo"] for lp in layers])),
        "srm1": f32(np.stack([lp["sr"]["mlp_w1"] for lp in layers])),
        "srm2": f32(np.stack([lp["sr"]["mlp_w2"] for lp in layers])),
        "srnw": f32(np.stack([[lp["sr"]["n1"], lp["sr"]["n2"],
                               lp["sr"]["n3"], lp["sr"]["n4"]]
                              for lp in layers])),
        "onw": f32(p["norm_out"]),
        "lmh": np.ascontiguousarray(
            np.asarray(p["lm_head"]).astype(np.float16)),
        "tri": tri,
        "ew1h": np.ascontiguousarray(
            np.asarray(layers[L - 1]["exp_w1"]).astype(np.float16)),
        "ew2h": np.ascontiguousarray(
            np.asarray(layers[L - 1]["exp_w2"]).astype(np.float16)),
        "sradh": np.ascontiguousarray(
            np.asarray(layers[L - 1]["sr"]["adapter"]).astype(np.float16)),
        "srwvh": np.ascontiguousarray(
            np.asarray(layers[L - 1]["sr"]["attn_wv"]).astype(np.float16)),
        "srwoh": np.ascontiguousarray(
            np.asarray(layers[L - 1]["sr"]["attn_wo"]).astype(np.float16)),
        "srm1h": np.ascontiguousarray(
            np.asarray(layers[L - 1]["sr"]["mlp_w1"]).astype(np.float16)),
        "srm2h": np.ascontiguousarray(
            np.asarray(layers[L - 1]["sr"]["mlp_w2"]).astype(np.float16)),
    }
    wq = f32(np.stack([lp["attn_wq"] for lp in layers]))
    wk = f32(np.stack([lp["attn_wk"] for lp in layers]))
    wv = f32(np.stack([lp["attn_wv"] for lp in layers]))

    in_maps = []
    for c in range(NCORES):
        rows = np.concatenate([np.arange(b_ * 128, b_ * 128 + 128)
                               for b_ in CORE_BLOCKS[c]])
        im = dict(shared)
        im["x0T"] = np.ascontiguousarray(x0[rows].T)
        im["s0T"] = np.ascontiguousarray(
            np.stack([s0[li][rows].T for li in range(L)]))
        hs = slice(c * HD, (c + 1) * HD)
        im["wqh"] = np.ascontiguousarray(wq[:, :, hs])
        im["wkh"] = np.ascontiguousarray(wk[:, :, hs])
        im["wvh"] = np.ascontiguousarray(wv[:, :, hs])
        in_maps.append(im)

    res = bass_utils.run_bass_kernel_spmd(
        nc, in_maps, core_ids=list(range(NCORES)))
    LAST_RESULT = res

    logits = np.zeros((T, V), dtype=np.float32)
    fsum = np.zeros((L, E), dtype=np.float64)
    psum = np.zeros((L, E), dtype=np.float64)
    for c in range(NCORES):
        o = res.results[c]
        lg = o["logits"]
        for j, b_ in enumerate(CORE_BLOCKS[c]):
            logits[b_ * 128:(b_ + 1) * 128] = lg[j * 128:(j + 1) * 128]
        fsum += o["aux"][:, 0, :]
        psum += o["aux"][:, 1, :]

    total_aux = np.float32(0.0)
    for li in range(L):
        f = (fsum[li] / T).astype(np.float32)
        pr = (psum[li] / T).astype(np.float32)
        total_aux = np.float32(total_aux + np.float32(E) *
                               np.float32(np.sum(f * pr, dtype=np.float32)))
    return logits.reshape(B, S, V), total_aux


# revision 16
# speedup vs baseline: 1.2469x; 1.0410x over previous
"""Trainium2 Bass kernel for nn_MoREModelSynthesisIOptionB (moe_routing).

Sharding: 8 NeuronCores. Token-data-parallel for MoE/recurrent/lm_head
(core c owns token blocks {c, 15-c} of 128 tokens), head-parallel for
attention (core c owns head c; head weight slices are passed as per-core
input data so the compiled program is identical on every core).
Activations are kept transposed ([d, t]) so weight matrices serve as the
stationary matmul operand exactly as stored. Dense expert dispatch with
the one-hot top-1 combine mask applied to the gelu output before the w2
matmul (PSUM accumulates over experts). fp32 matmuls throughout.
Collectives per layer: AllGather of normed x (attention input), AllToAll
of per-head attention outputs back to token shards.
"""
import os
import sys
import numpy as np

sys.path.insert(0, "/opt/trn_rl_repo")
sys.path.insert(0, "/opt/trn_rl_repo/concourse")

from concourse import bass, bacc, tile, mybir, masks  # noqa: E402
from concourse import bass_utils  # noqa: E402
from concourse.alu_op_type import AluOpType  # noqa: E402

AF = mybir.ActivationFunctionType
F32 = mybir.dt.float32

NCORES = 8
B, S, V, D, H, L, E, NR = 2, 1024, 32000, 512, 8, 4, 8, 2
DFE = 2 * D
DFM = 4 * D
HD = D // H
T = B * S
NBLK = T // 128
TLOC = 256
DC = D // 128        # 4
FE = DFE // 128      # 8
FM = DFM // 128      # 16
ADC = 2 * D // 128   # 8
EPS = 1e-6

CORE_BLOCKS = [[c, NBLK - 1 - c] for c in range(NCORES)]
BLK_SRC = [(m, 0) if m < NCORES else (NBLK - 1 - m, 1) for m in range(NBLK)]

N_LAYERS = int(os.environ.get("KLAYERS", str(L)))
DO_HEAD = os.environ.get("KHEAD", "1") == "1"

_CACHE = {}
LAST_RESULT = None


def build():
    nc = bacc.Bacc("TRN2", target_bir_lowering=False, debug=False,
                   enable_asserts=False, num_devices=NCORES)

    def din(name, shape):
        return nc.dram_tensor(name, list(shape), F32, kind="ExternalInput")

    x0T_d = din("x0T", [D, TLOC])
    s0T_d = din("s0T", [L, D, TLOC])
    wqh_d = din("wqh", [L, D, HD])
    wkh_d = din("wkh", [L, D, HD])
    wvh_d = din("wvh", [L, D, HD])
    wo_d = din("wo", [L, D, D])
    anw_d = din("anw", [L, 2, D])
    rtr_d = din("rtr", [L, D, E])
    BF16 = mybir.dt.bfloat16
    ew1hi_d = nc.dram_tensor("ew1hi", [L - 1, E, D, DFE], BF16,
                             kind="ExternalInput")
    ew1lo_d = nc.dram_tensor("ew1lo", [L - 1, E, D, DFE], BF16,
                             kind="ExternalInput")
    ew2hi_d = nc.dram_tensor("ew2hi", [L - 1, E, DFE, D], BF16,
                             kind="ExternalInput")
    ew2lo_d = nc.dram_tensor("ew2lo", [L - 1, E, DFE, D], BF16,
                             kind="ExternalInput")
    srad_d = din("srad", [L, 2 * D, D])
    srwv_d = din("srwv", [L, D, D])
    srwo_d = din("srwo", [L, D, D])
    srm1_d = din("srm1", [L, D, DFM])
    srm2_d = din("srm2", [L, DFM, D])
    srnw_d = din("srnw", [L, 4, D])
    onw_d = din("onw", [D])
    lmh_d = nc.dram_tensor("lmh", [D, V], mybir.dt.float16,
                           kind="ExternalInput")
    tri_d = din("tri", [128, 128])
    F16 = mybir.dt.float16
    ew1h_d = nc.dram_tensor("ew1h", [E, D, DFE], F16, kind="ExternalInput")
    ew2h_d = nc.dram_tensor("ew2h", [E, DFE, D], F16, kind="ExternalInput")
    sradh_d = nc.dram_tensor("sradh", [2 * D, D], F16, kind="ExternalInput")
    srwvh_d = nc.dram_tensor("srwvh", [D, D], F16, kind="ExternalInput")
    srwoh_d = nc.dram_tensor("srwoh", [D, D], F16, kind="ExternalInput")
    srm1h_d = nc.dram_tensor("srm1h", [D, DFM], F16, kind="ExternalInput")
    srm2h_d = nc.dram_tensor("srm2h", [DFM, D], F16, kind="ExternalInput")

    logits_d = nc.dram_tensor("logits", [TLOC, V], F32, kind="ExternalOutput")
    aux_d = nc.dram_tensor("aux", [L, 2, E], F32, kind="ExternalOutput")
    dbg_d = nc.dram_tensor("dbg", [D, TLOC], F32, kind="ExternalOutput")

    rg = [list(range(NCORES))]

    def r128(ap):
        return ap.rearrange("(a p) f -> p a f", p=128)

    with tile.TileContext(nc) as tc:
        octx = [
            tc.tile_pool(name="cpool", bufs=1),
            tc.tile_pool(name="wpool", bufs=2),
            tc.tile_pool(name="apool", bufs=1),
            tc.tile_pool(name="dram", bufs=2, space="DRAM"),
        ]
        cpool, wpool, apool, dram = [p.__enter__() for p in octx]
        try:
            ident = cpool.tile([128, 128], F32)
            masks.make_identity(nc, ident[:])
            ones_col = cpool.tile([128, 1], F32)
            nc.vector.memset(ones_col[:], 1.0)
            tri = cpool.tile([128, 128], F32)
            nc.sync.dma_start(tri[:], tri_d.ap())

            xT = cpool.tile([128, DC, TLOC], F32, name="xT")
            nc.sync.dma_start(xT[:], r128(x0T_d.ap()))

            def rmsT(dst, src, w_col, ps_r):
                sq = apool.tile([128, DC, TLOC], F32, tag="rms_sq",
                                name="rms_sq")
                for kc in range(DC):
                    nc.vector.tensor_tensor(sq[:, kc, :], src[:, kc, :],
                                            src[:, kc, :], AluOpType.mult)
                ss = ps_r.tile([1, TLOC], F32, tag="rms_ss", name="rms_ss",
                               bufs=2)
                for kc in range(DC):
                    nc.tensor.matmul(ss[:], ones_col[:], sq[:, kc, :],
                                     start=(kc == 0), stop=(kc == DC - 1))
                st = apool.tile([1, TLOC], F32, tag="rms_st", name="rms_st",
                                bufs=2)
                nc.vector.tensor_scalar(st[:], ss[:], 1.0 / D, EPS,
                                        AluOpType.mult, AluOpType.add)
                st2 = apool.tile([1, TLOC], F32, tag="rms_st2", name="rms_st2",
                                 bufs=2)
                nc.scalar.sqrt(st2[:], st[:])
                st3 = apool.tile([1, TLOC], F32, tag="rms_st3", name="rms_st3",
                                 bufs=2)
                nc.vector.reciprocal(st3[:], st2[:])
                bc = apool.tile([128, TLOC], F32, tag="rms_bc", name="rms_bc",
                                bufs=2)
                nc.gpsimd.partition_broadcast(bc[:], st3[:])
                for kc in range(DC):
                    nc.vector.scalar_tensor_tensor(
                        dst[:, kc, :], src[:, kc, :], w_col[:, kc:kc + 1],
                        bc[:], AluOpType.mult, AluOpType.mult)

            def load_wcol(dram_ap, tag):
                t = wpool.tile([128, DC], F32, tag=tag, name=tag)
                nc.sync.dma_start(t[:], dram_ap.rearrange("(a p) -> p a",
                                                          p=128))
                return t

            # ================= layers =================
            for l in range(N_LAYERS):
                anw1 = load_wcol(anw_d.ap()[l, 0], "anw1")
                anw2 = load_wcol(anw_d.ap()[l, 1], "anw2")

                # ---------- attention ----------
                with (tc.tile_pool(name="aap", bufs=1) as aap,
                      tc.tile_pool(name="ps_r", bufs=2, space="PSUM") as ps_r):
                    wqh = aap.tile([128, DC, HD], F32, tag="wqh", name="wqh")
                    nc.sync.dma_start(wqh[:], r128(wqh_d.ap()[l]))
                    wkh = aap.tile([128, DC, HD], F32, tag="wkh", name="wkh")
                    nc.sync.dma_start(wkh[:], r128(wkh_d.ap()[l]))
                    wvh = aap.tile([128, DC, HD], F32, tag="wvh", name="wvh")
                    nc.sync.dma_start(wvh[:], r128(wvh_d.ap()[l]))
                    wo = aap.tile([128, DC, D], F32, tag="wo", name="wo")
                    nc.sync.dma_start(wo[:], r128(wo_d.ap()[l]))

                    xn = aap.tile([128, DC, TLOC], F32, tag="xn", name="xn")
                    rmsT(xn[:], xT[:], anw1[:], ps_r)

                    ag_in = dram.tile([D * TLOC], F32, tag="ag_in",
                                      name="ag_in")
                    nc.sync.dma_start(
                        ag_in[:].rearrange("(a p f) -> p a f", p=128, a=DC),
                        xn[:])
                    ag_out = dram.tile([NCORES, D * TLOC], F32, tag="ag_out",
                                       name="ag_out", addr_space="Shared")
                    nc.gpsimd.collective_compute(
                        "AllGather", AluOpType.bypass, replica_groups=rg,
                        ins=[ag_in.opt()], outs=[ag_out.opt()])

                    qT = aap.tile([64, NCORES, TLOC], F32, tag="qT", name="qT")
                    kT = aap.tile([64, NCORES, TLOC], F32, tag="kT", name="kT")
                    v2 = aap.tile([128, NCORES, 2, HD + 1], F32, tag="v2",
                                  name="v2")
                    nc.vector.memset(v2[:, :, :, HD:HD + 1], 1.0)
                    with tc.tile_pool(name="ps_qk", bufs=3,
                                      space="PSUM") as ps_qk:
                        for s_ in range(NCORES):
                            xa = aap.tile([128, DC, TLOC], F32, tag="xa",
                                          name="xa", bufs=2)
                            nc.sync.dma_start(
                                xa[:],
                                ag_out[:].rearrange(
                                    "s (a p f) -> s p a f", p=128, a=DC)[s_])
                            pq = ps_qk.tile([64, TLOC], F32, tag="pqk",
                                            name="pq")
                            pk = ps_qk.tile([64, TLOC], F32, tag="pqk",
                                            name="pk")
                            for kc in range(DC):
                                nc.tensor.matmul(pq[:], wqh[:, kc, :],
                                                 xa[:, kc, :], start=(kc == 0),
                                                 stop=(kc == DC - 1))
                            for kc in range(DC):
                                nc.tensor.matmul(pk[:], wkh[:, kc, :],
                                                 xa[:, kc, :], start=(kc == 0),
                                                 stop=(kc == DC - 1))
                            nc.scalar.activation(
                                qT[:, s_, :], pq[:], AF.Copy,
                                scale=1.0 / float(np.sqrt(HD)))
                            nc.scalar.copy(kT[:, s_, :], pk[:])
                            for ti in range(2):
                                pv = ps_qk.tile([128, TLOC], F32, tag="pqk",
                                                name="pv")[:, 0:HD]
                                for kc in range(DC):
                                    nc.tensor.matmul(
                                        pv[:],
                                        xa[:, kc, ti * 128:(ti + 1) * 128],
                                        wvh[:, kc, :], start=(kc == 0),
                                        stop=(kc == DC - 1))
                                nc.scalar.copy(v2[:, s_, ti, 0:HD], pv[:])

                    ao = aap.tile([64, NCORES, TLOC], F32, tag="ao", name="ao")
                    den = aap.tile([1, NCORES, TLOC], F32, tag="den",
                                   name="den")
                    with (tc.tile_pool(name="ps_s", bufs=4,
                                       space="PSUM") as ps_s,
                          tc.tile_pool(name="ps_pv", bufs=2,
                                       space="PSUM") as ps_pv):
                        for qb in range(NBLK):
                            batch, qpos = qb // NCORES, qb % NCORES
                            qs, qh = BLK_SRC[qb]
                            q_ap = qT[:, qs, qh * 128:(qh + 1) * 128]
                            pv = ps_pv.tile([HD + 1, 128], F32, tag="pv_acc",
                                            name="pv_acc")
                            nkb = qpos + 1
                            for kb in range(nkb):
                                m = batch * NCORES + kb
                                ks, kh = BLK_SRC[m]
                                st = ps_s.tile([128, 128], F32, tag="st",
                                               name="st")
                                nc.tensor.matmul(
                                    st[:],
                                    kT[:, ks, kh * 128:(kh + 1) * 128],
                                    q_ap, start=True, stop=True)
                                if kb == nkb - 1:
                                    nc.vector.tensor_tensor(
                                        st[:], st[:], tri[:], AluOpType.add)
                                es = aap.tile([128, 128], F32, tag="es",
                                              name="es", bufs=3)
                                nc.scalar.activation(es[:], st[:], AF.Exp)
                                nc.tensor.matmul(
                                    pv[:], v2[:, ks, kh, :], es[:],
                                    start=(kb == 0), stop=(kb == nkb - 1))
                            nc.scalar.copy(
                                ao[:, qs, qh * 128:(qh + 1) * 128],
                                pv[0:HD, :])
                            nc.scalar.copy(
                                den[:, qs, qh * 128:(qh + 1) * 128],
                                pv[HD:HD + 1, :])

                    rden = aap.tile([1, NCORES, TLOC], F32, tag="rden",
                                    name="rden")
                    nc.vector.reciprocal(rden[:].opt(), den[:].opt())
                    rbc = aap.tile([64, NCORES, TLOC], F32, tag="rbc",
                                   name="rbc")
                    nc.gpsimd.partition_broadcast(
                        rbc[:].opt(), rden[:].opt(), channels=64)
                    aos = aap.tile([64, NCORES, TLOC], F32, tag="aos",
                                   name="aos")
                    nc.vector.tensor_tensor(aos[:].opt(), ao[:].opt(),
                                            rbc[:].opt(), AluOpType.mult)
                    a2a_in = dram.tile([NCORES, 64 * TLOC], F32, tag="a2a_in",
                                       name="a2a_in")
                    for s_ in range(NCORES):
                        nc.sync.dma_start(
                            a2a_in[:].rearrange("s (p f) -> s p f", p=64)[s_],
                            aos[:, s_, :])
                    a2a_out = dram.tile([NCORES, 64 * TLOC], F32,
                                        tag="a2a_out", name="a2a_out")
                    nc.gpsimd.collective_compute(
                        "AllToAll", AluOpType.bypass, replica_groups=rg,
                        ins=[a2a_in.opt()], outs=[a2a_out.opt()])
                    atT = aap.tile([128, DC, TLOC], F32, tag="atT", name="atT")
                    nc.sync.dma_start(
                        atT[:],
                        a2a_out[:].rearrange("s (p f) -> (s p) f", p=64)
                        .rearrange("(a p) f -> p a f", p=128))

                    with tc.tile_pool(name="ps_o", bufs=2,
                                      space="PSUM") as ps_o:
                        for mc in range(DC):
                            po = ps_o.tile([128, TLOC], F32, tag="po",
                                           name="po")
                            for kc in range(DC):
                                nc.tensor.matmul(
                                    po[:], wo[:, kc, mc * 128:(mc + 1) * 128],
                                    atT[:, kc, :], start=(kc == 0),
                                    stop=(kc == DC - 1))
                            nc.vector.tensor_tensor(xT[:, mc, :], xT[:, mc, :],
                                                    po[:], AluOpType.add)

                # ---------- router ----------
                xf = apool.tile([128, DC, TLOC], F32, tag="xf", name="xf")
                cwT = apool.tile([1, E, TLOC], F32, tag="cwT", name="cwT")
                with (tc.tile_pool(name="ps_l", bufs=2, space="PSUM") as ps_l,
                      tc.tile_pool(name="ps_x", bufs=3, space="PSUM") as ps_x):
                    rmsT(xf[:], xT[:], anw2[:], ps_l)
                    rtr = apool.tile([128, DC, E], F32, tag="rtr", name="rtr",
                                     bufs=2)
                    nc.sync.dma_start(rtr[:], r128(rtr_d.ap()[l]))
                    ohp = apool.tile([128, 2, E], F32, tag="ohp", name="ohp")
                    prb = apool.tile([128, 2, E], F32, tag="prb", name="prb")
                    for ti in range(2):
                        pl = ps_l.tile([128, E], F32, tag="pl", name="pl",
                                        bufs=1)
                        for kc in range(DC):
                            nc.tensor.matmul(
                                pl[:], xf[:, kc, ti * 128:(ti + 1) * 128],
                                rtr[:, kc, :], start=(kc == 0),
                                stop=(kc == DC - 1))
                        lg = apool.tile([128, E], F32, tag="lg", name="lg",
                                        bufs=2)
                        nc.vector.tensor_copy(lg[:], pl[:])
                        mx = apool.tile([128, 1], F32, tag="mx", name="mx",
                                        bufs=2)
                        nc.vector.tensor_reduce(mx[:], lg[:],
                                                mybir.AxisListType.X,
                                                AluOpType.max)
                        nc.vector.tensor_scalar(ohp[:, ti, :], lg[:], mx[:],
                                                None, AluOpType.is_equal)
                        nmx = apool.tile([128, 1], F32, tag="nmx", name="nmx",
                                         bufs=2)
                        nc.vector.tensor_scalar(nmx[:], mx[:], -1.0, None,
                                                AluOpType.mult)
                        rs = apool.tile([128, 1], F32, tag="rs", name="rs",
                                        bufs=2)
                        ex = apool.tile([128, E], F32, tag="ex", name="ex",
                                        bufs=2)
                        nc.scalar.activation(ex[:], lg[:], AF.Exp,
                                             bias=nmx[:], accum_out=rs[:])
                        rrs = apool.tile([128, 1], F32, tag="rrs", name="rrs",
                                         bufs=2)
                        nc.vector.reciprocal(rrs[:], rs[:])
                        nc.vector.tensor_scalar(prb[:, ti, :], ex[:], rrs[:],
                                                None, AluOpType.mult)
                    pf = ps_x.tile([1, E], F32, tag="pf", name="pf", bufs=1)
                    pp = ps_x.tile([1, E], F32, tag="pp", name="pp", bufs=1)
                    for ti in range(2):
                        nc.tensor.matmul(pf[:], ones_col[:], ohp[:, ti, :],
                                         start=(ti == 0), stop=(ti == 1))
                    for ti in range(2):
                        nc.tensor.matmul(pp[:], ones_col[:], prb[:, ti, :],
                                         start=(ti == 0), stop=(ti == 1))
                    auxs = apool.tile([1, 2, E], F32, tag="auxs", name="auxs")
                    nc.vector.tensor_copy(auxs[:, 0, :], pf[:])
                    nc.vector.tensor_copy(auxs[:, 1, :], pp[:])
                    nc.sync.dma_start(aux_d.ap()[l], auxs[:].opt())

                    for ti in range(2):
                        pt = ps_x.tile([E, 128], F32, tag="pt", name="pt",
                                         bufs=1)
                        nc.tensor.transpose(pt[:], ohp[:, ti, :], ident[:])
                        ptc = apool.tile([E, 128], F32, tag="ptc", name="ptc",
                                         bufs=2)
                        nc.vector.tensor_copy(ptc[:], pt[:])
                        nc.sync.dma_start(
                            cwT[0:1, :, ti * 128:(ti + 1) * 128],
                            ptc[:])

                # ---------- experts (dense, masked before w2) ----------
                # layers 0..L-2: bf16 hi/lo split matmuls (3 passes, ~16-bit
                # effective mantissa, fp32 accumulate) - flip-safe per sim.
                # last layer: plain fp16 (no routing downstream).
                eoutT = apool.tile([128, DC, TLOC], F32, tag="eoutT",
                                   name="eoutT")
                fp16x = (l == L - 1)
                edt = mybir.dt.float16 if fp16x else F32
                with (tc.tile_pool(name="wep", bufs=2) as wep,
                      tc.tile_pool(name="ps_h", bufs=3, space="PSUM") as ps_h,
                      tc.tile_pool(name="ps_eo", bufs=4,
                                   space="PSUM") as ps_eo):
                    if fp16x:
                        xfh = apool.tile([128, DC, TLOC], edt, tag="xfh",
                                         name="xfh")
                        nc.vector.tensor_copy(xfh[:].opt(), xf[:].opt())
                        xfl = None
                    else:
                        xfh = apool.tile([128, DC, TLOC], BF16, tag="xfh16",
                                         name="xfh16")
                        nc.vector.tensor_copy(xfh[:].opt(), xf[:].opt())
                        xfhf = apool.tile([128, DC, TLOC], F32, tag="xfhf",
                                          name="xfhf")
                        nc.vector.tensor_copy(xfhf[:].opt(), xfh[:].opt())
                        xfl = apool.tile([128, DC, TLOC], BF16, tag="xfl16",
                                         name="xfl16")
                        nc.vector.tensor_tensor(xfl[:].opt(), xf[:].opt(),
                                                xfhf[:].opt(),
                                                AluOpType.subtract)
                    eo = [ps_eo.tile([128, TLOC], F32, tag=f"eo{mc}",
                                     name=f"eo{mc}", bufs=1)
                          for mc in range(DC)]
                    for e_ in range(E):
                        if fp16x:
                            w1 = wep.tile([128, DC, DFE], edt, tag="ew1",
                                          name="ew1")
                            nc.sync.dma_start(w1[:], r128(ew1h_d.ap()[e_]))
                            w2 = wep.tile([128, FE, D], edt, tag="ew2",
                                          name="ew2")
                            nc.sync.dma_start(w2[:], r128(ew2h_d.ap()[e_]))
                        else:
                            w1 = wep.tile([128, DC, DFE], BF16, tag="ew1",
                                          name="ew1")
                            nc.sync.dma_start(w1[:],
                                              r128(ew1hi_d.ap()[l, e_]))
                            w1l = wep.tile([128, DC, DFE], BF16, tag="ew1l",
                                           name="ew1l")
                            nc.sync.dma_start(w1l[:],
                                              r128(ew1lo_d.ap()[l, e_]))
                            w2 = wep.tile([128, FE, D], BF16, tag="ew2",
                                          name="ew2")
                            nc.sync.dma_start(w2[:],
                                              r128(ew2hi_d.ap()[l, e_]))
                            w2l = wep.tile([128, FE, D], BF16, tag="ew2l",
                                           name="ew2l")
                            nc.sync.dma_start(w2l[:],
                                              r128(ew2lo_d.ap()[l, e_]))
                        bce = apool.tile([128, TLOC], F32, tag="bce",
                                         name="bce", bufs=2)
                        nc.gpsimd.partition_broadcast(bce[:],
                                                      cwT[0:1, e_, :])
                        for fc in range(FE):
                            ph = ps_h.tile([128, TLOC], F32, tag="ph",
                                           name="ph")
                            fsl = slice(fc * 128, (fc + 1) * 128)
                            if fp16x:
                                for kc in range(DC):
                                    nc.tensor.matmul(
                                        ph[:], w1[:, kc, fsl],
                                        xfh[:, kc, :], start=(kc == 0),
                                        stop=(kc == DC - 1))
                            else:
                                i_mm = 0
                                for kc in range(DC):
                                    for wt, xt in ((w1, xfh), (w1, xfl),
                                                   (w1l, xfh)):
                                        nc.tensor.matmul(
                                            ph[:], wt[:, kc, fsl],
                                            xt[:, kc, :], start=(i_mm == 0),
                                            stop=(i_mm == 3 * DC - 1))
                                        i_mm += 1
                            hr = apool.tile([128, TLOC], F32, tag="hr",
                                            name="hr", bufs=2)
                            nc.scalar.activation(hr[:], ph[:], AF.Gelu)
                            hs = apool.tile([128, TLOC], edt, tag="hs",
                                            name="hs", bufs=3)
                            nc.vector.tensor_tensor(hs[:], hr[:], bce[:],
                                                    AluOpType.mult)
                            if fp16x:
                                for mc in range(DC):
                                    nc.tensor.matmul(
                                        eo[mc][:],
                                        w2[:, fc, mc * 128:(mc + 1) * 128],
                                        hs[:],
                                        start=(e_ == 0 and fc == 0),
                                        stop=(e_ == E - 1 and fc == FE - 1))
                            else:
                                hsh = apool.tile([128, TLOC], BF16,
                                                 tag="hsh", name="hsh",
                                                 bufs=3)
                                nc.vector.tensor_copy(hsh[:], hs[:])
                                hshf = apool.tile([128, TLOC], F32,
                                                  tag="hshf", name="hshf",
                                                  bufs=1)
                                nc.vector.tensor_copy(hshf[:], hsh[:])
                                hsl = apool.tile([128, TLOC], BF16,
                                                 tag="hsl", name="hsl",
                                                 bufs=3)
                                nc.vector.tensor_tensor(hsl[:], hs[:],
                                                        hshf[:],
                                                        AluOpType.subtract)
                                for mc in range(DC):
                                    msl = slice(mc * 128, (mc + 1) * 128)
                                    for j, (wt, ht) in enumerate(
                                            ((w2, hsh), (w2, hsl),
                                             (w2l, hsh))):
                                        nc.tensor.matmul(
                                            eo[mc][:], wt[:, fc, msl], ht[:],
                                            start=(e_ == 0 and fc == 0
                                                   and j == 0),
                                            stop=(e_ == E - 1
                                                  and fc == FE - 1
                                                  and j == 2))
                    for mc in range(DC):
                        nc.vector.tensor_copy(eoutT[:, mc, :], eo[mc][:])

                # ---------- shared recurrent ----------
                rdt = mybir.dt.float16 if fp16x else F32
                with tc.tile_pool(name="wrp", bufs=1) as wrp:
                    srad = wrp.tile([128, ADC, D], rdt, tag="srad",
                                    name="srad")
                    nc.sync.dma_start(srad[:], r128(sradh_d.ap()) if fp16x
                                      else r128(srad_d.ap()[l]))
                    srwv = wrp.tile([128, DC, D], rdt, tag="srwv", name="srwv")
                    nc.sync.dma_start(srwv[:], r128(srwvh_d.ap()) if fp16x
                                      else r128(srwv_d.ap()[l]))
                    srwo = wrp.tile([128, DC, D], rdt, tag="srwo", name="srwo")
                    nc.sync.dma_start(srwo[:], r128(srwoh_d.ap()) if fp16x
                                      else r128(srwo_d.ap()[l]))
                    srm1 = wrp.tile([128, DC, DFM], rdt, tag="srm1",
                                    name="srm1")
                    nc.sync.dma_start(srm1[:], r128(srm1h_d.ap()) if fp16x
                                      else r128(srm1_d.ap()[l]))
                    srm2 = wrp.tile([128, FM, D], rdt, tag="srm2", name="srm2")
                    nc.sync.dma_start(srm2[:], r128(srm2h_d.ap()) if fp16x
                                      else r128(srm2_d.ap()[l]))
                    n1 = load_wcol(srnw_d.ap()[l, 0], "srn1")
                    n2 = load_wcol(srnw_d.ap()[l, 1], "srn2")
                    n3 = load_wcol(srnw_d.ap()[l, 2], "srn3")
                    n4 = load_wcol(srnw_d.ap()[l, 3], "srn4")

                    state = apool.tile([128, DC, TLOC], F32, tag="state",
                                       name="state")
                    nc.sync.dma_start(state[:], r128(s0T_d.ap()[l]))
                    if fp16x:
                        st16 = apool.tile([128, DC, TLOC], rdt, tag="st16",
                                          name="st16")
                        nc.vector.tensor_copy(st16[:].opt(), state[:].opt())
                        state = st16
                        eo16 = apool.tile([128, DC, TLOC], rdt, tag="eo16",
                                          name="eo16")
                        nc.vector.tensor_copy(eo16[:].opt(), eoutT[:].opt())
                        eoutT = eo16

                    for r_ in range(NR):
                        with (tc.tile_pool(name="ps_a", bufs=2,
                                           space="PSUM") as ps_a,
                              tc.tile_pool(name="ps_pm", bufs=4,
                                           space="PSUM") as ps_pm,
                              tc.tile_pool(name="ps_rr", bufs=2,
                                           space="PSUM") as ps_rr):
                            s1 = apool.tile([128, DC, TLOC], F32, tag="s1",
                                            name="s1")
                            for mc in range(DC):
                                pa = ps_a.tile([128, TLOC], F32, tag="pa",
                                               name="pa")
                                for kc in range(ADC):
                                    rhs = (state[:, kc, :] if kc < DC
                                           else eoutT[:, kc - DC, :])
                                    nc.tensor.matmul(
                                        pa[:],
                                        srad[:, kc, mc * 128:(mc + 1) * 128],
                                        rhs, start=(kc == 0),
                                        stop=(kc == ADC - 1))
                                nc.scalar.copy(s1[:, mc, :], pa[:])
                            sn = apool.tile([128, DC, TLOC], rdt, tag="sn",
                                            name="sn")
                            rmsT(sn[:], s1[:], n1[:], ps_rr)
                            av = apool.tile([128, DC, TLOC], rdt, tag="av",
                                            name="av")
                            for mc in range(DC):
                                pa = ps_a.tile([128, TLOC], F32, tag="pa",
                                               name="pa")
                                for kc in range(DC):
                                    nc.tensor.matmul(
                                        pa[:],
                                        srwv[:, kc, mc * 128:(mc + 1) * 128],
                                        sn[:, kc, :], start=(kc == 0),
                                        stop=(kc == DC - 1))
                                nc.scalar.copy(av[:, mc, :], pa[:])
                            r2 = apool.tile([128, DC, TLOC], F32, tag="r2",
                                            name="r2")
                            for mc in range(DC):
                                pa = ps_a.tile([128, TLOC], F32, tag="pa",
                                               name="pa")
                                for kc in range(DC):
                                    nc.tensor.matmul(
                                        pa[:],
                                        srwo[:, kc, mc * 128:(mc + 1) * 128],
                                        av[:, kc, :], start=(kc == 0),
                                        stop=(kc == DC - 1))
                                nc.vector.tensor_tensor(
                                    r2[:, mc, :], s1[:, mc, :], pa[:],
                                    AluOpType.add)
                            r2n = apool.tile([128, DC, TLOC], F32, tag="r2n",
                                             name="r2n")
                            rmsT(r2n[:], r2[:], n2[:], ps_rr)
                            s3n = apool.tile([128, DC, TLOC], rdt,
                                             tag="s3n", name="s3n")
                            rmsT(s3n[:], r2n[:], n3[:], ps_rr)
                            pm = [ps_pm.tile([128, TLOC], F32, tag=f"pm{mc}",
                                             name=f"pm{mc}", bufs=1)
                                  for mc in range(DC)]
                            for fc in range(FM):
                                pa = ps_a.tile([128, TLOC], F32, tag="pa",
                                               name="pa")
                                for kc in range(DC):
                                    nc.tensor.matmul(
                                        pa[:],
                                        srm1[:, kc, fc * 128:(fc + 1) * 128],
                                        s3n[:, kc, :], start=(kc == 0),
                                        stop=(kc == DC - 1))
                                ms = apool.tile([128, TLOC], rdt, tag="ms",
                                                name="ms", bufs=3)
                                nc.scalar.activation(ms[:], pa[:], AF.Silu)
                                for mc in range(DC):
                                    nc.tensor.matmul(
                                        pm[mc][:],
                                        srm2[:, fc, mc * 128:(mc + 1) * 128],
                                        ms[:], start=(fc == 0),
                                        stop=(fc == FM - 1))
                            r4 = apool.tile([128, DC, TLOC], F32, tag="r4",
                                            name="r4")
                            for mc in range(DC):
                                nc.vector.tensor_tensor(
                                    r4[:, mc, :], r2n[:, mc, :], pm[mc][:],
                                    AluOpType.add)
                            stn = apool.tile([128, DC, TLOC], rdt,
                                             tag="state_n", name="state_n")
                            rmsT(stn[:], r4[:], n4[:], ps_rr)
                            state = stn

                for mc in range(DC):
                    nc.vector.tensor_tensor(xT[:, mc, :], xT[:, mc, :],
                                            state[:, mc, :], AluOpType.add)

            # ---------- final norm + lm_head ----------
            nc.sync.dma_start(r128(dbg_d.ap()), xT[:])
            if DO_HEAD:
                onw = load_wcol(onw_d.ap(), "onw")
                xo = apool.tile([128, DC, TLOC], F32, tag="xo", name="xo")
                with tc.tile_pool(name="ps_f", bufs=2, space="PSUM") as ps_f:
                    rmsT(xo[:], xT[:], onw[:], ps_f)
                xoh = apool.tile([128, DC, TLOC], mybir.dt.float16,
                                 tag="xoh", name="xoh")
                nc.vector.tensor_copy(xoh[:].opt(), xo[:].opt())
                with (tc.tile_pool(name="wlm", bufs=3) as wlm,
                      tc.tile_pool(name="ps_lm", bufs=4,
                                   space="PSUM") as ps_lm):
                    NV = 512
                    for vs in range(0, V, NV):
                        nv = min(NV, V - vs)
                        lw = wlm.tile([128, DC, NV], mybir.dt.float16,
                                      tag="lmw", name="lmw")
                        nc.sync.dma_start(
                            lw[:, :, 0:nv],
                            lmh_d.ap()[:, vs:vs + nv].rearrange(
                                "(a p) f -> p a f", p=128))
                        for ti in range(2):
                            pl2 = ps_lm.tile([128, NV], F32, tag="plm",
                                             name="plm")
                            for kc in range(DC):
                                nc.tensor.matmul(
                                    pl2[:, 0:nv],
                                    xoh[:, kc, ti * 128:(ti + 1) * 128],
                                    lw[:, kc, 0:nv], start=(kc == 0),
                                    stop=(kc == DC - 1))
                            ot = apool.tile([128, NV], F32, tag="ot",
                                            name="ot", bufs=3)
                            nc.scalar.copy(ot[:, 0:nv], pl2[:, 0:nv])
                            nc.sync.dma_start(
                                logits_d.ap()[ti * 128:(ti + 1) * 128,
                                              vs:vs + nv],
                                ot[:, 0:nv])
        finally:
            for p in reversed(octx):
                p.__exit__(None, None, None)

    nc.compile()
    return nc


def _state_inits():
    import jax
    import jax.numpy as jnp
    key = jax.random.key(42)
    out = []
    for i in range(L):
        s = jax.random.normal(jax.random.fold_in(key, i), (T, D), jnp.float32)
        out.append(np.asarray(s) * np.float32(0.02))
    return out


def kernel(input_ids, params):
    global LAST_RESULT
    input_ids = np.asarray(input_ids)
    p = params

    if "nc" not in _CACHE:
        _CACHE["nc"] = build()
    nc = _CACHE["nc"]

    tok = np.asarray(p["tok_emb"], dtype=np.float32)
    pos = np.asarray(p["pos_emb"], dtype=np.float32)[:S]
    x0 = tok[np.asarray(input_ids).reshape(-1)].reshape(B, S, D) + pos[None]
    x0 = x0.reshape(T, D)

    s0 = _state_inits()

    tri = np.zeros((128, 128), dtype=np.float32)
    for k_ in range(128):
        tri[k_, :k_] = -1.0e30

    def f32(a):
        return np.ascontiguousarray(np.asarray(a, dtype=np.float32))

    layers = p["layers"]
    import ml_dtypes
    _e1 = f32(np.stack([lp["exp_w1"] for lp in layers]))[:L - 1]
    _e2 = f32(np.stack([lp["exp_w2"] for lp in layers]))[:L - 1]
    _hi1 = np.ascontiguousarray(_e1.astype(ml_dtypes.bfloat16))
    _lo1 = np.ascontiguousarray(
        (_e1 - _hi1.astype(np.float32)).astype(ml_dtypes.bfloat16))
    _hi2 = np.ascontiguousarray(_e2.astype(ml_dtypes.bfloat16))
    _lo2 = np.ascontiguousarray(
        (_e2 - _hi2.astype(np.float32)).astype(ml_dtypes.bfloat16))
    shared = {
        "wo": f32(np.stack([lp["attn_wo"] for lp in layers])),
        "anw": f32(np.stack([[lp["norm1"], lp["norm2"]] for lp in layers])),
        "rtr": f32(np.stack([lp["router"] for lp in layers])),
        "srad": f32(np.stack([lp["sr"]["adapter"] for lp in layers])),
        "srwv": f32(np.stack([lp["sr"]["attn_wv"] for lp in layers])),
        "srwo": f32(np.stack([lp["sr"]["attn_wo"] for lp in layers])),
        "srm1": f32(np.stack([lp["sr"]["mlp_w1"] for lp in layers])),
        "srm2": f32(np.stack([lp["sr"]["mlp_w2"] for lp in layers])),
        "srnw": f32(np.stack([[lp["sr"]["n1"], lp["sr"]["n2"],
                               lp["sr"]["n3"], lp["sr"]["n4"]]
                              for lp in layers])),
        "onw": f32(p["norm_out"]),
        "lmh": np.ascontiguousarray(
            np.asarray(p["lm_head"]).astype(np.float16)),
        "tri": tri,
        "ew1hi": _hi1, "ew1lo": _lo1, "ew2hi": _hi2, "ew2lo": _lo2,
        "ew1h": np.ascontiguousarray(
            np.asarray(layers[L - 1]["exp_w1"]).astype(np.float16)),
        "ew2h": np.ascontiguousarray(
            np.asarray(layers[L - 1]["exp_w2"]).astype(np.float16)),
        "sradh": np.ascontiguousarray(
            np.asarray(layers[L - 1]["sr"]["adapter"]).astype(np.float16)),
        "srwvh": np.ascontiguousarray(
            np.asarray(layers[L - 1]["sr"]["attn_wv"]).astype(np.float16)),
        "srwoh": np.ascontiguousarray(
            np.asarray(layers[L - 1]["sr"]["attn_wo"]).astype(np.float16)),
        "srm1h": np.ascontiguousarray(
            np.asarray(layers[L - 1]["sr"]["mlp_w1"]).astype(np.float16)),
        "srm2h": np.ascontiguousarray(
            np.asarray(layers[L - 1]["sr"]["mlp_w2"]).astype(np.float16)),
    }
    wq = f32(np.stack([lp["attn_wq"] for lp in layers]))
    wk = f32(np.stack([lp["attn_wk"] for lp in layers]))
    wv = f32(np.stack([lp["attn_wv"] for lp in layers]))

    in_maps = []
    for c in range(NCORES):
        rows = np.concatenate([np.arange(b_ * 128, b_ * 128 + 128)
                               for b_ in CORE_BLOCKS[c]])
        im = dict(shared)
        im["x0T"] = np.ascontiguousarray(x0[rows].T)
        im["s0T"] = np.ascontiguousarray(
            np.stack([s0[li][rows].T for li in range(L)]))
        hs = slice(c * HD, (c + 1) * HD)
        im["wqh"] = np.ascontiguousarray(wq[:, :, hs])
        im["wkh"] = np.ascontiguousarray(wk[:, :, hs])
        im["wvh"] = np.ascontiguousarray(wv[:, :, hs])
        in_maps.append(im)

    res = bass_utils.run_bass_kernel_spmd(
        nc, in_maps, core_ids=list(range(NCORES)))
    LAST_RESULT = res

    logits = np.zeros((T, V), dtype=np.float32)
    fsum = np.zeros((L, E), dtype=np.float64)
    psum = np.zeros((L, E), dtype=np.float64)
    for c in range(NCORES):
        o = res.results[c]
        lg = o["logits"]
        for j, b_ in enumerate(CORE_BLOCKS[c]):
            logits[b_ * 128:(b_ + 1) * 128] = lg[j * 128:(j + 1) * 128]
        fsum += o["aux"][:, 0, :]
        psum += o["aux"][:, 1, :]

    total_aux = np.float32(0.0)
    for li in range(L):
        f = (fsum[li] / T).astype(np.float32)
        pr = (psum[li] / T).astype(np.float32)
        total_aux = np.float32(total_aux + np.float32(E) *
                               np.float32(np.sum(f * pr, dtype=np.float32)))
    return logits.reshape(B, S, V), total_aux


# revision 17
# speedup vs baseline: 1.2961x; 1.0394x over previous
"""Trainium2 Bass kernel for nn_MoREModelSynthesisIOptionB (moe_routing).

Sharding: 8 NeuronCores. Token-data-parallel for MoE/recurrent/lm_head
(core c owns token blocks {c, 15-c} of 128 tokens), head-parallel for
attention (core c owns head c; head weight slices are passed as per-core
input data so the compiled program is identical on every core).
Activations are kept transposed ([d, t]) so weight matrices serve as the
stationary matmul operand exactly as stored. Dense expert dispatch with
the one-hot top-1 combine mask applied to the gelu output before the w2
matmul (PSUM accumulates over experts). fp32 matmuls throughout.
Collectives per layer: AllGather of normed x (attention input), AllToAll
of per-head attention outputs back to token shards.
"""
import os
import sys
import numpy as np

sys.path.insert(0, "/opt/trn_rl_repo")
sys.path.insert(0, "/opt/trn_rl_repo/concourse")

from concourse import bass, bacc, tile, mybir, masks  # noqa: E402
from concourse import bass_utils  # noqa: E402
from concourse.alu_op_type import AluOpType  # noqa: E402

AF = mybir.ActivationFunctionType
F32 = mybir.dt.float32

NCORES = 8
B, S, V, D, H, L, E, NR = 2, 1024, 32000, 512, 8, 4, 8, 2
DFE = 2 * D
DFM = 4 * D
HD = D // H
T = B * S
NBLK = T // 128
TLOC = 256
DC = D // 128        # 4
FE = DFE // 128      # 8
FM = DFM // 128      # 16
ADC = 2 * D // 128   # 8
EPS = 1e-6

CORE_BLOCKS = [[c, NBLK - 1 - c] for c in range(NCORES)]
BLK_SRC = [(m, 0) if m < NCORES else (NBLK - 1 - m, 1) for m in range(NBLK)]

N_LAYERS = int(os.environ.get("KLAYERS", str(L)))
DO_HEAD = os.environ.get("KHEAD", "1") == "1"

_CACHE = {}
LAST_RESULT = None


def build():
    nc = bacc.Bacc("TRN2", target_bir_lowering=False, debug=False,
                   enable_asserts=False, num_devices=NCORES)

    def din(name, shape):
        return nc.dram_tensor(name, list(shape), F32, kind="ExternalInput")

    x0T_d = din("x0T", [D, TLOC])
    s0T_d = din("s0T", [L, D, TLOC])
    wqh_d = din("wqh", [L, D, HD])
    wkh_d = din("wkh", [L, D, HD])
    wvh_d = din("wvh", [L, D, HD])
    wo_d = din("wo", [L, D, D])
    anw_d = din("anw", [L, 2, D])
    rtr_d = din("rtr", [L, D, E])
    BF16 = mybir.dt.bfloat16
    ew1hi_d = nc.dram_tensor("ew1hi", [L - 1, E, D, DFE], BF16,
                             kind="ExternalInput")
    ew1lo_d = nc.dram_tensor("ew1lo", [L - 1, E, D, DFE], BF16,
                             kind="ExternalInput")
    ew2hi_d = nc.dram_tensor("ew2hi", [L - 1, E, DFE, D], BF16,
                             kind="ExternalInput")
    ew2lo_d = nc.dram_tensor("ew2lo", [L - 1, E, DFE, D], BF16,
                             kind="ExternalInput")
    srad_d = din("srad", [L, 2 * D, D])
    srwv_d = din("srwv", [L, D, D])
    srwo_d = din("srwo", [L, D, D])
    srm1_d = din("srm1", [L, D, DFM])
    srm2_d = din("srm2", [L, DFM, D])
    srnw_d = din("srnw", [L, 4, D])
    onw_d = din("onw", [D])
    lmh_d = nc.dram_tensor("lmh", [D, V], mybir.dt.float16,
                           kind="ExternalInput")
    tri_d = din("tri", [128, 128])
    F16 = mybir.dt.float16
    ew1h_d = nc.dram_tensor("ew1h", [E, D, DFE], F16, kind="ExternalInput")
    ew2h_d = nc.dram_tensor("ew2h", [E, DFE, D], F16, kind="ExternalInput")
    sradh_d = nc.dram_tensor("sradh", [2 * D, D], F16, kind="ExternalInput")
    srwvh_d = nc.dram_tensor("srwvh", [D, D], F16, kind="ExternalInput")
    srwoh_d = nc.dram_tensor("srwoh", [D, D], F16, kind="ExternalInput")
    srm1h_d = nc.dram_tensor("srm1h", [D, DFM], F16, kind="ExternalInput")
    srm2h_d = nc.dram_tensor("srm2h", [DFM, D], F16, kind="ExternalInput")

    logits_d = nc.dram_tensor("logits", [TLOC, V], F32, kind="ExternalOutput")
    aux_d = nc.dram_tensor("aux", [L, 2, E], F32, kind="ExternalOutput")
    dbg_d = nc.dram_tensor("dbg", [D, TLOC], F32, kind="ExternalOutput")

    rg = [list(range(NCORES))]

    def r128(ap):
        return ap.rearrange("(a p) f -> p a f", p=128)

    with tile.TileContext(nc) as tc:
        octx = [
            tc.tile_pool(name="cpool", bufs=1),
            tc.tile_pool(name="wpool", bufs=2),
            tc.tile_pool(name="apool", bufs=1),
            tc.tile_pool(name="dram", bufs=2, space="DRAM"),
        ]
        cpool, wpool, apool, dram = [p.__enter__() for p in octx]
        try:
            ident = cpool.tile([128, 128], F32)
            masks.make_identity(nc, ident[:])
            ones_col = cpool.tile([128, 1], F32)
            nc.vector.memset(ones_col[:], 1.0)
            tri = cpool.tile([128, 128], F32)
            nc.sync.dma_start(tri[:], tri_d.ap())

            xT = cpool.tile([128, DC, TLOC], F32, name="xT")
            nc.sync.dma_start(xT[:], r128(x0T_d.ap()))

            def rmsT(dst, src, w_col, ps_r):
                sq = apool.tile([128, DC, TLOC], F32, tag="rms_sq",
                                name="rms_sq")
                for kc in range(DC):
                    nc.vector.tensor_tensor(sq[:, kc, :], src[:, kc, :],
                                            src[:, kc, :], AluOpType.mult)
                ss = ps_r.tile([1, TLOC], F32, tag="rms_ss", name="rms_ss",
                               bufs=2)
                for kc in range(DC):
                    nc.tensor.matmul(ss[:], ones_col[:], sq[:, kc, :],
                                     start=(kc == 0), stop=(kc == DC - 1))
                st = apool.tile([1, TLOC], F32, tag="rms_st", name="rms_st",
                                bufs=2)
                nc.vector.tensor_scalar(st[:], ss[:], 1.0 / D, EPS,
                                        AluOpType.mult, AluOpType.add)
                st2 = apool.tile([1, TLOC], F32, tag="rms_st2", name="rms_st2",
                                 bufs=2)
                nc.scalar.sqrt(st2[:], st[:])
                st3 = apool.tile([1, TLOC], F32, tag="rms_st3", name="rms_st3",
                                 bufs=2)
                nc.vector.reciprocal(st3[:], st2[:])
                bc = apool.tile([128, TLOC], F32, tag="rms_bc", name="rms_bc",
                                bufs=2)
                nc.gpsimd.partition_broadcast(bc[:], st3[:])
                for kc in range(DC):
                    nc.vector.scalar_tensor_tensor(
                        dst[:, kc, :], src[:, kc, :], w_col[:, kc:kc + 1],
                        bc[:], AluOpType.mult, AluOpType.mult)

            def load_wcol(dram_ap, tag):
                t = wpool.tile([128, DC], F32, tag=tag, name=tag)
                nc.sync.dma_start(t[:], dram_ap.rearrange("(a p) -> p a",
                                                          p=128))
                return t

            # ================= layers =================
            for l in range(N_LAYERS):
                anw1 = load_wcol(anw_d.ap()[l, 0], "anw1")
                anw2 = load_wcol(anw_d.ap()[l, 1], "anw2")

                # ---------- attention ----------
                with (tc.tile_pool(name="aap", bufs=1) as aap,
                      tc.tile_pool(name="ps_r", bufs=2, space="PSUM") as ps_r):
                    wqh = aap.tile([128, DC, HD], F32, tag="wqh", name="wqh")
                    nc.sync.dma_start(wqh[:], r128(wqh_d.ap()[l]))
                    wkh = aap.tile([128, DC, HD], F32, tag="wkh", name="wkh")
                    nc.sync.dma_start(wkh[:], r128(wkh_d.ap()[l]))
                    wvh = aap.tile([128, DC, HD], F32, tag="wvh", name="wvh")
                    nc.sync.dma_start(wvh[:], r128(wvh_d.ap()[l]))
                    wo = aap.tile([128, DC, D], F32, tag="wo", name="wo")
                    nc.sync.dma_start(wo[:], r128(wo_d.ap()[l]))

                    xn = aap.tile([128, DC, TLOC], F32, tag="xn", name="xn")
                    rmsT(xn[:], xT[:], anw1[:], ps_r)

                    ag_in = dram.tile([D * TLOC], F32, tag="ag_in",
                                      name="ag_in")
                    nc.sync.dma_start(
                        ag_in[:].rearrange("(a p f) -> p a f", p=128, a=DC),
                        xn[:])
                    ag_out = dram.tile([NCORES, D * TLOC], F32, tag="ag_out",
                                       name="ag_out", addr_space="Shared")
                    nc.gpsimd.collective_compute(
                        "AllGather", AluOpType.bypass, replica_groups=rg,
                        ins=[ag_in.opt()], outs=[ag_out.opt()])

                    qT = aap.tile([64, NCORES, TLOC], F32, tag="qT", name="qT")
                    kT = aap.tile([64, NCORES, TLOC], F32, tag="kT", name="kT")
                    v2 = aap.tile([128, NCORES, 2, HD + 1], F32, tag="v2",
                                  name="v2")
                    nc.vector.memset(v2[:, :, :, HD:HD + 1], 1.0)
                    with tc.tile_pool(name="ps_qk", bufs=3,
                                      space="PSUM") as ps_qk:
                        for s_ in range(NCORES):
                            xa = aap.tile([128, DC, TLOC], F32, tag="xa",
                                          name="xa", bufs=2)
                            nc.sync.dma_start(
                                xa[:],
                                ag_out[:].rearrange(
                                    "s (a p f) -> s p a f", p=128, a=DC)[s_])
                            pq = ps_qk.tile([64, TLOC], F32, tag="pqk",
                                            name="pq")
                            pk = ps_qk.tile([64, TLOC], F32, tag="pqk",
                                            name="pk")
                            for kc in range(DC):
                                nc.tensor.matmul(pq[:], wqh[:, kc, :],
                                                 xa[:, kc, :], start=(kc == 0),
                                                 stop=(kc == DC - 1))
                            for kc in range(DC):
                                nc.tensor.matmul(pk[:], wkh[:, kc, :],
                                                 xa[:, kc, :], start=(kc == 0),
                                                 stop=(kc == DC - 1))
                            nc.scalar.activation(
                                qT[:, s_, :], pq[:], AF.Copy,
                                scale=1.0 / float(np.sqrt(HD)))
                            nc.scalar.copy(kT[:, s_, :], pk[:])
                            for ti in range(2):
                                pv = ps_qk.tile([128, TLOC], F32, tag="pqk",
                                                name="pv")[:, 0:HD]
                                for kc in range(DC):
                                    nc.tensor.matmul(
                                        pv[:],
                                        xa[:, kc, ti * 128:(ti + 1) * 128],
                                        wvh[:, kc, :], start=(kc == 0),
                                        stop=(kc == DC - 1))
                                nc.scalar.copy(v2[:, s_, ti, 0:HD], pv[:])

                    ao = aap.tile([64, NCORES, TLOC], F32, tag="ao", name="ao")
                    den = aap.tile([1, NCORES, TLOC], F32, tag="den",
                                   name="den")
                    with (tc.tile_pool(name="ps_s", bufs=4,
                                       space="PSUM") as ps_s,
                          tc.tile_pool(name="ps_pv", bufs=2,
                                       space="PSUM") as ps_pv):
                        for qb in range(NBLK):
                            batch, qpos = qb // NCORES, qb % NCORES
                            qs, qh = BLK_SRC[qb]
                            q_ap = qT[:, qs, qh * 128:(qh + 1) * 128]
                            pv = ps_pv.tile([HD + 1, 128], F32, tag="pv_acc",
                                            name="pv_acc")
                            nkb = qpos + 1
                            for kb in range(nkb):
                                m = batch * NCORES + kb
                                ks, kh = BLK_SRC[m]
                                st = ps_s.tile([128, 128], F32, tag="st",
                                               name="st")
                                nc.tensor.matmul(
                                    st[:],
                                    kT[:, ks, kh * 128:(kh + 1) * 128],
                                    q_ap, start=True, stop=True)
                                if kb == nkb - 1:
                                    nc.vector.tensor_tensor(
                                        st[:], st[:], tri[:], AluOpType.add)
                                es = aap.tile([128, 128], F32, tag="es",
                                              name="es", bufs=3)
                                nc.scalar.activation(es[:], st[:], AF.Exp)
                                nc.tensor.matmul(
                                    pv[:], v2[:, ks, kh, :], es[:],
                                    start=(kb == 0), stop=(kb == nkb - 1))
                            nc.scalar.copy(
                                ao[:, qs, qh * 128:(qh + 1) * 128],
                                pv[0:HD, :])
                            nc.scalar.copy(
                                den[:, qs, qh * 128:(qh + 1) * 128],
                                pv[HD:HD + 1, :])

                    rden = aap.tile([1, NCORES, TLOC], F32, tag="rden",
                                    name="rden")
                    rbc = aap.tile([64, NCORES, TLOC], F32, tag="rbc",
                                   name="rbc")
                    aos = aap.tile([64, NCORES, TLOC], F32, tag="aos",
                                   name="aos")
                    atT = aap.tile([128, DC, TLOC], F32, tag="atT", name="atT")
                    # A2A split in two halves: half 0 (q-blocks 0-7, the
                    # first 8 of the qb loop) ships while the PE computes
                    # scores for q-blocks 8-15.
                    for hf in range(2):
                        hsl = slice(hf * 128, (hf + 1) * 128)
                        nc.vector.reciprocal(rden[:, :, hsl].opt(),
                                             den[:, :, hsl].opt())
                        nc.gpsimd.partition_broadcast(
                            rbc[:, :, hsl].opt(), rden[:, :, hsl].opt(),
                            channels=64)
                        nc.vector.tensor_tensor(aos[:, :, hsl].opt(),
                                                ao[:, :, hsl].opt(),
                                                rbc[:, :, hsl].opt(),
                                                AluOpType.mult)
                        a2a_in = dram.tile([NCORES, 64 * 128], F32,
                                           tag=f"a2a_in{hf}",
                                           name=f"a2a_in{hf}")
                        for s_ in range(NCORES):
                            nc.sync.dma_start(
                                a2a_in[:].rearrange("s (p f) -> s p f",
                                                    p=64)[s_],
                                aos[:, s_, hsl])
                        a2a_out = dram.tile([NCORES, 64 * 128], F32,
                                            tag=f"a2a_out{hf}",
                                            name=f"a2a_out{hf}")
                        nc.gpsimd.collective_compute(
                            "AllToAll", AluOpType.bypass, replica_groups=rg,
                            ins=[a2a_in.opt()], outs=[a2a_out.opt()])
                        nc.sync.dma_start(
                            atT[:, :, hsl],
                            a2a_out[:].rearrange("s (p f) -> (s p) f", p=64)
                            .rearrange("(a p) f -> p a f", p=128))

                    with tc.tile_pool(name="ps_o", bufs=2,
                                      space="PSUM") as ps_o:
                        for mc in range(DC):
                            po = ps_o.tile([128, TLOC], F32, tag="po",
                                           name="po")
                            for kc in range(DC):
                                nc.tensor.matmul(
                                    po[:], wo[:, kc, mc * 128:(mc + 1) * 128],
                                    atT[:, kc, :], start=(kc == 0),
                                    stop=(kc == DC - 1))
                            nc.vector.tensor_tensor(xT[:, mc, :], xT[:, mc, :],
                                                    po[:], AluOpType.add)

                # ---------- router ----------
                xf = apool.tile([128, DC, TLOC], F32, tag="xf", name="xf")
                cwT = apool.tile([1, E, TLOC], F32, tag="cwT", name="cwT")
                with (tc.tile_pool(name="ps_l", bufs=2, space="PSUM") as ps_l,
                      tc.tile_pool(name="ps_x", bufs=3, space="PSUM") as ps_x):
                    rmsT(xf[:], xT[:], anw2[:], ps_l)
                    rtr = apool.tile([128, DC, E], F32, tag="rtr", name="rtr",
                                     bufs=2)
                    nc.sync.dma_start(rtr[:], r128(rtr_d.ap()[l]))
                    ohp = apool.tile([128, 2, E], F32, tag="ohp", name="ohp")
                    prb = apool.tile([128, 2, E], F32, tag="prb", name="prb")
                    for ti in range(2):
                        pl = ps_l.tile([128, E], F32, tag="pl", name="pl",
                                        bufs=1)
                        for kc in range(DC):
                            nc.tensor.matmul(
                                pl[:], xf[:, kc, ti * 128:(ti + 1) * 128],
                                rtr[:, kc, :], start=(kc == 0),
                                stop=(kc == DC - 1))
                        lg = apool.tile([128, E], F32, tag="lg", name="lg",
                                        bufs=2)
                        nc.vector.tensor_copy(lg[:], pl[:])
                        mx = apool.tile([128, 1], F32, tag="mx", name="mx",
                                        bufs=2)
                        nc.vector.tensor_reduce(mx[:], lg[:],
                                                mybir.AxisListType.X,
                                                AluOpType.max)
                        nc.vector.tensor_scalar(ohp[:, ti, :], lg[:], mx[:],
                                                None, AluOpType.is_equal)
                        nmx = apool.tile([128, 1], F32, tag="nmx", name="nmx",
                                         bufs=2)
                        nc.vector.tensor_scalar(nmx[:], mx[:], -1.0, None,
                                                AluOpType.mult)
                        rs = apool.tile([128, 1], F32, tag="rs", name="rs",
                                        bufs=2)
                        ex = apool.tile([128, E], F32, tag="ex", name="ex",
                                        bufs=2)
                        nc.scalar.activation(ex[:], lg[:], AF.Exp,
                                             bias=nmx[:], accum_out=rs[:])
                        rrs = apool.tile([128, 1], F32, tag="rrs", name="rrs",
                                         bufs=2)
                        nc.vector.reciprocal(rrs[:], rs[:])
                        nc.vector.tensor_scalar(prb[:, ti, :], ex[:], rrs[:],
                                                None, AluOpType.mult)
                    pf = ps_x.tile([1, E], F32, tag="pf", name="pf", bufs=1)
                    pp = ps_x.tile([1, E], F32, tag="pp", name="pp", bufs=1)
                    for ti in range(2):
                        nc.tensor.matmul(pf[:], ones_col[:], ohp[:, ti, :],
                                         start=(ti == 0), stop=(ti == 1))
                    for ti in range(2):
                        nc.tensor.matmul(pp[:], ones_col[:], prb[:, ti, :],
                                         start=(ti == 0), stop=(ti == 1))
                    auxs = apool.tile([1, 2, E], F32, tag="auxs", name="auxs")
                    nc.vector.tensor_copy(auxs[:, 0, :], pf[:])
                    nc.vector.tensor_copy(auxs[:, 1, :], pp[:])
                    nc.sync.dma_start(aux_d.ap()[l], auxs[:].opt())

                    for ti in range(2):
                        pt = ps_x.tile([E, 128], F32, tag="pt", name="pt",
                                         bufs=1)
                        nc.tensor.transpose(pt[:], ohp[:, ti, :], ident[:])
                        ptc = apool.tile([E, 128], F32, tag="ptc", name="ptc",
                                         bufs=2)
                        nc.vector.tensor_copy(ptc[:], pt[:])
                        nc.sync.dma_start(
                            cwT[0:1, :, ti * 128:(ti + 1) * 128],
                            ptc[:])

                # ---------- experts (dense, masked before w2) ----------
                # layers 0..L-2: bf16 hi/lo split matmuls (3 passes, ~16-bit
                # effective mantissa, fp32 accumulate) - flip-safe per sim.
                # last layer: plain fp16 (no routing downstream).
                eoutT = apool.tile([128, DC, TLOC], F32, tag="eoutT",
                                   name="eoutT")
                fp16x = (l == L - 1)
                edt = mybir.dt.float16 if fp16x else F32
                with (tc.tile_pool(name="wep", bufs=2) as wep,
                      tc.tile_pool(name="ps_h", bufs=3, space="PSUM") as ps_h,
                      tc.tile_pool(name="ps_eo", bufs=4,
                                   space="PSUM") as ps_eo):
                    if fp16x:
                        xfh = apool.tile([128, DC, TLOC], edt, tag="xfh",
                                         name="xfh")
                        nc.vector.tensor_copy(xfh[:].opt(), xf[:].opt())
                        xfl = None
                    else:
                        xfh = apool.tile([128, DC, TLOC], BF16, tag="xfh16",
                                         name="xfh16")
                        nc.vector.tensor_copy(xfh[:].opt(), xf[:].opt())
                        xfhf = apool.tile([128, DC, TLOC], F32, tag="xfhf",
                                          name="xfhf")
                        nc.vector.tensor_copy(xfhf[:].opt(), xfh[:].opt())
                        xfl = apool.tile([128, DC, TLOC], BF16, tag="xfl16",
                                         name="xfl16")
                        nc.vector.tensor_tensor(xfl[:].opt(), xf[:].opt(),
                                                xfhf[:].opt(),
                                                AluOpType.subtract)
                    eo = [ps_eo.tile([128, TLOC], F32, tag=f"eo{mc}",
                                     name=f"eo{mc}", bufs=1)
                          for mc in range(DC)]
                    for e_ in range(E):
                        if fp16x:
                            w1 = wep.tile([128, DC, DFE], edt, tag="ew1",
                                          name="ew1")
                            nc.sync.dma_start(w1[:], r128(ew1h_d.ap()[e_]))
                            w2 = wep.tile([128, FE, D], edt, tag="ew2",
                                          name="ew2")
                            nc.sync.dma_start(w2[:], r128(ew2h_d.ap()[e_]))
                        else:
                            w1 = wep.tile([128, DC, DFE], BF16, tag="ew1",
                                          name="ew1")
                            nc.sync.dma_start(w1[:],
                                              r128(ew1hi_d.ap()[l, e_]))
                            w1l = wep.tile([128, DC, DFE], BF16, tag="ew1l",
                                           name="ew1l")
                            nc.sync.dma_start(w1l[:],
                                              r128(ew1lo_d.ap()[l, e_]))
                            w2 = wep.tile([128, FE, D], BF16, tag="ew2",
                                          name="ew2")
                            nc.sync.dma_start(w2[:],
                                              r128(ew2hi_d.ap()[l, e_]))
                            w2l = wep.tile([128, FE, D], BF16, tag="ew2l",
                                           name="ew2l")
                            nc.sync.dma_start(w2l[:],
                                              r128(ew2lo_d.ap()[l, e_]))
                        bce = apool.tile([128, TLOC], F32, tag="bce",
                                         name="bce", bufs=2)
                        nc.gpsimd.partition_broadcast(bce[:],
                                                      cwT[0:1, e_, :])
                        for fc in range(FE):
                            ph = ps_h.tile([128, TLOC], F32, tag="ph",
                                           name="ph")
                            fsl = slice(fc * 128, (fc + 1) * 128)
                            if fp16x:
                                for kc in range(DC):
                                    nc.tensor.matmul(
                                        ph[:], w1[:, kc, fsl],
                                        xfh[:, kc, :], start=(kc == 0),
                                        stop=(kc == DC - 1))
                            else:
                                i_mm = 0
                                for kc in range(DC):
                                    for wt, xt in ((w1, xfh), (w1, xfl),
                                                   (w1l, xfh)):
                                        nc.tensor.matmul(
                                            ph[:], wt[:, kc, fsl],
                                            xt[:, kc, :], start=(i_mm == 0),
                                            stop=(i_mm == 3 * DC - 1))
                                        i_mm += 1
                            hr = apool.tile([128, TLOC], F32, tag="hr",
                                            name="hr", bufs=2)
                            nc.scalar.activation(hr[:], ph[:], AF.Gelu)
                            hs = apool.tile([128, TLOC], edt, tag="hs",
                                            name="hs", bufs=3)
                            nc.vector.tensor_tensor(hs[:], hr[:], bce[:],
                                                    AluOpType.mult)
                            if fp16x:
                                for mc in range(DC):
                                    nc.tensor.matmul(
                                        eo[mc][:],
                                        w2[:, fc, mc * 128:(mc + 1) * 128],
                                        hs[:],
                                        start=(e_ == 0 and fc == 0),
                                        stop=(e_ == E - 1 and fc == FE - 1))
                            else:
                                hsh = apool.tile([128, TLOC], BF16,
                                                 tag="hsh", name="hsh",
                                                 bufs=3)
                                nc.vector.tensor_copy(hsh[:], hs[:])
                                hshf = apool.tile([128, TLOC], F32,
                                                  tag="hshf", name="hshf",
                                                  bufs=1)
                                nc.vector.tensor_copy(hshf[:], hsh[:])
                                hsl = apool.tile([128, TLOC], BF16,
                                                 tag="hsl", name="hsl",
                                                 bufs=3)
                                nc.vector.tensor_tensor(hsl[:], hs[:],
                                                        hshf[:],
                                                        AluOpType.subtract)
                                for mc in range(DC):
                                    msl = slice(mc * 128, (mc + 1) * 128)
                                    for j, (wt, ht) in enumerate(
                                            ((w2, hsh), (w2, hsl),
                                             (w2l, hsh))):
                                        nc.tensor.matmul(
                                            eo[mc][:], wt[:, fc, msl], ht[:],
                                            start=(e_ == 0 and fc == 0
                                                   and j == 0),
                                            stop=(e_ == E - 1
                                                  and fc == FE - 1
                                                  and j == 2))
                    for mc in range(DC):
                        nc.vector.tensor_copy(eoutT[:, mc, :], eo[mc][:])

                # ---------- shared recurrent ----------
                rdt = mybir.dt.float16 if fp16x else F32
                with tc.tile_pool(name="wrp", bufs=1) as wrp:
                    srad = wrp.tile([128, ADC, D], rdt, tag="srad",
                                    name="srad")
                    nc.sync.dma_start(srad[:], r128(sradh_d.ap()) if fp16x
                                      else r128(srad_d.ap()[l]))
                    srwv = wrp.tile([128, DC, D], rdt, tag="srwv", name="srwv")
                    nc.sync.dma_start(srwv[:], r128(srwvh_d.ap()) if fp16x
                                      else r128(srwv_d.ap()[l]))
                    srwo = wrp.tile([128, DC, D], rdt, tag="srwo", name="srwo")
                    nc.sync.dma_start(srwo[:], r128(srwoh_d.ap()) if fp16x
                                      else r128(srwo_d.ap()[l]))
                    srm1 = wrp.tile([128, DC, DFM], rdt, tag="srm1",
                                    name="srm1")
                    nc.sync.dma_start(srm1[:], r128(srm1h_d.ap()) if fp16x
                                      else r128(srm1_d.ap()[l]))
                    srm2 = wrp.tile([128, FM, D], rdt, tag="srm2", name="srm2")
                    nc.sync.dma_start(srm2[:], r128(srm2h_d.ap()) if fp16x
                                      else r128(srm2_d.ap()[l]))
                    n1 = load_wcol(srnw_d.ap()[l, 0], "srn1")
                    n2 = load_wcol(srnw_d.ap()[l, 1], "srn2")
                    n3 = load_wcol(srnw_d.ap()[l, 2], "srn3")
                    n4 = load_wcol(srnw_d.ap()[l, 3], "srn4")

                    state = apool.tile([128, DC, TLOC], F32, tag="state",
                                       name="state")
                    nc.sync.dma_start(state[:], r128(s0T_d.ap()[l]))
                    if fp16x:
                        st16 = apool.tile([128, DC, TLOC], rdt, tag="st16",
                                          name="st16")
                        nc.vector.tensor_copy(st16[:].opt(), state[:].opt())
                        state = st16
                        eo16 = apool.tile([128, DC, TLOC], rdt, tag="eo16",
                                          name="eo16")
                        nc.vector.tensor_copy(eo16[:].opt(), eoutT[:].opt())
                        eoutT = eo16

                    for r_ in range(NR):
                        with (tc.tile_pool(name="ps_a", bufs=2,
                                           space="PSUM") as ps_a,
                              tc.tile_pool(name="ps_pm", bufs=4,
                                           space="PSUM") as ps_pm,
                              tc.tile_pool(name="ps_rr", bufs=2,
                                           space="PSUM") as ps_rr):
                            s1 = apool.tile([128, DC, TLOC], F32, tag="s1",
                                            name="s1")
                            for mc in range(DC):
                                pa = ps_a.tile([128, TLOC], F32, tag="pa",
                                               name="pa")
                                for kc in range(ADC):
                                    rhs = (state[:, kc, :] if kc < DC
                                           else eoutT[:, kc - DC, :])
                                    nc.tensor.matmul(
                                        pa[:],
                                        srad[:, kc, mc * 128:(mc + 1) * 128],
                                        rhs, start=(kc == 0),
                                        stop=(kc == ADC - 1))
                                nc.scalar.copy(s1[:, mc, :], pa[:])
                            sn = apool.tile([128, DC, TLOC], rdt, tag="sn",
                                            name="sn")
                            rmsT(sn[:], s1[:], n1[:], ps_rr)
                            av = apool.tile([128, DC, TLOC], rdt, tag="av",
                                            name="av")
                            for mc in range(DC):
                                pa = ps_a.tile([128, TLOC], F32, tag="pa",
                                               name="pa")
                                for kc in range(DC):
                                    nc.tensor.matmul(
                                        pa[:],
                                        srwv[:, kc, mc * 128:(mc + 1) * 128],
                                        sn[:, kc, :], start=(kc == 0),
                                        stop=(kc == DC - 1))
                                nc.scalar.copy(av[:, mc, :], pa[:])
                            r2 = apool.tile([128, DC, TLOC], F32, tag="r2",
                                            name="r2")
                            for mc in range(DC):
                                pa = ps_a.tile([128, TLOC], F32, tag="pa",
                                               name="pa")
                                for kc in range(DC):
                                    nc.tensor.matmul(
                                        pa[:],
                                        srwo[:, kc, mc * 128:(mc + 1) * 128],
                                        av[:, kc, :], start=(kc == 0),
                                        stop=(kc == DC - 1))
                                nc.vector.tensor_tensor(
                                    r2[:, mc, :], s1[:, mc, :], pa[:],
                                    AluOpType.add)
                            r2n = apool.tile([128, DC, TLOC], F32, tag="r2n",
                                             name="r2n")
                            rmsT(r2n[:], r2[:], n2[:], ps_rr)
                            s3n = apool.tile([128, DC, TLOC], rdt,
                                             tag="s3n", name="s3n")
                            rmsT(s3n[:], r2n[:], n3[:], ps_rr)
                            pm = [ps_pm.tile([128, TLOC], F32, tag=f"pm{mc}",
                                             name=f"pm{mc}", bufs=1)
                                  for mc in range(DC)]
                            for fc in range(FM):
                                pa = ps_a.tile([128, TLOC], F32, tag="pa",
                                               name="pa")
                                for kc in range(DC):
                                    nc.tensor.matmul(
                                        pa[:],
                                        srm1[:, kc, fc * 128:(fc + 1) * 128],
                                        s3n[:, kc, :], start=(kc == 0),
                                        stop=(kc == DC - 1))
                                ms = apool.tile([128, TLOC], rdt, tag="ms",
                                                name="ms", bufs=3)
                                nc.scalar.activation(ms[:], pa[:], AF.Silu)
                                for mc in range(DC):
                                    nc.tensor.matmul(
                                        pm[mc][:],
                                        srm2[:, fc, mc * 128:(mc + 1) * 128],
                                        ms[:], start=(fc == 0),
                                        stop=(fc == FM - 1))
                            r4 = apool.tile([128, DC, TLOC], F32, tag="r4",
                                            name="r4")
                            for mc in range(DC):
                                nc.vector.tensor_tensor(
                                    r4[:, mc, :], r2n[:, mc, :], pm[mc][:],
                                    AluOpType.add)
                            stn = apool.tile([128, DC, TLOC], rdt,
                                             tag="state_n", name="state_n")
                            rmsT(stn[:], r4[:], n4[:], ps_rr)
                            state = stn

                for mc in range(DC):
                    nc.vector.tensor_tensor(xT[:, mc, :], xT[:, mc, :],
                                            state[:, mc, :], AluOpType.add)

            # ---------- final norm + lm_head ----------
            nc.sync.dma_start(r128(dbg_d.ap()), xT[:])
            if DO_HEAD:
                onw = load_wcol(onw_d.ap(), "onw")
                xo = apool.tile([128, DC, TLOC], F32, tag="xo", name="xo")
                with tc.tile_pool(name="ps_f", bufs=2, space="PSUM") as ps_f:
                    rmsT(xo[:], xT[:], onw[:], ps_f)
                xoh = apool.tile([128, DC, TLOC], mybir.dt.float16,
                                 tag="xoh", name="xoh")
                nc.vector.tensor_copy(xoh[:].opt(), xo[:].opt())
                with (tc.tile_pool(name="wlm", bufs=3) as wlm,
                      tc.tile_pool(name="ps_lm", bufs=4,
                                   space="PSUM") as ps_lm):
                    NV = 512
                    for vs in range(0, V, NV):
                        nv = min(NV, V - vs)
                        lw = wlm.tile([128, DC, NV], mybir.dt.float16,
                                      tag="lmw", name="lmw")
                        nc.sync.dma_start(
                            lw[:, :, 0:nv],
                            lmh_d.ap()[:, vs:vs + nv].rearrange(
                                "(a p) f -> p a f", p=128))
                        for ti in range(2):
                            pl2 = ps_lm.tile([128, NV], F32, tag="plm",
                                             name="plm")
                            for kc in range(DC):
                                nc.tensor.matmul(
                                    pl2[:, 0:nv],
                                    xoh[:, kc, ti * 128:(ti + 1) * 128],
                                    lw[:, kc, 0:nv], start=(kc == 0),
                                    stop=(kc == DC - 1))
                            ot = apool.tile([128, NV], F32, tag="ot",
                                            name="ot", bufs=3)
                            nc.scalar.copy(ot[:, 0:nv], pl2[:, 0:nv])
                            nc.sync.dma_start(
                                logits_d.ap()[ti * 128:(ti + 1) * 128,
                                              vs:vs + nv],
                                ot[:, 0:nv])
        finally:
            for p in reversed(octx):
                p.__exit__(None, None, None)

    nc.compile()
    return nc


def _state_inits():
    import jax
    import jax.numpy as jnp
    key = jax.random.key(42)
    out = []
    for i in range(L):
        s = jax.random.normal(jax.random.fold_in(key, i), (T, D), jnp.float32)
        out.append(np.asarray(s) * np.float32(0.02))
    return out


def kernel(input_ids, params):
    global LAST_RESULT
    input_ids = np.asarray(input_ids)
    p = params

    if "nc" not in _CACHE:
        _CACHE["nc"] = build()
    nc = _CACHE["nc"]

    tok = np.asarray(p["tok_emb"], dtype=np.float32)
    pos = np.asarray(p["pos_emb"], dtype=np.float32)[:S]
    x0 = tok[np.asarray(input_ids).reshape(-1)].reshape(B, S, D) + pos[None]
    x0 = x0.reshape(T, D)

    s0 = _state_inits()

    tri = np.zeros((128, 128), dtype=np.float32)
    for k_ in range(128):
        tri[k_, :k_] = -1.0e30

    def f32(a):
        return np.ascontiguousarray(np.asarray(a, dtype=np.float32))

    layers = p["layers"]
    import ml_dtypes
    _e1 = f32(np.stack([lp["exp_w1"] for lp in layers]))[:L - 1]
    _e2 = f32(np.stack([lp["exp_w2"] for lp in layers]))[:L - 1]
    _hi1 = np.ascontiguousarray(_e1.astype(ml_dtypes.bfloat16))
    _lo1 = np.ascontiguousarray(
        (_e1 - _hi1.astype(np.float32)).astype(ml_dtypes.bfloat16))
    _hi2 = np.ascontiguousarray(_e2.astype(ml_dtypes.bfloat16))
    _lo2 = np.ascontiguousarray(
        (_e2 - _hi2.astype(np.float32)).astype(ml_dtypes.bfloat16))
    shared = {
        "wo": f32(np.stack([lp["attn_wo"] for lp in layers])),
        "anw": f32(np.stack([[lp["norm1"], lp["norm2"]] for lp in layers])),
        "rtr": f32(np.stack([lp["router"] for lp in layers])),
        "srad": f32(np.stack([lp["sr"]["adapter"] for lp in layers])),
        "srwv": f32(np.stack([lp["sr"]["attn_wv"] for lp in layers])),
        "srwo": f32(np.stack([lp["sr"]["attn_wo"] for lp in layers])),
        "srm1": f32(np.stack([lp["sr"]["mlp_w1"] for lp in layers])),
        "srm2": f32(np.stack([lp["sr"]["mlp_w2"] for lp in layers])),
        "srnw": f32(np.stack([[lp["sr"]["n1"], lp["sr"]["n2"],
                               lp["sr"]["n3"], lp["sr"]["n4"]]
                              for lp in layers])),
        "onw": f32(p["norm_out"]),
        "lmh": np.ascontiguousarray(
            np.asarray(p["lm_head"]).astype(np.float16)),
        "tri": tri,
        "ew1hi": _hi1, "ew1lo": _lo1, "ew2hi": _hi2, "ew2lo": _lo2,
        "ew1h": np.ascontiguousarray(
            np.asarray(layers[L - 1]["exp_w1"]).astype(np.float16)),
        "ew2h": np.ascontiguousarray(
            np.asarray(layers[L - 1]["exp_w2"]).astype(np.float16)),
        "sradh": np.ascontiguousarray(
            np.asarray(layers[L - 1]["sr"]["adapter"]).astype(np.float16)),
        "srwvh": np.ascontiguousarray(
            np.asarray(layers[L - 1]["sr"]["attn_wv"]).astype(np.float16)),
        "srwoh": np.ascontiguousarray(
            np.asarray(layers[L - 1]["sr"]["attn_wo"]).astype(np.float16)),
        "srm1h": np.ascontiguousarray(
            np.asarray(layers[L - 1]["sr"]["mlp_w1"]).astype(np.float16)),
        "srm2h": np.ascontiguousarray(
            np.asarray(layers[L - 1]["sr"]["mlp_w2"]).astype(np.float16)),
    }
    wq = f32(np.stack([lp["attn_wq"] for lp in layers]))
    wk = f32(np.stack([lp["attn_wk"] for lp in layers]))
    wv = f32(np.stack([lp["attn_wv"] for lp in layers]))

    in_maps = []
    for c in range(NCORES):
        rows = np.concatenate([np.arange(b_ * 128, b_ * 128 + 128)
                               for b_ in CORE_BLOCKS[c]])
        im = dict(shared)
        im["x0T"] = np.ascontiguousarray(x0[rows].T)
        im["s0T"] = np.ascontiguousarray(
            np.stack([s0[li][rows].T for li in range(L)]))
        hs = slice(c * HD, (c + 1) * HD)
        im["wqh"] = np.ascontiguousarray(wq[:, :, hs])
        im["wkh"] = np.ascontiguousarray(wk[:, :, hs])
        im["wvh"] = np.ascontiguousarray(wv[:, :, hs])
        in_maps.append(im)

    res = bass_utils.run_bass_kernel_spmd(
        nc, in_maps, core_ids=list(range(NCORES)))
    LAST_RESULT = res

    logits = np.zeros((T, V), dtype=np.float32)
    fsum = np.zeros((L, E), dtype=np.float64)
    psum = np.zeros((L, E), dtype=np.float64)
    for c in range(NCORES):
        o = res.results[c]
        lg = o["logits"]
        for j, b_ in enumerate(CORE_BLOCKS[c]):
            logits[b_ * 128:(b_ + 1) * 128] = lg[j * 128:(j + 1) * 128]
        fsum += o["aux"][:, 0, :]
        psum += o["aux"][:, 1, :]

    total_aux = np.float32(0.0)
    for li in range(L):
        f = (fsum[li] / T).astype(np.float32)
        pr = (psum[li] / T).astype(np.float32)
        total_aux = np.float32(total_aux + np.float32(E) *
                               np.float32(np.sum(f * pr, dtype=np.float32)))
    return logits.reshape(B, S, V), total_aux


# revision 18
# speedup vs baseline: 1.3210x; 1.0192x over previous
"""Trainium2 Bass kernel for nn_MoREModelSynthesisIOptionB (moe_routing).

Sharding: 8 NeuronCores. Token-data-parallel for MoE/recurrent/lm_head
(core c owns token blocks {c, 15-c} of 128 tokens), head-parallel for
attention (core c owns head c; head weight slices are passed as per-core
input data so the compiled program is identical on every core).
Activations are kept transposed ([d, t]) so weight matrices serve as the
stationary matmul operand exactly as stored. Dense expert dispatch with
the one-hot top-1 combine mask applied to the gelu output before the w2
matmul (PSUM accumulates over experts). fp32 matmuls throughout.
Collectives per layer: AllGather of normed x (attention input), AllToAll
of per-head attention outputs back to token shards.
"""
import os
import sys
import numpy as np

sys.path.insert(0, "/opt/trn_rl_repo")
sys.path.insert(0, "/opt/trn_rl_repo/concourse")

from concourse import bass, bacc, tile, mybir, masks  # noqa: E402
from concourse import bass_utils  # noqa: E402
from concourse.alu_op_type import AluOpType  # noqa: E402

AF = mybir.ActivationFunctionType
F32 = mybir.dt.float32

NCORES = 8
B, S, V, D, H, L, E, NR = 2, 1024, 32000, 512, 8, 4, 8, 2
DFE = 2 * D
DFM = 4 * D
HD = D // H
T = B * S
NBLK = T // 128
TLOC = 256
DC = D // 128        # 4
FE = DFE // 128      # 8
FM = DFM // 128      # 16
ADC = 2 * D // 128   # 8
EPS = 1e-6

CORE_BLOCKS = [[c, NBLK - 1 - c] for c in range(NCORES)]
BLK_SRC = [(m, 0) if m < NCORES else (NBLK - 1 - m, 1) for m in range(NBLK)]

N_LAYERS = int(os.environ.get("KLAYERS", str(L)))
DO_HEAD = os.environ.get("KHEAD", "1") == "1"

_CACHE = {}
LAST_RESULT = None


def build():
    nc = bacc.Bacc("TRN2", target_bir_lowering=False, debug=False,
                   enable_asserts=False, num_devices=NCORES)

    def din(name, shape):
        return nc.dram_tensor(name, list(shape), F32, kind="ExternalInput")

    x0T_d = din("x0T", [D, TLOC])
    s0T_d = din("s0T", [L, D, TLOC])
    wqh_d = din("wqh", [L, D, HD])
    wkh_d = din("wkh", [L, D, HD])
    wvh_d = din("wvh", [L, D, HD])
    wo_d = din("wo", [L, D, D])
    anw_d = din("anw", [L, 2, D])
    rtr_d = din("rtr", [L, D, E])
    BF16 = mybir.dt.bfloat16
    ew1hi_d = nc.dram_tensor("ew1hi", [L - 1, E, D, DFE], BF16,
                             kind="ExternalInput")
    ew1lo_d = nc.dram_tensor("ew1lo", [L - 1, E, D, DFE], BF16,
                             kind="ExternalInput")
    ew2hi_d = nc.dram_tensor("ew2hi", [L - 1, E, DFE, D], BF16,
                             kind="ExternalInput")
    ew2lo_d = nc.dram_tensor("ew2lo", [L - 1, E, DFE, D], BF16,
                             kind="ExternalInput")
    srad_d = din("srad", [L, 2 * D, D])
    srwv_d = din("srwv", [L, D, D])
    srwo_d = din("srwo", [L, D, D])
    srm1_d = din("srm1", [L, D, DFM])
    srm2_d = din("srm2", [L, DFM, D])
    srnw_d = din("srnw", [L, 4, D])
    onw_d = din("onw", [D])
    lmh_d = nc.dram_tensor("lmh", [D, V], mybir.dt.float16,
                           kind="ExternalInput")
    tri_d = din("tri", [128, 128])
    F16 = mybir.dt.float16
    ew1h_d = nc.dram_tensor("ew1h", [E, D, DFE], F16, kind="ExternalInput")
    ew2h_d = nc.dram_tensor("ew2h", [E, DFE, D], F16, kind="ExternalInput")
    sradh_d = nc.dram_tensor("sradh", [2 * D, D], F16, kind="ExternalInput")
    srwvh_d = nc.dram_tensor("srwvh", [D, D], F16, kind="ExternalInput")
    srwoh_d = nc.dram_tensor("srwoh", [D, D], F16, kind="ExternalInput")
    srm1h_d = nc.dram_tensor("srm1h", [D, DFM], F16, kind="ExternalInput")
    sm1hi_d = nc.dram_tensor("sm1hi", [L - 1, D, DFM], BF16,
                             kind="ExternalInput")
    sm1lo_d = nc.dram_tensor("sm1lo", [L - 1, D, DFM], BF16,
                             kind="ExternalInput")
    sm2hi_d = nc.dram_tensor("sm2hi", [L - 1, DFM, D], BF16,
                             kind="ExternalInput")
    sm2lo_d = nc.dram_tensor("sm2lo", [L - 1, DFM, D], BF16,
                             kind="ExternalInput")
    srm2h_d = nc.dram_tensor("srm2h", [DFM, D], F16, kind="ExternalInput")

    logits_d = nc.dram_tensor("logits", [TLOC, V], F32, kind="ExternalOutput")
    aux_d = nc.dram_tensor("aux", [L, 2, E], F32, kind="ExternalOutput")
    dbg_d = nc.dram_tensor("dbg", [D, TLOC], F32, kind="ExternalOutput")

    rg = [list(range(NCORES))]

    def r128(ap):
        return ap.rearrange("(a p) f -> p a f", p=128)

    with tile.TileContext(nc) as tc:
        octx = [
            tc.tile_pool(name="cpool", bufs=1),
            tc.tile_pool(name="wpool", bufs=2),
            tc.tile_pool(name="apool", bufs=1),
            tc.tile_pool(name="dram", bufs=2, space="DRAM"),
        ]
        cpool, wpool, apool, dram = [p.__enter__() for p in octx]
        try:
            ident = cpool.tile([128, 128], F32)
            masks.make_identity(nc, ident[:])
            ones_col = cpool.tile([128, 1], F32)
            nc.vector.memset(ones_col[:], 1.0)
            tri = cpool.tile([128, 128], F32)
            nc.sync.dma_start(tri[:], tri_d.ap())

            xT = cpool.tile([128, DC, TLOC], F32, name="xT")
            nc.sync.dma_start(xT[:], r128(x0T_d.ap()))

            def rmsT(dst, src, w_col, ps_r):
                sq = apool.tile([128, DC, TLOC], F32, tag="rms_sq",
                                name="rms_sq")
                for kc in range(DC):
                    nc.vector.tensor_tensor(sq[:, kc, :], src[:, kc, :],
                                            src[:, kc, :], AluOpType.mult)
                ss = ps_r.tile([1, TLOC], F32, tag="rms_ss", name="rms_ss",
                               bufs=2)
                for kc in range(DC):
                    nc.tensor.matmul(ss[:], ones_col[:], sq[:, kc, :],
                                     start=(kc == 0), stop=(kc == DC - 1))
                st = apool.tile([1, TLOC], F32, tag="rms_st", name="rms_st",
                                bufs=2)
                nc.vector.tensor_scalar(st[:], ss[:], 1.0 / D, EPS,
                                        AluOpType.mult, AluOpType.add)
                st2 = apool.tile([1, TLOC], F32, tag="rms_st2", name="rms_st2",
                                 bufs=2)
                nc.scalar.sqrt(st2[:], st[:])
                st3 = apool.tile([1, TLOC], F32, tag="rms_st3", name="rms_st3",
                                 bufs=2)
                nc.vector.reciprocal(st3[:], st2[:])
                bc = apool.tile([128, TLOC], F32, tag="rms_bc", name="rms_bc",
                                bufs=2)
                nc.gpsimd.partition_broadcast(bc[:], st3[:])
                for kc in range(DC):
                    nc.vector.scalar_tensor_tensor(
                        dst[:, kc, :], src[:, kc, :], w_col[:, kc:kc + 1],
                        bc[:], AluOpType.mult, AluOpType.mult)

            def load_wcol(dram_ap, tag):
                t = wpool.tile([128, DC], F32, tag=tag, name=tag)
                nc.sync.dma_start(t[:], dram_ap.rearrange("(a p) -> p a",
                                                          p=128))
                return t

            # ================= layers =================
            for l in range(N_LAYERS):
                anw1 = load_wcol(anw_d.ap()[l, 0], "anw1")
                anw2 = load_wcol(anw_d.ap()[l, 1], "anw2")

                # ---------- attention ----------
                with (tc.tile_pool(name="aap", bufs=1) as aap,
                      tc.tile_pool(name="ps_r", bufs=2, space="PSUM") as ps_r):
                    wqh = aap.tile([128, DC, HD], F32, tag="wqh", name="wqh")
                    nc.sync.dma_start(wqh[:], r128(wqh_d.ap()[l]))
                    wkh = aap.tile([128, DC, HD], F32, tag="wkh", name="wkh")
                    nc.sync.dma_start(wkh[:], r128(wkh_d.ap()[l]))
                    wvh = aap.tile([128, DC, HD], F32, tag="wvh", name="wvh")
                    nc.sync.dma_start(wvh[:], r128(wvh_d.ap()[l]))
                    wo = aap.tile([128, DC, D], F32, tag="wo", name="wo")
                    nc.sync.dma_start(wo[:], r128(wo_d.ap()[l]))

                    xn = aap.tile([128, DC, TLOC], F32, tag="xn", name="xn")
                    rmsT(xn[:], xT[:], anw1[:], ps_r)

                    ag_in = dram.tile([D * TLOC], F32, tag="ag_in",
                                      name="ag_in")
                    nc.sync.dma_start(
                        ag_in[:].rearrange("(a p f) -> p a f", p=128, a=DC),
                        xn[:])
                    ag_out = dram.tile([NCORES, D * TLOC], F32, tag="ag_out",
                                       name="ag_out", addr_space="Shared")
                    nc.gpsimd.collective_compute(
                        "AllGather", AluOpType.bypass, replica_groups=rg,
                        ins=[ag_in.opt()], outs=[ag_out.opt()])

                    qT = aap.tile([64, NCORES, TLOC], F32, tag="qT", name="qT")
                    kT = aap.tile([64, NCORES, TLOC], F32, tag="kT", name="kT")
                    v2 = aap.tile([128, NCORES, 2, HD + 1], F32, tag="v2",
                                  name="v2")
                    nc.vector.memset(v2[:, :, :, HD:HD + 1], 1.0)
                    with tc.tile_pool(name="ps_qk", bufs=3,
                                      space="PSUM") as ps_qk:
                        for s_ in range(NCORES):
                            xa = aap.tile([128, DC, TLOC], F32, tag="xa",
                                          name="xa", bufs=2)
                            nc.sync.dma_start(
                                xa[:],
                                ag_out[:].rearrange(
                                    "s (a p f) -> s p a f", p=128, a=DC)[s_])
                            pq = ps_qk.tile([64, TLOC], F32, tag="pqk",
                                            name="pq")
                            pk = ps_qk.tile([64, TLOC], F32, tag="pqk",
                                            name="pk")
                            for kc in range(DC):
                                nc.tensor.matmul(pq[:], wqh[:, kc, :],
                                                 xa[:, kc, :], start=(kc == 0),
                                                 stop=(kc == DC - 1))
                            for kc in range(DC):
                                nc.tensor.matmul(pk[:], wkh[:, kc, :],
                                                 xa[:, kc, :], start=(kc == 0),
                                                 stop=(kc == DC - 1))
                            nc.scalar.activation(
                                qT[:, s_, :], pq[:], AF.Copy,
                                scale=1.0 / float(np.sqrt(HD)))
                            nc.scalar.copy(kT[:, s_, :], pk[:])
                            for ti in range(2):
                                pv = ps_qk.tile([128, TLOC], F32, tag="pqk",
                                                name="pv")[:, 0:HD]
                                for kc in range(DC):
                                    nc.tensor.matmul(
                                        pv[:],
                                        xa[:, kc, ti * 128:(ti + 1) * 128],
                                        wvh[:, kc, :], start=(kc == 0),
                                        stop=(kc == DC - 1))
                                nc.scalar.copy(v2[:, s_, ti, 0:HD], pv[:])

                    ao = aap.tile([64, NCORES, TLOC], F32, tag="ao", name="ao")
                    den = aap.tile([1, NCORES, TLOC], F32, tag="den",
                                   name="den")
                    with (tc.tile_pool(name="ps_s", bufs=4,
                                       space="PSUM") as ps_s,
                          tc.tile_pool(name="ps_pv", bufs=2,
                                       space="PSUM") as ps_pv):
                        for qb in range(NBLK):
                            batch, qpos = qb // NCORES, qb % NCORES
                            qs, qh = BLK_SRC[qb]
                            q_ap = qT[:, qs, qh * 128:(qh + 1) * 128]
                            pv = ps_pv.tile([HD + 1, 128], F32, tag="pv_acc",
                                            name="pv_acc")
                            nkb = qpos + 1
                            for kb in range(nkb):
                                m = batch * NCORES + kb
                                ks, kh = BLK_SRC[m]
                                st = ps_s.tile([128, 128], F32, tag="st",
                                               name="st")
                                nc.tensor.matmul(
                                    st[:],
                                    kT[:, ks, kh * 128:(kh + 1) * 128],
                                    q_ap, start=True, stop=True)
                                if kb == nkb - 1:
                                    nc.vector.tensor_tensor(
                                        st[:], st[:], tri[:], AluOpType.add)
                                es = aap.tile([128, 128], F32, tag="es",
                                              name="es", bufs=3)
                                nc.scalar.activation(es[:], st[:], AF.Exp)
                                nc.tensor.matmul(
                                    pv[:], v2[:, ks, kh, :], es[:],
                                    start=(kb == 0), stop=(kb == nkb - 1))
                            nc.scalar.copy(
                                ao[:, qs, qh * 128:(qh + 1) * 128],
                                pv[0:HD, :])
                            nc.scalar.copy(
                                den[:, qs, qh * 128:(qh + 1) * 128],
                                pv[HD:HD + 1, :])

                    rden = aap.tile([1, NCORES, TLOC], F32, tag="rden",
                                    name="rden")
                    rbc = aap.tile([64, NCORES, TLOC], F32, tag="rbc",
                                   name="rbc")
                    aos = aap.tile([64, NCORES, TLOC], F32, tag="aos",
                                   name="aos")
                    atT = aap.tile([128, DC, TLOC], F32, tag="atT", name="atT")
                    # A2A split in two halves: half 0 (q-blocks 0-7, the
                    # first 8 of the qb loop) ships while the PE computes
                    # scores for q-blocks 8-15.
                    for hf in range(2):
                        hsl = slice(hf * 128, (hf + 1) * 128)
                        nc.vector.reciprocal(rden[:, :, hsl].opt(),
                                             den[:, :, hsl].opt())
                        nc.gpsimd.partition_broadcast(
                            rbc[:, :, hsl].opt(), rden[:, :, hsl].opt(),
                            channels=64)
                        nc.vector.tensor_tensor(aos[:, :, hsl].opt(),
                                                ao[:, :, hsl].opt(),
                                                rbc[:, :, hsl].opt(),
                                                AluOpType.mult)
                        a2a_in = dram.tile([NCORES, 64 * 128], F32,
                                           tag=f"a2a_in{hf}",
                                           name=f"a2a_in{hf}")
                        for s_ in range(NCORES):
                            nc.sync.dma_start(
                                a2a_in[:].rearrange("s (p f) -> s p f",
                                                    p=64)[s_],
                                aos[:, s_, hsl])
                        a2a_out = dram.tile([NCORES, 64 * 128], F32,
                                            tag=f"a2a_out{hf}",
                                            name=f"a2a_out{hf}")
                        nc.gpsimd.collective_compute(
                            "AllToAll", AluOpType.bypass, replica_groups=rg,
                            ins=[a2a_in.opt()], outs=[a2a_out.opt()])
                        nc.sync.dma_start(
                            atT[:, :, hsl],
                            a2a_out[:].rearrange("s (p f) -> (s p) f", p=64)
                            .rearrange("(a p) f -> p a f", p=128))

                    with tc.tile_pool(name="ps_o", bufs=2,
                                      space="PSUM") as ps_o:
                        for mc in range(DC):
                            po = ps_o.tile([128, TLOC], F32, tag="po",
                                           name="po")
                            for kc in range(DC):
                                nc.tensor.matmul(
                                    po[:], wo[:, kc, mc * 128:(mc + 1) * 128],
                                    atT[:, kc, :], start=(kc == 0),
                                    stop=(kc == DC - 1))
                            nc.vector.tensor_tensor(xT[:, mc, :], xT[:, mc, :],
                                                    po[:], AluOpType.add)

                # ---------- router ----------
                xf = apool.tile([128, DC, TLOC], F32, tag="xf", name="xf")
                cwT = apool.tile([1, E, TLOC], F32, tag="cwT", name="cwT")
                with (tc.tile_pool(name="ps_l", bufs=2, space="PSUM") as ps_l,
                      tc.tile_pool(name="ps_x", bufs=3, space="PSUM") as ps_x):
                    rmsT(xf[:], xT[:], anw2[:], ps_l)
                    rtr = apool.tile([128, DC, E], F32, tag="rtr", name="rtr",
                                     bufs=2)
                    nc.sync.dma_start(rtr[:], r128(rtr_d.ap()[l]))
                    ohp = apool.tile([128, 2, E], F32, tag="ohp", name="ohp")
                    prb = apool.tile([128, 2, E], F32, tag="prb", name="prb")
                    for ti in range(2):
                        pl = ps_l.tile([128, E], F32, tag="pl", name="pl",
                                        bufs=1)
                        for kc in range(DC):
                            nc.tensor.matmul(
                                pl[:], xf[:, kc, ti * 128:(ti + 1) * 128],
                                rtr[:, kc, :], start=(kc == 0),
                                stop=(kc == DC - 1))
                        lg = apool.tile([128, E], F32, tag="lg", name="lg",
                                        bufs=2)
                        nc.vector.tensor_copy(lg[:], pl[:])
                        mx = apool.tile([128, 1], F32, tag="mx", name="mx",
                                        bufs=2)
                        nc.vector.tensor_reduce(mx[:], lg[:],
                                                mybir.AxisListType.X,
                                                AluOpType.max)
                        nc.vector.tensor_scalar(ohp[:, ti, :], lg[:], mx[:],
                                                None, AluOpType.is_equal)
                        nmx = apool.tile([128, 1], F32, tag="nmx", name="nmx",
                                         bufs=2)
                        nc.vector.tensor_scalar(nmx[:], mx[:], -1.0, None,
                                                AluOpType.mult)
                        rs = apool.tile([128, 1], F32, tag="rs", name="rs",
                                        bufs=2)
                        ex = apool.tile([128, E], F32, tag="ex", name="ex",
                                        bufs=2)
                        nc.scalar.activation(ex[:], lg[:], AF.Exp,
                                             bias=nmx[:], accum_out=rs[:])
                        rrs = apool.tile([128, 1], F32, tag="rrs", name="rrs",
                                         bufs=2)
                        nc.vector.reciprocal(rrs[:], rs[:])
                        nc.vector.tensor_scalar(prb[:, ti, :], ex[:], rrs[:],
                                                None, AluOpType.mult)
                    pf = ps_x.tile([1, E], F32, tag="pf", name="pf", bufs=1)
                    pp = ps_x.tile([1, E], F32, tag="pp", name="pp", bufs=1)
                    for ti in range(2):
                        nc.tensor.matmul(pf[:], ones_col[:], ohp[:, ti, :],
                                         start=(ti == 0), stop=(ti == 1))
                    for ti in range(2):
                        nc.tensor.matmul(pp[:], ones_col[:], prb[:, ti, :],
                                         start=(ti == 0), stop=(ti == 1))
                    auxs = apool.tile([1, 2, E], F32, tag="auxs", name="auxs")
                    nc.vector.tensor_copy(auxs[:, 0, :], pf[:])
                    nc.vector.tensor_copy(auxs[:, 1, :], pp[:])
                    nc.sync.dma_start(aux_d.ap()[l], auxs[:].opt())

                    for ti in range(2):
                        pt = ps_x.tile([E, 128], F32, tag="pt", name="pt",
                                         bufs=1)
                        nc.tensor.transpose(pt[:], ohp[:, ti, :], ident[:])
                        ptc = apool.tile([E, 128], F32, tag="ptc", name="ptc",
                                         bufs=2)
                        nc.vector.tensor_copy(ptc[:], pt[:])
                        nc.sync.dma_start(
                            cwT[0:1, :, ti * 128:(ti + 1) * 128],
                            ptc[:])

                # ---------- experts (dense, masked before w2) ----------
                # layers 0..L-2: bf16 hi/lo split matmuls (3 passes, ~16-bit
                # effective mantissa, fp32 accumulate) - flip-safe per sim.
                # last layer: plain fp16 (no routing downstream).
                eoutT = apool.tile([128, DC, TLOC], F32, tag="eoutT",
                                   name="eoutT")
                fp16x = (l == L - 1)
                edt = mybir.dt.float16 if fp16x else F32
                with (tc.tile_pool(name="wep", bufs=2) as wep,
                      tc.tile_pool(name="ps_h", bufs=3, space="PSUM") as ps_h,
                      tc.tile_pool(name="ps_eo", bufs=4,
                                   space="PSUM") as ps_eo):
                    if fp16x:
                        xfh = apool.tile([128, DC, TLOC], edt, tag="xfh",
                                         name="xfh")
                        nc.vector.tensor_copy(xfh[:].opt(), xf[:].opt())
                        xfl = None
                    else:
                        xfh = apool.tile([128, DC, TLOC], BF16, tag="xfh16",
                                         name="xfh16")
                        nc.vector.tensor_copy(xfh[:].opt(), xf[:].opt())
                        xfhf = apool.tile([128, DC, TLOC], F32, tag="xfhf",
                                          name="xfhf")
                        nc.vector.tensor_copy(xfhf[:].opt(), xfh[:].opt())
                        xfl = apool.tile([128, DC, TLOC], BF16, tag="xfl16",
                                         name="xfl16")
                        nc.vector.tensor_tensor(xfl[:].opt(), xf[:].opt(),
                                                xfhf[:].opt(),
                                                AluOpType.subtract)
                    eo = [ps_eo.tile([128, TLOC], F32, tag=f"eo{mc}",
                                     name=f"eo{mc}", bufs=1)
                          for mc in range(DC)]
                    for e_ in range(E):
                        if fp16x:
                            w1 = wep.tile([128, DC, DFE], edt, tag="ew1",
                                          name="ew1")
                            nc.sync.dma_start(w1[:], r128(ew1h_d.ap()[e_]))
                            w2 = wep.tile([128, FE, D], edt, tag="ew2",
                                          name="ew2")
                            nc.sync.dma_start(w2[:], r128(ew2h_d.ap()[e_]))
                        else:
                            w1 = wep.tile([128, DC, DFE], BF16, tag="ew1",
                                          name="ew1")
                            nc.sync.dma_start(w1[:],
                                              r128(ew1hi_d.ap()[l, e_]))
                            w1l = wep.tile([128, DC, DFE], BF16, tag="ew1l",
                                           name="ew1l")
                            nc.sync.dma_start(w1l[:],
                                              r128(ew1lo_d.ap()[l, e_]))
                            w2 = wep.tile([128, FE, D], BF16, tag="ew2",
                                          name="ew2")
                            nc.sync.dma_start(w2[:],
                                              r128(ew2hi_d.ap()[l, e_]))
                            w2l = wep.tile([128, FE, D], BF16, tag="ew2l",
                                           name="ew2l")
                            nc.sync.dma_start(w2l[:],
                                              r128(ew2lo_d.ap()[l, e_]))
                        bce = apool.tile([128, TLOC], F32, tag="bce",
                                         name="bce", bufs=2)
                        nc.gpsimd.partition_broadcast(bce[:],
                                                      cwT[0:1, e_, :])
                        for fc in range(FE):
                            ph = ps_h.tile([128, TLOC], F32, tag="ph",
                                           name="ph")
                            fsl = slice(fc * 128, (fc + 1) * 128)
                            if fp16x:
                                for kc in range(DC):
                                    nc.tensor.matmul(
                                        ph[:], w1[:, kc, fsl],
                                        xfh[:, kc, :], start=(kc == 0),
                                        stop=(kc == DC - 1))
                            else:
                                i_mm = 0
                                for kc in range(DC):
                                    for wt, xt in ((w1, xfh), (w1, xfl),
                                                   (w1l, xfh)):
                                        nc.tensor.matmul(
                                            ph[:], wt[:, kc, fsl],
                                            xt[:, kc, :], start=(i_mm == 0),
                                            stop=(i_mm == 3 * DC - 1))
                                        i_mm += 1
                            hr = apool.tile([128, TLOC], F32, tag="hr",
                                            name="hr", bufs=2)
                            nc.scalar.activation(hr[:], ph[:], AF.Gelu)
                            hs = apool.tile([128, TLOC], edt, tag="hs",
                                            name="hs", bufs=3)
                            nc.vector.tensor_tensor(hs[:], hr[:], bce[:],
                                                    AluOpType.mult)
                            if fp16x:
                                for mc in range(DC):
                                    nc.tensor.matmul(
                                        eo[mc][:],
                                        w2[:, fc, mc * 128:(mc + 1) * 128],
                                        hs[:],
                                        start=(e_ == 0 and fc == 0),
                                        stop=(e_ == E - 1 and fc == FE - 1))
                            else:
                                hsh = apool.tile([128, TLOC], BF16,
                                                 tag="hsh", name="hsh",
                                                 bufs=3)
                                nc.vector.tensor_copy(hsh[:], hs[:])
                                hshf = apool.tile([128, TLOC], F32,
                                                  tag="hshf", name="hshf",
                                                  bufs=1)
                                nc.vector.tensor_copy(hshf[:], hsh[:])
                                hsl = apool.tile([128, TLOC], BF16,
                                                 tag="hsl", name="hsl",
                                                 bufs=3)
                                nc.vector.tensor_tensor(hsl[:], hs[:],
                                                        hshf[:],
                                                        AluOpType.subtract)
                                for mc in range(DC):
                                    msl = slice(mc * 128, (mc + 1) * 128)
                                    for j, (wt, ht) in enumerate(
                                            ((w2, hsh), (w2, hsl),
                                             (w2l, hsh))):
                                        nc.tensor.matmul(
                                            eo[mc][:], wt[:, fc, msl], ht[:],
                                            start=(e_ == 0 and fc == 0
                                                   and j == 0),
                                            stop=(e_ == E - 1
                                                  and fc == FE - 1
                                                  and j == 2))
                    for mc in range(DC):
                        nc.vector.tensor_copy(eoutT[:, mc, :], eo[mc][:])

                # ---------- shared recurrent ----------
                rdt = mybir.dt.float16 if fp16x else F32
                with tc.tile_pool(name="wrp", bufs=1) as wrp:
                    srad = wrp.tile([128, ADC, D], rdt, tag="srad",
                                    name="srad")
                    nc.sync.dma_start(srad[:], r128(sradh_d.ap()) if fp16x
                                      else r128(srad_d.ap()[l]))
                    srwv = wrp.tile([128, DC, D], rdt, tag="srwv", name="srwv")
                    nc.sync.dma_start(srwv[:], r128(srwvh_d.ap()) if fp16x
                                      else r128(srwv_d.ap()[l]))
                    srwo = wrp.tile([128, DC, D], rdt, tag="srwo", name="srwo")
                    nc.sync.dma_start(srwo[:], r128(srwoh_d.ap()) if fp16x
                                      else r128(srwo_d.ap()[l]))
                    mdt = rdt if fp16x else BF16
                    srm1 = wrp.tile([128, DC, DFM], mdt, tag="srm1",
                                    name="srm1")
                    nc.sync.dma_start(srm1[:], r128(srm1h_d.ap()) if fp16x
                                      else r128(sm1hi_d.ap()[l]))
                    srm2 = wrp.tile([128, FM, D], mdt, tag="srm2", name="srm2")
                    nc.sync.dma_start(srm2[:], r128(srm2h_d.ap()) if fp16x
                                      else r128(sm2hi_d.ap()[l]))
                    if not fp16x:
                        srm1l = wrp.tile([128, DC, DFM], BF16, tag="srm1l",
                                         name="srm1l")
                        nc.sync.dma_start(srm1l[:], r128(sm1lo_d.ap()[l]))
                        srm2l = wrp.tile([128, FM, D], BF16, tag="srm2l",
                                         name="srm2l")
                        nc.sync.dma_start(srm2l[:], r128(sm2lo_d.ap()[l]))
                    n1 = load_wcol(srnw_d.ap()[l, 0], "srn1")
                    n2 = load_wcol(srnw_d.ap()[l, 1], "srn2")
                    n3 = load_wcol(srnw_d.ap()[l, 2], "srn3")
                    n4 = load_wcol(srnw_d.ap()[l, 3], "srn4")

                    state = apool.tile([128, DC, TLOC], F32, tag="state",
                                       name="state")
                    nc.sync.dma_start(state[:], r128(s0T_d.ap()[l]))
                    if fp16x:
                        st16 = apool.tile([128, DC, TLOC], rdt, tag="st16",
                                          name="st16")
                        nc.vector.tensor_copy(st16[:].opt(), state[:].opt())
                        state = st16
                        eo16 = apool.tile([128, DC, TLOC], rdt, tag="eo16",
                                          name="eo16")
                        nc.vector.tensor_copy(eo16[:].opt(), eoutT[:].opt())
                        eoutT = eo16

                    for r_ in range(NR):
                        with (tc.tile_pool(name="ps_a", bufs=2,
                                           space="PSUM") as ps_a,
                              tc.tile_pool(name="ps_pm", bufs=4,
                                           space="PSUM") as ps_pm,
                              tc.tile_pool(name="ps_rr", bufs=2,
                                           space="PSUM") as ps_rr):
                            s1 = apool.tile([128, DC, TLOC], F32, tag="s1",
                                            name="s1")
                            for mc in range(DC):
                                pa = ps_a.tile([128, TLOC], F32, tag="pa",
                                               name="pa")
                                for kc in range(ADC):
                                    rhs = (state[:, kc, :] if kc < DC
                                           else eoutT[:, kc - DC, :])
                                    nc.tensor.matmul(
                                        pa[:],
                                        srad[:, kc, mc * 128:(mc + 1) * 128],
                                        rhs, start=(kc == 0),
                                        stop=(kc == ADC - 1))
                                nc.scalar.copy(s1[:, mc, :], pa[:])
                            sn = apool.tile([128, DC, TLOC], rdt, tag="sn",
                                            name="sn")
                            rmsT(sn[:], s1[:], n1[:], ps_rr)
                            av = apool.tile([128, DC, TLOC], rdt, tag="av",
                                            name="av")
                            for mc in range(DC):
                                pa = ps_a.tile([128, TLOC], F32, tag="pa",
                                               name="pa")
                                for kc in range(DC):
                                    nc.tensor.matmul(
                                        pa[:],
                                        srwv[:, kc, mc * 128:(mc + 1) * 128],
                                        sn[:, kc, :], start=(kc == 0),
                                        stop=(kc == DC - 1))
                                nc.scalar.copy(av[:, mc, :], pa[:])
                            r2 = apool.tile([128, DC, TLOC], F32, tag="r2",
                                            name="r2")
                            for mc in range(DC):
                                pa = ps_a.tile([128, TLOC], F32, tag="pa",
                                               name="pa")
                                for kc in range(DC):
                                    nc.tensor.matmul(
                                        pa[:],
                                        srwo[:, kc, mc * 128:(mc + 1) * 128],
                                        av[:, kc, :], start=(kc == 0),
                                        stop=(kc == DC - 1))
                                nc.vector.tensor_tensor(
                                    r2[:, mc, :], s1[:, mc, :], pa[:],
                                    AluOpType.add)
                            r2n = apool.tile([128, DC, TLOC], F32, tag="r2n",
                                             name="r2n")
                            rmsT(r2n[:], r2[:], n2[:], ps_rr)
                            s3n = apool.tile([128, DC, TLOC], rdt,
                                             tag="s3n", name="s3n")
                            rmsT(s3n[:], r2n[:], n3[:], ps_rr)
                            if not fp16x:
                                s3h = apool.tile([128, DC, TLOC], BF16,
                                                 tag="xfh16", name="s3h")
                                nc.vector.tensor_copy(s3h[:].opt(),
                                                      s3n[:].opt())
                                s3hf = apool.tile([128, DC, TLOC], F32,
                                                  tag="xfhf", name="s3hf")
                                nc.vector.tensor_copy(s3hf[:].opt(),
                                                      s3h[:].opt())
                                s3l = apool.tile([128, DC, TLOC], BF16,
                                                 tag="xfl16", name="s3l")
                                nc.vector.tensor_tensor(s3l[:].opt(),
                                                        s3n[:].opt(),
                                                        s3hf[:].opt(),
                                                        AluOpType.subtract)
                            pm = [ps_pm.tile([128, TLOC], F32, tag=f"pm{mc}",
                                             name=f"pm{mc}", bufs=1)
                                  for mc in range(DC)]
                            for fc in range(FM):
                                pa = ps_a.tile([128, TLOC], F32, tag="pa",
                                               name="pa")
                                fsl2 = slice(fc * 128, (fc + 1) * 128)
                                if fp16x:
                                    for kc in range(DC):
                                        nc.tensor.matmul(
                                            pa[:], srm1[:, kc, fsl2],
                                            s3n[:, kc, :], start=(kc == 0),
                                            stop=(kc == DC - 1))
                                else:
                                    i_mm = 0
                                    for kc in range(DC):
                                        for wt, xt in ((srm1, s3h),
                                                       (srm1, s3l),
                                                       (srm1l, s3h)):
                                            nc.tensor.matmul(
                                                pa[:], wt[:, kc, fsl2],
                                                xt[:, kc, :],
                                                start=(i_mm == 0),
                                                stop=(i_mm == 3 * DC - 1))
                                            i_mm += 1
                                ms = apool.tile([128, TLOC], rdt, tag="ms",
                                                name="ms", bufs=3)
                                nc.scalar.activation(ms[:], pa[:], AF.Silu)
                                if fp16x:
                                    for mc in range(DC):
                                        nc.tensor.matmul(
                                            pm[mc][:],
                                            srm2[:, fc,
                                                 mc * 128:(mc + 1) * 128],
                                            ms[:], start=(fc == 0),
                                            stop=(fc == FM - 1))
                                else:
                                    msh = apool.tile([128, TLOC], BF16,
                                                     tag="hsh", name="msh",
                                                     bufs=3)
                                    nc.vector.tensor_copy(msh[:], ms[:])
                                    mshf = apool.tile([128, TLOC], F32,
                                                      tag="hshf", name="mshf",
                                                      bufs=1)
                                    nc.vector.tensor_copy(mshf[:], msh[:])
                                    msl = apool.tile([128, TLOC], BF16,
                                                     tag="hsl", name="msl",
                                                     bufs=3)
                                    nc.vector.tensor_tensor(
                                        msl[:], ms[:], mshf[:],
                                        AluOpType.subtract)
                                    for mc in range(DC):
                                        msl2 = slice(mc * 128,
                                                     (mc + 1) * 128)
                                        for j, (wt, ht) in enumerate(
                                                ((srm2, msh), (srm2, msl),
                                                 (srm2l, msh))):
                                            nc.tensor.matmul(
                                                pm[mc][:], wt[:, fc, msl2],
                                                ht[:],
                                                start=(fc == 0 and j == 0),
                                                stop=(fc == FM - 1
                                                      and j == 2))
                            r4 = apool.tile([128, DC, TLOC], F32, tag="r4",
                                            name="r4")
                            for mc in range(DC):
                                nc.vector.tensor_tensor(
                                    r4[:, mc, :], r2n[:, mc, :], pm[mc][:],
                                    AluOpType.add)
                            stn = apool.tile([128, DC, TLOC], rdt,
                                             tag="state_n", name="state_n")
                            rmsT(stn[:], r4[:], n4[:], ps_rr)
                            state = stn

                for mc in range(DC):
                    nc.vector.tensor_tensor(xT[:, mc, :], xT[:, mc, :],
                                            state[:, mc, :], AluOpType.add)

            # ---------- final norm + lm_head ----------
            nc.sync.dma_start(r128(dbg_d.ap()), xT[:])
            if DO_HEAD:
                onw = load_wcol(onw_d.ap(), "onw")
                xo = apool.tile([128, DC, TLOC], F32, tag="xo", name="xo")
                with tc.tile_pool(name="ps_f", bufs=2, space="PSUM") as ps_f:
                    rmsT(xo[:], xT[:], onw[:], ps_f)
                xoh = apool.tile([128, DC, TLOC], mybir.dt.float16,
                                 tag="xoh", name="xoh")
                nc.vector.tensor_copy(xoh[:].opt(), xo[:].opt())
                with (tc.tile_pool(name="wlm", bufs=3) as wlm,
                      tc.tile_pool(name="ps_lm", bufs=4,
                                   space="PSUM") as ps_lm):
                    NV = 512
                    for vs in range(0, V, NV):
                        nv = min(NV, V - vs)
                        lw = wlm.tile([128, DC, NV], mybir.dt.float16,
                                      tag="lmw", name="lmw")
                        nc.sync.dma_start(
                            lw[:, :, 0:nv],
                            lmh_d.ap()[:, vs:vs + nv].rearrange(
                                "(a p) f -> p a f", p=128))
                        for ti in range(2):
                            pl2 = ps_lm.tile([128, NV], F32, tag="plm",
                                             name="plm")
                            for kc in range(DC):
                                nc.tensor.matmul(
                                    pl2[:, 0:nv],
                                    xoh[:, kc, ti * 128:(ti + 1) * 128],
                                    lw[:, kc, 0:nv], start=(kc == 0),
                                    stop=(kc == DC - 1))
                            ot = apool.tile([128, NV], F32, tag="ot",
                                            name="ot", bufs=3)
                            nc.scalar.copy(ot[:, 0:nv], pl2[:, 0:nv])
                            nc.sync.dma_start(
                                logits_d.ap()[ti * 128:(ti + 1) * 128,
                                              vs:vs + nv],
                                ot[:, 0:nv])
        finally:
            for p in reversed(octx):
                p.__exit__(None, None, None)

    nc.compile()
    return nc


def _state_inits():
    import jax
    import jax.numpy as jnp
    key = jax.random.key(42)
    out = []
    for i in range(L):
        s = jax.random.normal(jax.random.fold_in(key, i), (T, D), jnp.float32)
        out.append(np.asarray(s) * np.float32(0.02))
    return out


def kernel(input_ids, params):
    global LAST_RESULT
    input_ids = np.asarray(input_ids)
    p = params

    if "nc" not in _CACHE:
        _CACHE["nc"] = build()
    nc = _CACHE["nc"]

    tok = np.asarray(p["tok_emb"], dtype=np.float32)
    pos = np.asarray(p["pos_emb"], dtype=np.float32)[:S]
    x0 = tok[np.asarray(input_ids).reshape(-1)].reshape(B, S, D) + pos[None]
    x0 = x0.reshape(T, D)

    s0 = _state_inits()

    tri = np.zeros((128, 128), dtype=np.float32)
    for k_ in range(128):
        tri[k_, :k_] = -1.0e30

    def f32(a):
        return np.ascontiguousarray(np.asarray(a, dtype=np.float32))

    layers = p["layers"]
    import ml_dtypes
    _e1 = f32(np.stack([lp["exp_w1"] for lp in layers]))[:L - 1]
    _e2 = f32(np.stack([lp["exp_w2"] for lp in layers]))[:L - 1]
    _hi1 = np.ascontiguousarray(_e1.astype(ml_dtypes.bfloat16))
    _lo1 = np.ascontiguousarray(
        (_e1 - _hi1.astype(np.float32)).astype(ml_dtypes.bfloat16))
    _hi2 = np.ascontiguousarray(_e2.astype(ml_dtypes.bfloat16))
    _lo2 = np.ascontiguousarray(
        (_e2 - _hi2.astype(np.float32)).astype(ml_dtypes.bfloat16))
    _m1 = f32(np.stack([lp["sr"]["mlp_w1"] for lp in layers]))[:L - 1]
    _m2 = f32(np.stack([lp["sr"]["mlp_w2"] for lp in layers]))[:L - 1]
    _m1h = np.ascontiguousarray(_m1.astype(ml_dtypes.bfloat16))
    _m1l = np.ascontiguousarray(
        (_m1 - _m1h.astype(np.float32)).astype(ml_dtypes.bfloat16))
    _m2h = np.ascontiguousarray(_m2.astype(ml_dtypes.bfloat16))
    _m2l = np.ascontiguousarray(
        (_m2 - _m2h.astype(np.float32)).astype(ml_dtypes.bfloat16))
    shared = {
        "wo": f32(np.stack([lp["attn_wo"] for lp in layers])),
        "anw": f32(np.stack([[lp["norm1"], lp["norm2"]] for lp in layers])),
        "rtr": f32(np.stack([lp["router"] for lp in layers])),
        "srad": f32(np.stack([lp["sr"]["adapter"] for lp in layers])),
        "srwv": f32(np.stack([lp["sr"]["attn_wv"] for lp in layers])),
        "srwo": f32(np.stack([lp["sr"]["attn_wo"] for lp in layers])),
        "srm1": f32(np.stack([lp["sr"]["mlp_w1"] for lp in layers])),
        "srm2": f32(np.stack([lp["sr"]["mlp_w2"] for lp in layers])),
        "srnw": f32(np.stack([[lp["sr"]["n1"], lp["sr"]["n2"],
                               lp["sr"]["n3"], lp["sr"]["n4"]]
                              for lp in layers])),
        "onw": f32(p["norm_out"]),
        "lmh": np.ascontiguousarray(
            np.asarray(p["lm_head"]).astype(np.float16)),
        "tri": tri,
        "ew1hi": _hi1, "ew1lo": _lo1, "ew2hi": _hi2, "ew2lo": _lo2,
        "sm1hi": _m1h, "sm1lo": _m1l, "sm2hi": _m2h, "sm2lo": _m2l,
        "ew1h": np.ascontiguousarray(
            np.asarray(layers[L - 1]["exp_w1"]).astype(np.float16)),
        "ew2h": np.ascontiguousarray(
            np.asarray(layers[L - 1]["exp_w2"]).astype(np.float16)),
        "sradh": np.ascontiguousarray(
            np.asarray(layers[L - 1]["sr"]["adapter"]).astype(np.float16)),
        "srwvh": np.ascontiguousarray(
            np.asarray(layers[L - 1]["sr"]["attn_wv"]).astype(np.float16)),
        "srwoh": np.ascontiguousarray(
            np.asarray(layers[L - 1]["sr"]["attn_wo"]).astype(np.float16)),
        "srm1h": np.ascontiguousarray(
            np.asarray(layers[L - 1]["sr"]["mlp_w1"]).astype(np.float16)),
        "srm2h": np.ascontiguousarray(
            np.asarray(layers[L - 1]["sr"]["mlp_w2"]).astype(np.float16)),
    }
    wq = f32(np.stack([lp["attn_wq"] for lp in layers]))
    wk = f32(np.stack([lp["attn_wk"] for lp in layers]))
    wv = f32(np.stack([lp["attn_wv"] for lp in layers]))

    in_maps = []
    for c in range(NCORES):
        rows = np.concatenate([np.arange(b_ * 128, b_ * 128 + 128)
                               for b_ in CORE_BLOCKS[c]])
        im = dict(shared)
        im["x0T"] = np.ascontiguousarray(x0[rows].T)
        im["s0T"] = np.ascontiguousarray(
            np.stack([s0[li][rows].T for li in range(L)]))
        hs = slice(c * HD, (c + 1) * HD)
        im["wqh"] = np.ascontiguousarray(wq[:, :, hs])
        im["wkh"] = np.ascontiguousarray(wk[:, :, hs])
        im["wvh"] = np.ascontiguousarray(wv[:, :, hs])
        in_maps.append(im)

    res = bass_utils.run_bass_kernel_spmd(
        nc, in_maps, core_ids=list(range(NCORES)))
    LAST_RESULT = res

    logits = np.zeros((T, V), dtype=np.float32)
    fsum = np.zeros((L, E), dtype=np.float64)
    psum = np.zeros((L, E), dtype=np.float64)
    for c in range(NCORES):
        o = res.results[c]
        lg = o["logits"]
        for j, b_ in enumerate(CORE_BLOCKS[c]):
            logits[b_ * 128:(b_ + 1) * 128] = lg[j * 128:(j + 1) * 128]
        fsum += o["aux"][:, 0, :]
        psum += o["aux"][:, 1, :]

    total_aux = np.float32(0.0)
    for li in range(L):
        f = (fsum[li] / T).astype(np.float32)
        pr = (psum[li] / T).astype(np.float32)
        total_aux = np.float32(total_aux + np.float32(E) *
                               np.float32(np.sum(f * pr, dtype=np.float32)))
    return logits.reshape(B, S, V), total_aux
